# revision 1
# baseline (speedup 1.0000x reference)
"""Trainium2 Bass kernel for a causal multi-head attention block
(fused QKV proj + RoPE + causal softmax attention + out proj).

Sharding: 8 cores = 4 batches x 2 head-groups (8 heads each), no
on-chip collectives: each core emits a partial out-projection [N, C]
(row-parallel over heads); the host sums each batch's pair of partials
and adds the output bias.

Per-core pipeline (B=1 batch, 8 heads, N=2048, C=1024, D=64):
  - Phase 1 (jt-outer): qkT[j, n] = (Wqk x^T) in bf16, j head-major with
    a host-side deinterleave permutation (even RoPE components at
    d'=0..31, odd at 32..63) and q pre-scaled by D^-0.5; RoPE runs on
    DVE right after each tile (swap via partition-base-shifted reads
    against a sign-block-swapped sin table; all bf16 for 2x DVE mode).
    v stays natural [n, hd] (bf16) with a ones column per head (65-wide
    groups); v bias added via a K=1 matmul of ones x bv.
  - Phase 3 (nb / head-pair / k-tile, software-pipelined): scores
    S^T[k, q] row-packed for both heads of a pair into one [128,1024]
    PSUM tile (lhsT base partitions 0/64 -> concurrent PE sub-arrays),
    causal-masked on the diagonal 128-block, one wide exp (ScalarE)
    -> bf16 P^T; P^T @ [v|1] accumulates o^T and the softmax
    denominator Z per head; evacuation normalizes by 1/Z (DVE recip +
    K=1 ones-matmul broadcast + DVE multiply).
  - Phase 4 (inline per q-block): out partial [n, c] = oT.T @ outwT in
    f32r, staged through SBUF, overlapping the next block's attention.
Scores/PV run in bf16, projections in f32r (full-rate fp32 path);
measured end-to-end rel err vs the fp32 reference ~7e-3.
"""

import sys

sys.path.insert(0, "/opt/trn_rl_repo")

import numpy as np

import concourse.bass as bass
import concourse.mybir as mybir
from concourse import bacc, library_config
from concourse.tile import TileContext

F32 = mybir.dt.float32
F32R = mybir.dt.float32r
BF16 = mybir.dt.bfloat16

B, N, C = 4, 2048, 1024
H_ALL, D = 16, 64
HPC = 8  # heads per core
JQK = HPC * D  # 512 rows for q (and k) per core
ROPE_THETA = 10000.0
SCALE = D**-0.5
NEG = -1e9

NT = N // 128  # 16 n-tiles
NB = N // 512  # 4 n-blocks
CC = C // 128  # 8 contraction chunks


def r(ap):
    return ap.bitcast(F32R)


def build_nc(reps=1):
    nc = bacc.Bacc(None, target_bir_lowering=False)

    xt = nc.declare_dram_parameter("xt", [CC, 128, N], BF16, isOutput=False)
    wt = nc.declare_dram_parameter("wt", [CC, 128, 1536], BF16, isOutput=False)
    bqk = nc.declare_dram_parameter("bqk", [128, 8], F32, isOutput=False)
    bv = nc.declare_dram_parameter("bv", [1, JQK], F32R, isOutput=False)
    cosb = nc.declare_dram_parameter("cosb", [128, N], BF16, isOutput=False)
    sinb = nc.declare_dram_parameter("sinb", [128, N], BF16, isOutput=False)
    maskp = nc.declare_dram_parameter("maskp", [128, 128], F32, isOutput=False)
    owt = nc.declare_dram_parameter("owt", [4, 128, C], F32R, isOutput=False)
    onesp = nc.declare_dram_parameter("onesp", [1, 128], F32R, isOutput=False)
    ones16 = nc.declare_dram_parameter("ones16", [128, 8], BF16, isOutput=False)
    out = nc.declare_dram_parameter("out", [N, C], F32, isOutput=True)

    with TileContext(nc) as tc:
      for _rep in range(reps):
        with tc.tile_pool(name="persist", bufs=1) as pp:
            qkT = [pp.tile([128, N], BF16, tag=f"qkT{t}", name=f"qkT{t}") for t in range(8)]
            vN = [pp.tile([128, HPC * 65], BF16, tag=f"vN{t}", name=f"vN{t}") for t in range(NT)]
            cos_sb = pp.tile([128, N], BF16, tag="cos_sb", name="cos_sb")
            sin_sb = pp.tile([128, N], BF16, tag="sin_sb", name="sin_sb")
            mask_sb = pp.tile([128, 128], F32, tag="mask_sb", name="mask_sb")
            bqk_sb = pp.tile([128, 8], F32, tag="bqk_sb", name="bqk_sb")
            bv_sb = pp.tile([1, JQK], F32R, tag="bv_sb", name="bv_sb")
            ones_sb = pp.tile([1, 128], F32R, tag="ones_sb", name="ones_sb")
            ones16_sb = pp.tile([128, 8], BF16, tag="ones16_sb", name="ones16_sb")

            # ========== Phase 1: QKV projection + RoPE (interleaved) ========
            # jt-outer so each q/k tile finishes early; RoPE (pure DVE,
            # partition-base-shifted reads) follows its tile immediately and
            # hides under the remaining projection matmuls.
            with (
                tc.tile_pool(name="wpool", bufs=1) as wp,
                tc.tile_pool(name="xpool", bufs=1) as xp,
                tc.tile_pool(name="rope", bufs=2) as rp,
                tc.tile_pool(name="ppsum", bufs=4, space="PSUM") as pqk,
            ):
                xts = [xp.tile([128, N], BF16, tag=f"xt{cch}", name=f"xt{cch}") for cch in range(CC)]
                wt_sb = [wp.tile([128, 1536], BF16, tag=f"wt{cch}", name=f"wt{cch}") for cch in range(CC)]
                # all input DMAs on the SP queue (ACT queue stays free for
                # evacuations/exp), ordered by first consumption: chunk 0 for
                # the first accumulation chain, rope tables next, then the
                # remaining chunks
                nc.sync.dma_start(out=wt_sb[0][:, :], in_=wt[0, :, :])
                nc.sync.dma_start(out=xts[0][:, :], in_=xt[0, :, :])
                nc.sync.dma_start(out=cos_sb[:, :], in_=cosb[:, :])
                nc.sync.dma_start(out=sin_sb[:, :], in_=sinb[:, :])
                nc.sync.dma_start(out=bqk_sb[:, :], in_=bqk[:, :])
                nc.sync.dma_start(out=mask_sb[:, :], in_=maskp[:, :])
                nc.sync.dma_start(out=bv_sb[:, :], in_=bv[:, :])
                nc.sync.dma_start(out=ones_sb[:, :], in_=onesp[:, :])
                nc.sync.dma_start(out=ones16_sb[:, :], in_=ones16[:, :])
                for cch in range(1, CC):
                    nc.sync.dma_start(out=wt_sb[cch][:, :], in_=wt[cch, :, :])
                    nc.sync.dma_start(out=xts[cch][:, :], in_=xt[cch, :, :])

                def rope(t):
                    # r[a] = q[a]*cos[a] + q[a^1]*sinSigned[a] per 32-block
                    sw = rp.tile([128, N], BF16, tag="sw", name=f"sw{t}", bufs=2)
                    for a in range(4):
                        b = (a ^ 1) * 32
                        nc.vector.tensor_mul(
                            sw[a * 32 : a * 32 + 32, :],
                            qkT[t][b : b + 32, :],
                            sin_sb[b : b + 32, :],
                        )
                    nc.vector.tensor_mul(
                        qkT[t][:, :], qkT[t][:, :], cos_sb[:, :]
                    )
                    nc.vector.tensor_add(
                        qkT[t][:, :], qkT[t][:, :], sw[:, :]
                    )

                # q,k: transposed layout, per j-tile then rope; order
                # 0,4,1,5,... so attention pair hp gets its q (hp) and k
                # (4+hp) tiles rope'd in consumption order
                for jt in [0, 4, 1, 5, 2, 6, 3, 7]:
                    for nb in range(NB):
                        ps = pqk.tile([128, 512], F32, tag="ps_qk", name=f"psqk_{jt}_{nb}")
                        for cch in range(CC):
                            nc.tensor.matmul(
                                ps[:, :],
                                wt_sb[cch][:, jt * 128 : (jt + 1) * 128],
                                xts[cch][:, nb * 512 : (nb + 1) * 512],
                                start=(cch == 0),
                                stop=(cch == CC - 1),
                            )
                        nc.vector.tensor_scalar_add(
                            out=qkT[jt][:, nb * 512 : (nb + 1) * 512],
                            in0=ps[:, :],
                            scalar1=bqk_sb[:, jt : jt + 1],
                        )
                    rope(jt)

                # gpsimd: load the 'attn' ucode library (partition_broadcast)
                # before any custom gpsimd op; same queue => ordered
                nc.gpsimd.load_library(library_config.attn)
                # ones column (col 64 of each head group) - needed by PV
                # only; on gpsimd so it cannot block the DVE stream
                for t in range(NT):
                    nc.gpsimd.tensor_copy(
                        out=vN[t][:, 64 : HPC * 65 : 65], in_=ones16_sb[:, :]
                    )

                # v: natural layout, evacuated on ACT (idle in phase 1)
                for t in range(NT):
                    psv = pqk.tile([128, 512], F32, tag="ps_v", name=f"psv_{t}")
                    for cch in range(CC):
                        nc.tensor.matmul(
                            psv[:, :],
                            xts[cch][:, t * 128 : (t + 1) * 128],
                            wt_sb[cch][:, 1024:1536],
                            start=(cch == 0),
                            stop=False,
                        )
                    nc.tensor.matmul(
                        psv[:, :],
                        r(ones_sb[:, 0:128]),
                        r(bv_sb[:, :]),
                        start=False,
                        stop=True,
                    )
                    nc.scalar.copy(
                        vN[t].rearrange("p (h e) -> p h e", e=65)[:, :, 0:64],
                        psv[:, :].rearrange("p (h d) -> p h d", d=64),
                    )

            # oT + out-proj SBUF allocated after phase-1 pools close
            with (
                tc.tile_pool(name="opool", bufs=1) as opl,
                tc.tile_pool(name="owpool", bufs=1) as owp,
                tc.tile_pool(name="ostage", bufs=4) as osg,
            ):
                oT = [opl.tile([128, N], F32R, tag=f"oT{t}", name=f"oT{t}") for t in range(4)]
                owt_sb = [owp.tile([128, C], F32R, tag=f"owt{hc}", name=f"owt{hc}") for hc in range(4)]
                for hc in range(4):
                    nc.sync.dma_start(out=owt_sb[hc][:, :], in_=owt[hc, :, :])

                # ========== Phases 3+4: attention with inline out-proj ======
                # nb-outer / head-pair / k-tile-inner. Scores for the two
                # heads of a pair are row-packed (lhsT base partitions 0/64
                # -> concurrent PE sub-arrays) into one [128,1024] PSUM
                # tile; one wide exp covers both heads. P^T @ [v|1]
                # accumulates o^T and Z per head; evacuation normalizes by
                # 1/Z (DVE recip + PE ones-broadcast + DVE multiply). The
                # out-proj for each finished q-block overlaps the next
                # block's attention.
                with (
                    tc.tile_pool(name="attn_ps", bufs=2, space="PSUM") as sp,
                    tc.tile_pool(name="o_ps", bufs=3, space="PSUM") as op,
                    tc.tile_pool(name="pt_pool", bufs=6) as ptp,
                    tc.tile_pool(name="znorm", bufs=4) as zp,
                ):
                    for nb in range(NB):
                        for hp in range(4):
                            o_ps = [
                                op.tile([65, 512], F32, tag="o", name=f"o_{nb}_{hp}_{hh}")
                                for hh in range(2)
                            ]
                            # software-pipelined: scores/exp for j+1 are
                            # emitted before PV of j, so the PE stream never
                            # stalls waiting for ACT's exp
                            pend = None  # (j, pt, off2, w, ooff)
                            for j in range(4 * nb + 4):
                                if j // 4 == nb:
                                    qoff = j * 128
                                    w = 512 * (nb + 1) - qoff
                                else:
                                    qoff, w = nb * 512, 512
                                # scores always full 512 wide: clamp the window
                                # base so every PSUM byte exp reads is written;
                                # cols below qoff are computed-but-unread
                                qbase = min(qoff, N - 512)
                                off2 = qoff - qbase
                                ooff = qoff - 512 * nb
                                st = sp.tile([128, 1024], F32, tag="st", name=f"st_{nb}_{hp}_{j}")
                                for hh in range(2):
                                    nc.tensor.matmul(
                                        st[:, hh * 512 : hh * 512 + 512],
                                        qkT[4 + hp][hh * 64 : hh * 64 + 64, j * 128 : (j + 1) * 128],
                                        qkT[hp][hh * 64 : hh * 64 + 64, qbase : qbase + 512],
                                        start=True,
                                        stop=True,
                                    )
                                if j // 4 == nb:
                                    diag = st[:, 0:1024].rearrange("p (b q) -> p b q", b=2)[:, :, off2 : off2 + 128]
                                    nc.vector.tensor_add(
                                        diag,
                                        diag,
                                        mask_sb[:, None, :].broadcast_to([128, 2, 128]),
                                    )
                                pt = ptp.tile([128, 1024], BF16, tag="pt", name=f"pt_{nb}_{hp}_{j}")
                                if off2:
                                    # partial tile: exp only the causal range
                                    # of each head's half (strided 2-block AP)
                                    nc.scalar.activation(
                                        pt.rearrange("p (b q) -> p b q", b=2)[:, :, off2:512],
                                        st[:, 0:1024].rearrange("p (b q) -> p b q", b=2)[:, :, off2:512],
                                        mybir.ActivationFunctionType.Exp,
                                    )
                                else:
                                    nc.scalar.activation(
                                        pt[:, :],
                                        st[:, :],
                                        mybir.ActivationFunctionType.Exp,
                                    )
                                if pend is not None:
                                    pj, ppt, poff2, pw, pooff = pend
                                    for hh in range(2):
                                        h = 2 * hp + hh
                                        nc.tensor.matmul(
                                            o_ps[hh][:, pooff : pooff + pw],
                                            vN[pj][:, h * 65 : h * 65 + 65],
                                            ppt[:, hh * 512 + poff2 : hh * 512 + poff2 + pw],
                                            start=(pj == 0),
                                            stop=False,
                                            skip_group_check=True,
                                        )
                                pend = (j, pt, off2, w, ooff)
                            pj, ppt, poff2, pw, pooff = pend
                            for hh in range(2):
                                h = 2 * hp + hh
                                nc.tensor.matmul(
                                    o_ps[hh][:, pooff : pooff + pw],
                                    vN[pj][:, h * 65 : h * 65 + 65],
                                    ppt[:, hh * 512 + poff2 : hh * 512 + poff2 + pw],
                                    start=(pj == 0),
                                    stop=True,
                                    skip_group_check=True,
                                )
                            for hh in range(2):
                                h = 2 * hp + hh
                                half = hh * 64
                                rz = zp.tile([1, 512], F32R, tag="rz", name=f"rz_{nb}_{h}")
                                with nc.allow_low_precision(reason="f32r recip feeds broadcast matmul"):
                                    nc.vector.reciprocal(rz[:, :], o_ps[hh][64:65, :])
                                bc = sp.tile([64, 512], F32, tag="pso", name=f"bc_{nb}_{h}", bufs=1)
                                nc.tensor.matmul(
                                    bc[:, :],
                                    r(ones_sb[:, 0:64]),
                                    r(rz[:, :]),
                                    start=True,
                                    stop=True,
                                )
                                rzb = zp.tile([64, 512], F32, tag="rzb", name=f"rzb_{nb}_{h}")
                                nc.vector.tensor_copy(out=rzb[:, :], in_=bc[:, :])
                                nc.vector.tensor_mul(
                                    oT[hp][half : half + 64, nb * 512 : (nb + 1) * 512],
                                    o_ps[hh][0:64, :],
                                    rzb[:, :],
                                )

                        # out-proj for this q-block; shares the bc PSUM slot
                        for i in range(4 * nb, 4 * nb + 4):
                            for cb in range(2):
                                pso = sp.tile([128, 512], F32, tag="pso", name=f"pso_{i}_{cb}", bufs=1)
                                for hc in range(4):
                                    nc.tensor.matmul(
                                        pso[:, :],
                                        r(oT[hc][:, i * 128 : (i + 1) * 128]),
                                        r(owt_sb[hc][:, cb * 512 : (cb + 1) * 512]),
                                        start=(hc == 0),
                                        stop=(hc == 3),
                                    )
                                ost = osg.tile([128, 512], F32, tag="ost", name=f"ost_{i}_{cb}")
                                nc.vector.tensor_copy(out=ost[:, :], in_=pso[:, :])
                                nc.sync.dma_start(
                                    out=out[i * 128 : (i + 1) * 128, cb * 512 : (cb + 1) * 512],
                                    in_=ost[:, :],
                                )
    nc.compile()
    return nc


def make_in_maps(x, Wqkv_w, Wqkv_b, out_w):
    """Host-side sharding/layout prep. Returns per-core input dicts."""
    in_maps = []
    # deinterleave perm within one head: even rope components then odd
    perm = np.concatenate([np.arange(0, D, 2), np.arange(1, D, 2)])
    # rope tables
    inv = 1.0 / (ROPE_THETA ** (np.arange(0, D, 2, dtype=np.float64) / D))
    ang = np.arange(N, dtype=np.float64)[:, None] * inv[None, :]  # [N, 32]
    cosT = np.cos(ang).T.astype(np.float32)  # [32, N]
    sinT = np.sin(ang).T.astype(np.float32)
    cosb = np.tile(cosT, (4, 1))  # [128, N]
    sinb = np.concatenate([sinT, -sinT, sinT, -sinT], axis=0)  # [128, N], block a holds out-block a^1's signed sin
    qc, kc = np.arange(128), np.arange(128)
    maskp = np.where(qc[None, :] >= kc[:, None], 0.0, NEG).astype(np.float32)

    for c in range(8):
        b, g = c // 2, c % 2
        heads = np.arange(g * HPC, (g + 1) * HPC)
        qk_rows = (heads[:, None] * D + perm[None, :]).reshape(-1)  # [512]
        v_rows = (heads[:, None] * D + np.arange(D)[None, :]).reshape(-1)
        Wq = Wqkv_w[qk_rows] * SCALE
        bq = Wqkv_b[qk_rows] * SCALE
        Wk = Wqkv_w[C + qk_rows]
        bk = Wqkv_b[C + qk_rows]
        Wv = Wqkv_w[2 * C + v_rows]
        bv = Wqkv_b[2 * C + v_rows]
        Wcat = np.concatenate([Wq, Wk, Wv], axis=0)  # [1536, C]
        wt = np.ascontiguousarray(Wcat.T).reshape(CC, 128, 1536)
        xt = np.ascontiguousarray(x[b].T).reshape(CC, 128, N)
        bqk = np.ascontiguousarray(
            np.concatenate([bq, bk]).reshape(8, 128).T
        )  # [128, 8]
        owt = np.ascontiguousarray(out_w[:, g * JQK : (g + 1) * JQK].T).reshape(
            4, 128, C
        )
        import ml_dtypes
        in_maps.append(
            dict(
                onesp=np.ones((1, 128), dtype=np.float32),
                ones16=np.ones((128, 8), dtype=ml_dtypes.bfloat16),
                xt=xt.astype(ml_dtypes.bfloat16),
                wt=wt.astype(ml_dtypes.bfloat16),
                bqk=bqk.astype(np.float32),
                bv=np.ascontiguousarray(bv[None, :]).astype(np.float32),
                cosb=cosb.astype(ml_dtypes.bfloat16),
                sinb=sinb.astype(ml_dtypes.bfloat16),
                maskp=maskp,
                owt=owt.astype(np.float32),
            )
        )
    return in_maps


_CACHED_NC = None


def kernel(x, Wqkv_w, Wqkv_b, out_w, out_b):
    from concourse.bass_utils import run_bass_kernel_spmd

    global _CACHED_NC
    x = np.asarray(x, dtype=np.float32)
    Wqkv_w = np.asarray(Wqkv_w, dtype=np.float32)
    Wqkv_b = np.asarray(Wqkv_b, dtype=np.float32)
    out_w = np.asarray(out_w, dtype=np.float32)
    out_b = np.asarray(out_b, dtype=np.float32)

    if _CACHED_NC is None:
        _CACHED_NC = build_nc()
    nc = _CACHED_NC
    in_maps = make_in_maps(x, Wqkv_w, Wqkv_b, out_w)
    res = run_bass_kernel_spmd(nc, in_maps, core_ids=list(range(8)))
    out = np.empty((B, N, C), dtype=np.float32)
    for b in range(B):
        out[b] = res.results[2 * b]["out"] + res.results[2 * b + 1]["out"] + out_b
    return out



# revision 19
# speedup vs baseline: 1.1367x; 1.1367x over previous
"""Trainium2 Bass kernel for a causal multi-head attention block
(fused QKV proj + RoPE + causal softmax attention + out proj).

Sharding: 8 cores = 4 batches x 2 head-groups (8 heads each), no
on-chip collectives: each core emits a partial out-projection [N, C]
(row-parallel over heads); the host sums each batch's pair of partials
and adds the output bias.

Schedule (single fused stream, PE never phase-barriers):
  - Prefix: chunk-major projection of q0/k0 (pair 0) across 8 PSUM
    banks while the wt/xt chunks stream in, then v tiles 0-3.
  - Attention runs head-pair-outer / q-block-inner. All remaining
    projection work (v4-15, q/k pairs 1-3, their RoPE) lives in a fill
    queue drained one item per k-tile iteration, so the PE pipeline
    stays dense while ACT's exp stream (the per-iteration clock) runs.
  - Scores S^T[k, q] for both heads of a pair row-packed into one
    [128,1024] PSUM tile; causal-trimmed on diagonal tiles; one wide
    exp -> bf16 P^T; P^T @ [v|1] accumulates o^T and the softmax
    denominator Z per head.
  - Normalization is entirely off the PE path: o_ps evacuates to SBUF
    (bf16) immediately (PSUM recycles in <1us), then DVE recip ->
    gpsimd partition-broadcast -> DVE multiply produce oT in bf16.
  - Out-proj (bf16) for q-block nb is enqueued as fill during the
    last head-pair, one block behind its norm, and drains at the tail.
Scores/PV/projections in bf16 (f32 PSUM accumulation); v-bias via a
K=1 ones-matmul; q pre-scaled by D^-0.5 on the host.
"""

import sys

sys.path.insert(0, "/opt/trn_rl_repo")

import numpy as np

import concourse.bass as bass
import concourse.mybir as mybir
from concourse import bacc, library_config
from concourse.tile import TileContext

F32 = mybir.dt.float32
F32R = mybir.dt.float32r
BF16 = mybir.dt.bfloat16

B, N, C = 4, 2048, 1024
H_ALL, D = 16, 64
HPC = 8  # heads per core
JQK = HPC * D  # 512 rows for q (and k) per core
ROPE_THETA = 10000.0
SCALE = D**-0.5
NEG = -1e9

NT = N // 128  # 16 n-tiles
NB = N // 512  # 4 n-blocks
CC = C // 128  # 8 contraction chunks


def r(ap):
    return ap.bitcast(F32R)


PE_LABELS = []
_CUR = ["?"]


def _lbl(s):
    _CUR[0] = s


def build_nc(reps=1):
    PE_LABELS.clear()
    nc = bacc.Bacc(None, target_bir_lowering=False)
    _orig_mm = nc.tensor.matmul

    def _mm(*a, **k):
        PE_LABELS.append(_CUR[0])
        return _orig_mm(*a, **k)

    nc.tensor.matmul = _mm

    xt = nc.declare_dram_parameter("xt", [CC, 128, N], BF16, isOutput=False)
    wt = nc.declare_dram_parameter("wt", [CC, 128, 1536], BF16, isOutput=False)
    bqk = nc.declare_dram_parameter("bqk", [128, 8], F32, isOutput=False)
    bv = nc.declare_dram_parameter("bv", [1, JQK], F32R, isOutput=False)
    cosb = nc.declare_dram_parameter("cosb", [128, N], BF16, isOutput=False)
    sinb = nc.declare_dram_parameter("sinb", [128, N], BF16, isOutput=False)
    maskb = nc.declare_dram_parameter("maskb", [128, 128], BF16, isOutput=False)
    identb = nc.declare_dram_parameter("identb", [128, 128], BF16, isOutput=False)
    owt = nc.declare_dram_parameter("owt", [4, 128, C], BF16, isOutput=False)
    onesp = nc.declare_dram_parameter("onesp", [1, 128], F32R, isOutput=False)
    ones16 = nc.declare_dram_parameter("ones16", [128, 8], BF16, isOutput=False)
    out = nc.declare_dram_parameter("out", [N, C], F32, isOutput=True)

    with TileContext(nc) as tc:
      for _rep in range(reps):
        with tc.tile_pool(name="persist", bufs=1) as pp:
            qkT = [pp.tile([128, N], BF16, tag=f"qkT{t}", name=f"qkT{t}") for t in range(8)]
            vN = [pp.tile([128, HPC * 65], BF16, tag=f"vN{t}", name=f"vN{t}") for t in range(NT)]
            oT = [pp.tile([128, N], BF16, tag=f"oT{t}", name=f"oT{t}") for t in range(4)]
            owt_sb = [pp.tile([128, C], BF16, tag=f"owt{hc}", name=f"owt{hc}") for hc in range(4)]
            cos_sb = pp.tile([128, N], BF16, tag="cos_sb", name="cos_sb")
            sin_sb = pp.tile([128, N], BF16, tag="sin_sb", name="sin_sb")
            mask_sb = pp.tile([128, 128], BF16, tag="mask_sb", name="mask_sb")
            ident_sb = pp.tile([128, 128], BF16, tag="ident_sb", name="ident_sb")
            bqk_sb = pp.tile([128, 8], F32, tag="bqk_sb", name="bqk_sb")
            bv_sb = pp.tile([1, JQK], F32R, tag="bv_sb", name="bv_sb")
            ones_sb = pp.tile([1, 128], F32R, tag="ones_sb", name="ones_sb")
            ones16_sb = pp.tile([128, 8], BF16, tag="ones16_sb", name="ones16_sb")
            xts = [pp.tile([128, N], BF16, tag=f"xt{cch}", name=f"xt{cch}") for cch in range(CC)]
            wt_sb = [pp.tile([128, 1536], BF16, tag=f"wt{cch}", name=f"wt{cch}") for cch in range(CC)]

            # input DMAs in consumption order: per chunk wt then two halves
            # of xt (half pieces advance the chunk-major prefix earlier
            # without blowing the serial HWDGE desc-gen budget); tables
            # after the chunks; owt last (first consumed ~80us in).
            for cch in range(CC):
                nc.sync.dma_start(out=wt_sb[cch][:, :], in_=wt[cch, :, :])
                for nbp in range(2):
                    nc.sync.dma_start(
                        out=xts[cch][:, nbp * 1024 : (nbp + 1) * 1024],
                        in_=xt[cch, :, nbp * 1024 : (nbp + 1) * 1024],
                    )
                if cch == 1:
                    nc.sync.dma_start(out=bqk_sb[:, :], in_=bqk[:, :])
                    nc.sync.dma_start(out=ones16_sb[:, :], in_=ones16[:, :])
                    nc.sync.dma_start(out=bv_sb[:, :], in_=bv[:, :])
                    nc.sync.dma_start(out=ones_sb[:, :], in_=onesp[:, :])
            nc.sync.dma_start(out=cos_sb[:, :], in_=cosb[:, :])
            nc.sync.dma_start(out=sin_sb[:, :], in_=sinb[:, :])
            nc.sync.dma_start(out=mask_sb[:, :], in_=maskb[:, :])
            nc.sync.dma_start(out=ident_sb[:, :], in_=identb[:, :])
            for hc in range(4):
                nc.sync.dma_start(out=owt_sb[hc][:, :], in_=owt[hc, :, :])

            # gpsimd: library + the ones column (col 64 of each head group)
            # for every v tile - independent of the v projections
            nc.gpsimd.load_library(library_config.attn)
            for t in range(NT):
                nc.gpsimd.tensor_copy(
                    out=vN[t][:, 64 : HPC * 65 : 65], in_=ones16_sb[:, :]
                )

            with tc.tile_pool(name="rope", bufs=2) as rp:
                sw_cache = {}

                def get_sw(jt):
                    # one sw tile per jt, shared by its rope block-items;
                    # 2 rotating buffers (jt usage windows are sequential)
                    if jt not in sw_cache:
                        sw_cache[jt] = rp.tile(
                            [128, N], BF16, tag="swf", name=f"swf{jt}", bufs=2
                        )
                    return sw_cache[jt]

                def rope_block(jt, nbp):
                    # r[a] = q[a]*cos[a] + q[a^1]*sinSigned[a] per 32-block,
                    # applied to one 512-wide n-block so the first consumer
                    # never waits on a full-row DVE chain
                    sw = get_sw(jt)
                    s = slice(nbp * 512, (nbp + 1) * 512)
                    for a in range(4):
                        b = (a ^ 1) * 32
                        nc.vector.tensor_mul(
                            sw[a * 32 : a * 32 + 32, s],
                            qkT[jt][b : b + 32, s],
                            sin_sb[b : b + 32, s],
                        )
                    nc.vector.tensor_mul(qkT[jt][:, s], qkT[jt][:, s], cos_sb[:, s])
                    nc.vector.tensor_add(qkT[jt][:, s], qkT[jt][:, s], sw[:, s])

                # ---- prefix: pair 0 (q=jt0, k=jt4) chunk-major across 8
                # PSUM banks so PE tracks the chunk DMA stream ----
                with tc.tile_pool(name="prefix_ps", bufs=1, space="PSUM") as pfx:
                    pf = {
                        (jt, nbp): pfx.tile(
                            [128, 512], F32, tag=f"pf{jt}_{nbp}", name=f"pf{jt}_{nbp}"
                        )
                        for jt in (0, 4)
                        for nbp in range(NB)
                    }
                    _lbl("prefix")
                    for cch in range(CC):
                        for nbp in range(NB):
                            for jt in (0, 4):
                                nc.tensor.matmul(
                                    pf[(jt, nbp)][:, :],
                                    wt_sb[cch][:, jt * 128 : (jt + 1) * 128],
                                    xts[cch][:, nbp * 512 : (nbp + 1) * 512],
                                    start=(cch == 0),
                                    stop=(cch == CC - 1),
                                )
                    # all evacs first (each frees a PSUM bank; keeps the DVE
                    # queue short ahead of the v evacuations), then only the
                    # nb0 rope blocks -- the rest run after v0-3 below
                    for nbp in range(NB):
                        for jt in (0, 4):
                            nc.vector.tensor_scalar_add(
                                out=qkT[jt][:, nbp * 512 : (nbp + 1) * 512],
                                in0=pf[(jt, nbp)][:, :],
                                scalar1=bqk_sb[:, jt : jt + 1],
                            )
                    rope_block(0, 0)
                    rope_block(4, 0)

                # ---- fused attention + fill stream ----
                with (
                    tc.tile_pool(name="attn_ps", bufs=2, space="PSUM") as sp,
                    tc.tile_pool(name="o_ps", bufs=2, space="PSUM") as op,
                    tc.tile_pool(name="fill_ps", bufs=2, space="PSUM") as fp,
                    tc.tile_pool(name="pt_pool", bufs=6) as ptp,
                    tc.tile_pool(name="znorm", bufs=4) as zp,
                    tc.tile_pool(name="osb_pool", bufs=4) as obp,
                    tc.tile_pool(name="ostage", bufs=4) as osg,
                ):
                    # ---------------- fill queue machinery ----------------
                    def emit_v(t):
                        _lbl(f"fill_v{t}")
                        psv = fp.tile([128, 512], F32, tag="fill", name=f"psv_{t}")
                        for cch in range(CC):
                            nc.tensor.matmul(
                                psv[:, :],
                                xts[cch][:, t * 128 : (t + 1) * 128],
                                wt_sb[cch][:, 1024:1536],
                                start=(cch == 0),
                                stop=False,
                            )
                        nc.tensor.matmul(
                            psv[:, :],
                            r(ones_sb[:, 0:128]),
                            r(bv_sb[:, :]),
                            start=False,
                            stop=True,
                        )
                        nc.vector.tensor_copy(
                            out=vN[t].rearrange("p (h e) -> p h e", e=65)[:, :, 0:64],
                            in_=psv[:, :].rearrange("p (h d) -> p h d", d=64),
                        )

                    def emit_qk(jt, nbp):
                        _lbl(f"fill_qk{jt}_{nbp}")
                        ps = fp.tile([128, 512], F32, tag="fill", name=f"psqk_{jt}_{nbp}")
                        for cch in range(CC):
                            nc.tensor.matmul(
                                ps[:, :],
                                wt_sb[cch][:, jt * 128 : (jt + 1) * 128],
                                xts[cch][:, nbp * 512 : (nbp + 1) * 512],
                                start=(cch == 0),
                                stop=(cch == CC - 1),
                            )
                        nc.vector.tensor_scalar_add(
                            out=qkT[jt][:, nbp * 512 : (nbp + 1) * 512],
                            in0=ps[:, :],
                            scalar1=bqk_sb[:, jt : jt + 1],
                        )

                    def emit_outproj_i(i, cb):
                        _lbl(f"outproj{i}_{cb}")
                        pso = fp.tile([128, 512], F32, tag="fill", name=f"pso_{i}_{cb}")
                        for hc in range(4):
                            nc.tensor.matmul(
                                pso[:, :],
                                oT[hc][:, i * 128 : (i + 1) * 128],
                                owt_sb[hc][:, cb * 512 : (cb + 1) * 512],
                                start=(hc == 0),
                                stop=(hc == 3),
                            )
                        ost = osg.tile([128, 512], F32, tag="ost", name=f"ost_{i}_{cb}")
                        nc.vector.tensor_copy(out=ost[:, :], in_=pso[:, :])
                        nc.sync.dma_start(
                            out=out[i * 128 : (i + 1) * 128, cb * 512 : (cb + 1) * 512],
                            in_=ost[:, :],
                        )

                    # v tiles 0-3 (needed by the first attention block) and
                    # the remaining pair-0 rope blocks run before attention;
                    # v evacs land early in the DVE queue
                    for t in range(4):
                        emit_v(t)
                    for nbp in range(1, NB):
                        rope_block(0, nbp)
                        rope_block(4, nbp)

                    fill = []  # (level, marker_key_or_None, emitfn)
                    for t in range(4, NT):
                        fill.append((0, ("v", t), lambda t=t: emit_v(t)))
                    for p in range(1, 4):
                        # qk chain for one n-block, then its rope right away
                        # (per-block items keep DVE bursts short so the
                        # mask->exp chain is never delayed long); level p-1
                        # paces pair p's chains into head-pair p-1's loop so
                        # late head-pairs keep PE fill against the exp clock
                        for jt in (p, 4 + p):
                            for nbp in range(NB):
                                def qk_and_rope(jt=jt, nbp=nbp):
                                    emit_qk(jt, nbp)
                                    rope_block(jt, nbp)
                                fill.append(
                                    (
                                        p - 1,
                                        ("pair", p) if (jt >= 4 and nbp == NB - 1) else None,
                                        qk_and_rope,
                                    )
                                )

                    state = {"pos": 0}
                    done_markers = set()

                    def drain_one(cap):
                        if state["pos"] < len(fill):
                            lev, key, fn = fill[state["pos"]]
                            if lev > cap:
                                return
                            state["pos"] += 1
                            fn()
                            if key is not None:
                                done_markers.add(key)

                    def drain_until(key):
                        if key in done_markers:
                            return
                        while state["pos"] < len(fill):
                            _lev, k, fn = fill[state["pos"]]
                            state["pos"] += 1
                            fn()
                            if k is not None:
                                done_markers.add(k)
                            if k == key:
                                return

                    # ---------------- attention ----------------
                    for hp in range(4):
                        if hp > 0:
                            drain_until(("pair", hp))
                        for nb in range(NB):
                            if 4 * nb + 3 >= 4:
                                drain_until(("v", 4 * nb + 3))
                            o_ps = [
                                op.tile([65, 512], F32, tag="o", name=f"o_{nb}_{hp}_{hh}")
                                for hh in range(2)
                            ]
                            # software-pipelined: scores/exp for j+1 are
                            # emitted before PV of j so PE never waits on exp
                            pend = None  # (j, pt, off2, w, ooff)
                            for j in range(4 * nb + 4):
                                if j // 4 == nb:
                                    qoff = j * 128
                                    w = 512 * (nb + 1) - qoff
                                else:
                                    qoff, w = nb * 512, 512
                                # diag tiles: score/exp only the causal width
                                # w of each head's half; qbase clamp keeps the
                                # window in-bounds at the tail (nb=3), where
                                # the causal range sits at [off2, off2+w)
                                qbase = min(qoff, N - 512)
                                off2 = qoff - qbase
                                ooff = qoff - 512 * nb
                                st = sp.tile([128, 1024], F32, tag="st", name=f"st_{nb}_{hp}_{j}")
                                _lbl(f"score{hp}_{nb}_{j}")
                                dg = j // 4 == nb
                                for hh in range(2):
                                    nc.tensor.matmul(
                                        st[:, hh * 512 + off2 : hh * 512 + off2 + w],
                                        qkT[4 + hp][hh * 64 : hh * 64 + 64, j * 128 : (j + 1) * 128],
                                        qkT[hp][hh * 64 : hh * 64 + 64, qbase + off2 : qbase + off2 + w],
                                        start=True,
                                        stop=not dg,
                                    )
                                if dg:
                                    # causal mask on PE: accumulate the 0/-1e9
                                    # triangle table through an identity lhsT
                                    # (keeps DVE out of the exp chain)
                                    for hh in range(2):
                                        nc.tensor.matmul(
                                            st[:, hh * 512 + off2 : hh * 512 + off2 + 128],
                                            ident_sb[:, :],
                                            mask_sb[:, :],
                                            start=False,
                                            stop=True,
                                            skip_group_check=True,
                                        )
                                pt = ptp.tile([128, 1024], BF16, tag="pt", name=f"pt_{nb}_{hp}_{j}")
                                if w < 512:
                                    nc.scalar.activation(
                                        pt.rearrange("p (b q) -> p b q", b=2)[:, :, off2 : off2 + w],
                                        st[:, 0:1024].rearrange("p (b q) -> p b q", b=2)[:, :, off2 : off2 + w],
                                        mybir.ActivationFunctionType.Exp,
                                    )
                                else:
                                    nc.scalar.activation(
                                        pt[:, :],
                                        st[:, :],
                                        mybir.ActivationFunctionType.Exp,
                                    )
                                if pend is not None:
                                    pj, ppt, poff2, pw, pooff = pend
                                    _lbl(f"pv{hp}_{nb}_{pj}")
                                    for hh in range(2):
                                        h = 2 * hp + hh
                                        nc.tensor.matmul(
                                            o_ps[hh][:, pooff : pooff + pw],
                                            vN[pj][:, h * 65 : h * 65 + 65],
                                            ppt[:, hh * 512 + poff2 : hh * 512 + poff2 + pw],
                                            start=(pj == 0),
                                            stop=False,
                                            skip_group_check=True,
                                        )
                                pend = (j, pt, off2, w, ooff)
                                drain_one(hp)
                            pj, ppt, poff2, pw, pooff = pend
                            _lbl(f"pvL{hp}_{nb}_{pj}")
                            for hh in range(2):
                                h = 2 * hp + hh
                                nc.tensor.matmul(
                                    o_ps[hh][:, pooff : pooff + pw],
                                    vN[pj][:, h * 65 : h * 65 + 65],
                                    ppt[:, hh * 512 + poff2 : hh * 512 + poff2 + pw],
                                    start=(pj == 0),
                                    stop=True,
                                    skip_group_check=True,
                                )
                            # evacuate o+Z to SBUF right away (PSUM recycles
                            # fast); normalization runs off the PE path:
                            # DVE recip -> gpsimd partition-broadcast -> mul
                            for hh in range(2):
                                h = 2 * hp + hh
                                half = hh * 64
                                osb = obp.tile([65, 512], BF16, tag="osb", name=f"osb_{nb}_{h}")
                                nc.vector.tensor_copy(out=osb[:, :], in_=o_ps[hh][:, :])
                                rz = zp.tile([1, 512], BF16, tag="rz", name=f"rz_{nb}_{h}")
                                with nc.allow_low_precision(reason="bf16 1/Z scale"):
                                    nc.vector.reciprocal(rz[:, :], osb[64:65, :])
                                rzb = zp.tile([64, 512], BF16, tag="rzb", name=f"rzb_{nb}_{h}")
                                nc.gpsimd.partition_broadcast(rzb[:, :], rz[:, :])
                                nc.vector.tensor_mul(
                                    oT[hp][half : half + 64, nb * 512 : (nb + 1) * 512],
                                    osb[0:64, :],
                                    rzb[:, :],
                                )
                            if hp == 3:
                                # out-proj for q-block nb, one block behind
                                for i in range(4 * nb, 4 * nb + 4):
                                    for cb in range(2):
                                        fill.append(
                                            (0, None, lambda i=i, cb=cb: emit_outproj_i(i, cb))
                                        )
                    while state["pos"] < len(fill):
                        drain_one(99)
    nc.compile()
    return nc


def make_in_maps(x, Wqkv_w, Wqkv_b, out_w):
    """Host-side sharding/layout prep. Returns per-core input dicts."""
    in_maps = []
    # deinterleave perm within one head: even rope components then odd
    perm = np.concatenate([np.arange(0, D, 2), np.arange(1, D, 2)])
    # rope tables
    inv = 1.0 / (ROPE_THETA ** (np.arange(0, D, 2, dtype=np.float64) / D))
    ang = np.arange(N, dtype=np.float64)[:, None] * inv[None, :]  # [N, 32]
    cosT = np.cos(ang).T.astype(np.float32)  # [32, N]
    sinT = np.sin(ang).T.astype(np.float32)
    cosb = np.tile(cosT, (4, 1))  # [128, N]
    sinb = np.concatenate([sinT, -sinT, sinT, -sinT], axis=0)  # [128, N], block a holds out-block a^1's signed sin
    qc, kc = np.arange(128), np.arange(128)
    maskp = np.where(qc[None, :] >= kc[:, None], 0.0, NEG).astype(np.float32)
    identp = np.eye(128, dtype=np.float32)

    for c in range(8):
        b, g = c // 2, c % 2
        heads = np.arange(g * HPC, (g + 1) * HPC)
        qk_rows = (heads[:, None] * D + perm[None, :]).reshape(-1)  # [512]
        v_rows = (heads[:, None] * D + np.arange(D)[None, :]).reshape(-1)
        Wq = Wqkv_w[qk_rows] * SCALE
        bq = Wqkv_b[qk_rows] * SCALE
        Wk = Wqkv_w[C + qk_rows]
        bk = Wqkv_b[C + qk_rows]
        Wv = Wqkv_w[2 * C + v_rows]
        bv = Wqkv_b[2 * C + v_rows]
        Wcat = np.concatenate([Wq, Wk, Wv], axis=0)  # [1536, C]
        wt = np.ascontiguousarray(Wcat.T).reshape(CC, 128, 1536)
        xt = np.ascontiguousarray(x[b].T).reshape(CC, 128, N)
        bqk = np.ascontiguousarray(
            np.concatenate([bq, bk]).reshape(8, 128).T
        )  # [128, 8]
        owt = np.ascontiguousarray(out_w[:, g * JQK : (g + 1) * JQK].T).reshape(
            4, 128, C
        )
        import ml_dtypes
        in_maps.append(
            dict(
                onesp=np.ones((1, 128), dtype=np.float32),
                ones16=np.ones((128, 8), dtype=ml_dtypes.bfloat16),
                xt=xt.astype(ml_dtypes.bfloat16),
                wt=wt.astype(ml_dtypes.bfloat16),
                bqk=bqk.astype(np.float32),
                bv=np.ascontiguousarray(bv[None, :]).astype(np.float32),
                cosb=cosb.astype(ml_dtypes.bfloat16),
                sinb=sinb.astype(ml_dtypes.bfloat16),
                maskb=maskp.astype(ml_dtypes.bfloat16),
                identb=identp.astype(ml_dtypes.bfloat16),
                owt=owt.astype(ml_dtypes.bfloat16),
            )
        )
    return in_maps


_CACHED_NC = None


def kernel(x, Wqkv_w, Wqkv_b, out_w, out_b):
    from concourse.bass_utils import run_bass_kernel_spmd

    global _CACHED_NC
    x = np.asarray(x, dtype=np.float32)
    Wqkv_w = np.asarray(Wqkv_w, dtype=np.float32)
    Wqkv_b = np.asarray(Wqkv_b, dtype=np.float32)
    out_w = np.asarray(out_w, dtype=np.float32)
    out_b = np.asarray(out_b, dtype=np.float32)

    if _CACHED_NC is None:
        _CACHED_NC = build_nc()
    nc = _CACHED_NC
    in_maps = make_in_maps(x, Wqkv_w, Wqkv_b, out_w)
    res = run_bass_kernel_spmd(nc, in_maps, core_ids=list(range(8)))
    out = np.empty((B, N, C), dtype=np.float32)
    for b in range(B):
        out[b] = res.results[2 * b]["out"] + res.results[2 * b + 1]["out"] + out_b
    return out


# revision 25
# speedup vs baseline: 1.1988x; 1.0547x over previous
"""Trainium2 Bass kernel for a causal multi-head attention block
(fused QKV proj + RoPE + causal softmax attention + out proj).

Sharding: 8 cores = 4 batches x 2 head-groups (8 heads each), no
on-chip collectives: each core emits a partial out-projection [N, C]
(row-parallel over heads); the host sums each batch's pair of partials
and adds the output bias.

Schedule (single fused stream, PE never phase-barriers):
  - Prefix: chunk-major projection of q0/k0 (pair 0) across 8 PSUM
    banks while the wt/xt chunks stream in, then v tiles 0-3.
  - Attention runs head-pair-outer / q-block-inner. All remaining
    projection work (v4-15, q/k pairs 1-3, their RoPE) lives in a fill
    queue drained one item per k-tile iteration, so the PE pipeline
    stays dense while ACT's exp stream (the per-iteration clock) runs.
  - Scores S^T[k, q] for both heads of a pair row-packed into one
    [128,1024] PSUM tile; causal-trimmed on diagonal tiles; one wide
    exp -> bf16 P^T; P^T @ [v|1] accumulates o^T and the softmax
    denominator Z per head.
  - Normalization is entirely off the PE path: o_ps evacuates to SBUF
    (bf16) immediately (PSUM recycles in <1us), then DVE recip ->
    gpsimd partition-broadcast -> DVE multiply produce oT in bf16.
  - Out-proj (bf16) for q-block nb is enqueued as fill during the
    last head-pair, one block behind its norm, and drains at the tail.
Scores/PV/projections in bf16 (f32 PSUM accumulation); v-bias via a
K=1 ones-matmul; q pre-scaled by D^-0.5 on the host.
"""

import sys

sys.path.insert(0, "/opt/trn_rl_repo")

import numpy as np

import concourse.bass as bass
import concourse.mybir as mybir
from concourse import bacc, library_config
from concourse.tile import TileContext

F32 = mybir.dt.float32
F32R = mybir.dt.float32r
BF16 = mybir.dt.bfloat16

B, N, C = 4, 2048, 1024
H_ALL, D = 16, 64
HPC = 8  # heads per core
JQK = HPC * D  # 512 rows for q (and k) per core
ROPE_THETA = 10000.0
SCALE = D**-0.5
NEG = -1e9

NT = N // 128  # 16 n-tiles
NB = N // 512  # 4 n-blocks
CC = C // 128  # 8 contraction chunks


def r(ap):
    return ap.bitcast(F32R)


PE_LABELS = []
_CUR = ["?"]


def _lbl(s):
    _CUR[0] = s


def build_nc(reps=1):
    PE_LABELS.clear()
    nc = bacc.Bacc(None, target_bir_lowering=False)
    _orig_mm = nc.tensor.matmul

    def _mm(*a, **k):
        PE_LABELS.append(_CUR[0])
        return _orig_mm(*a, **k)

    nc.tensor.matmul = _mm

    xt = nc.declare_dram_parameter("xt", [CC, 128, N], BF16, isOutput=False)
    wt = nc.declare_dram_parameter("wt", [CC, 128, 1536], BF16, isOutput=False)
    bqk = nc.declare_dram_parameter("bqk", [128, 8], F32, isOutput=False)
    bv = nc.declare_dram_parameter("bv", [1, JQK], F32R, isOutput=False)
    cosb = nc.declare_dram_parameter("cosb", [128, N], BF16, isOutput=False)
    sinb = nc.declare_dram_parameter("sinb", [128, N], BF16, isOutput=False)
    maskb = nc.declare_dram_parameter("maskb", [128, 128], BF16, isOutput=False)
    identb = nc.declare_dram_parameter("identb", [128, 128], BF16, isOutput=False)
    owt = nc.declare_dram_parameter("owt", [4, 128, C], BF16, isOutput=False)
    onesp = nc.declare_dram_parameter("onesp", [1, 128], F32R, isOutput=False)
    ones16 = nc.declare_dram_parameter("ones16", [128, 8], BF16, isOutput=False)
    out = nc.declare_dram_parameter("out", [N, C], F32, isOutput=True)

    with TileContext(nc) as tc:
      for _rep in range(reps):
        with tc.tile_pool(name="persist", bufs=1) as pp:
            qkT = [pp.tile([128, N], BF16, tag=f"qkT{t}", name=f"qkT{t}") for t in range(8)]
            vN = [pp.tile([128, HPC * 65], BF16, tag=f"vN{t}", name=f"vN{t}") for t in range(NT)]
            oT = [pp.tile([128, N], BF16, tag=f"oT{t}", name=f"oT{t}") for t in range(4)]
            owt_sb = [pp.tile([128, C], BF16, tag=f"owt{hc}", name=f"owt{hc}") for hc in range(4)]
            cos_sb = pp.tile([128, N], BF16, tag="cos_sb", name="cos_sb")
            sin_sb = pp.tile([128, N], BF16, tag="sin_sb", name="sin_sb")
            mask_sb = pp.tile([128, 128], BF16, tag="mask_sb", name="mask_sb")
            ident_sb = pp.tile([128, 128], BF16, tag="ident_sb", name="ident_sb")
            bqk_sb = pp.tile([128, 8], F32, tag="bqk_sb", name="bqk_sb")
            bv_sb = pp.tile([1, JQK], F32R, tag="bv_sb", name="bv_sb")
            ones_sb = pp.tile([1, 128], F32R, tag="ones_sb", name="ones_sb")
            ones16_sb = pp.tile([128, 8], BF16, tag="ones16_sb", name="ones16_sb")
            xts = [pp.tile([128, N], BF16, tag=f"xt{cch}", name=f"xt{cch}") for cch in range(CC)]
            wt_sb = [pp.tile([128, 1536], BF16, tag=f"wt{cch}", name=f"wt{cch}") for cch in range(CC)]

            # input DMAs in consumption order: per chunk wt then two halves
            # of xt (half pieces advance the chunk-major prefix earlier
            # without blowing the serial HWDGE desc-gen budget); tables
            # after the chunks; owt last (first consumed ~80us in).
            for cch in range(CC):
                nc.sync.dma_start(out=wt_sb[cch][:, :], in_=wt[cch, :, :])
                for nbp in range(2):
                    nc.sync.dma_start(
                        out=xts[cch][:, nbp * 1024 : (nbp + 1) * 1024],
                        in_=xt[cch, :, nbp * 1024 : (nbp + 1) * 1024],
                    )
                if cch == 1:
                    nc.sync.dma_start(out=bqk_sb[:, :], in_=bqk[:, :])
                    nc.sync.dma_start(out=ones16_sb[:, :], in_=ones16[:, :])
                    nc.sync.dma_start(out=bv_sb[:, :], in_=bv[:, :])
                    nc.sync.dma_start(out=ones_sb[:, :], in_=onesp[:, :])
            nc.sync.dma_start(out=cos_sb[:, :], in_=cosb[:, :])
            nc.sync.dma_start(out=sin_sb[:, :], in_=sinb[:, :])
            nc.sync.dma_start(out=mask_sb[:, :], in_=maskb[:, :])
            nc.sync.dma_start(out=ident_sb[:, :], in_=identb[:, :])
            for hc in range(4):
                nc.sync.dma_start(out=owt_sb[hc][:, :], in_=owt[hc, :, :])

            # gpsimd: library + the ones column (col 64 of each head group)
            # for every v tile - independent of the v projections
            nc.gpsimd.load_library(library_config.attn)
            for t in range(NT):
                nc.gpsimd.tensor_copy(
                    out=vN[t][:, 64 : HPC * 65 : 65], in_=ones16_sb[:, :]
                )

            with tc.tile_pool(name="rope", bufs=2) as rp:
                sw_cache = {}

                def get_sw(jt):
                    # one sw tile per jt, shared by its rope block-items;
                    # 2 rotating buffers (jt usage windows are sequential)
                    if jt not in sw_cache:
                        sw_cache[jt] = rp.tile(
                            [128, N], BF16, tag="swf", name=f"swf{jt}", bufs=2
                        )
                    return sw_cache[jt]

                def rope_block(jt, nbp):
                    # r[a] = q[a]*cos[a] + q[a^1]*sinSigned[a] per 32-block,
                    # applied to one 512-wide n-block so the first consumer
                    # never waits on a full-row DVE chain
                    sw = get_sw(jt)
                    s = slice(nbp * 512, (nbp + 1) * 512)
                    for a in range(4):
                        b = (a ^ 1) * 32
                        nc.vector.tensor_mul(
                            sw[a * 32 : a * 32 + 32, s],
                            qkT[jt][b : b + 32, s],
                            sin_sb[b : b + 32, s],
                        )
                    nc.vector.tensor_mul(qkT[jt][:, s], qkT[jt][:, s], cos_sb[:, s])
                    nc.vector.tensor_add(qkT[jt][:, s], qkT[jt][:, s], sw[:, s])

                # ---- prefix: pair 0 (q=jt0, k=jt4) chunk-major across 8
                # PSUM banks so PE tracks the chunk DMA stream ----
                with tc.tile_pool(name="prefix_ps", bufs=1, space="PSUM") as pfx:
                    pf = {
                        (jt, nbp): pfx.tile(
                            [128, 512], F32, tag=f"pf{jt}_{nbp}", name=f"pf{jt}_{nbp}"
                        )
                        for jt in (0, 4)
                        for nbp in range(NB)
                    }
                    _lbl("prefix")
                    for cch in range(CC):
                        for nbp in range(NB):
                            for jt in (0, 4):
                                nc.tensor.matmul(
                                    pf[(jt, nbp)][:, :],
                                    wt_sb[cch][:, jt * 128 : (jt + 1) * 128],
                                    xts[cch][:, nbp * 512 : (nbp + 1) * 512],
                                    start=(cch == 0),
                                    stop=(cch == CC - 1),
                                )
                    # all evacs first (each frees a PSUM bank; keeps the DVE
                    # queue short ahead of the v evacuations), then only the
                    # nb0 rope blocks -- the rest run after v0-3 below
                    for nbp in range(NB):
                        for jt in (0, 4):
                            nc.vector.tensor_scalar_add(
                                out=qkT[jt][:, nbp * 512 : (nbp + 1) * 512],
                                in0=pf[(jt, nbp)][:, :],
                                scalar1=bqk_sb[:, jt : jt + 1],
                            )
                    rope_block(0, 0)
                    rope_block(4, 0)

                # ---- fused attention + fill stream ----
                with (
                    tc.tile_pool(name="attn_ps", bufs=2, space="PSUM") as sp,
                    tc.tile_pool(name="o_ps", bufs=2, space="PSUM") as op,
                    tc.tile_pool(name="fill_ps", bufs=2, space="PSUM") as fp,
                    tc.tile_pool(name="pt_pool", bufs=6) as ptp,
                    tc.tile_pool(name="znorm", bufs=4) as zp,
                    tc.tile_pool(name="onsb_pool", bufs=8) as obp,
                    tc.tile_pool(name="ostage", bufs=4) as osg,
                ):
                    # ---------------- fill queue machinery ----------------
                    def emit_v(t):
                        _lbl(f"fill_v{t}")
                        psv = fp.tile([128, 512], F32, tag="fill", name=f"psv_{t}")
                        for cch in range(CC):
                            nc.tensor.matmul(
                                psv[:, :],
                                xts[cch][:, t * 128 : (t + 1) * 128],
                                wt_sb[cch][:, 1024:1536],
                                start=(cch == 0),
                                stop=False,
                            )
                        nc.tensor.matmul(
                            psv[:, :],
                            r(ones_sb[:, 0:128]),
                            r(bv_sb[:, :]),
                            start=False,
                            stop=True,
                        )
                        nc.vector.tensor_copy(
                            out=vN[t].rearrange("p (h e) -> p h e", e=65)[:, :, 0:64],
                            in_=psv[:, :].rearrange("p (h d) -> p h d", d=64),
                        )

                    def emit_qk(jt, nbp):
                        _lbl(f"fill_qk{jt}_{nbp}")
                        ps = fp.tile([128, 512], F32, tag="fill", name=f"psqk_{jt}_{nbp}")
                        for cch in range(CC):
                            nc.tensor.matmul(
                                ps[:, :],
                                wt_sb[cch][:, jt * 128 : (jt + 1) * 128],
                                xts[cch][:, nbp * 512 : (nbp + 1) * 512],
                                start=(cch == 0),
                                stop=(cch == CC - 1),
                            )
                        nc.vector.tensor_scalar_add(
                            out=qkT[jt][:, nbp * 512 : (nbp + 1) * 512],
                            in0=ps[:, :],
                            scalar1=bqk_sb[:, jt : jt + 1],
                        )

                    def emit_outproj_i(i, cb):
                        _lbl(f"outproj{i}_{cb}")
                        pso = fp.tile([128, 512], F32, tag="fill", name=f"pso_{i}_{cb}")
                        for hc in range(4):
                            nc.tensor.matmul(
                                pso[:, :],
                                oT[hc][:, i * 128 : (i + 1) * 128],
                                owt_sb[hc][:, cb * 512 : (cb + 1) * 512],
                                start=(hc == 0),
                                stop=(hc == 3),
                            )
                        ost = osg.tile([128, 512], F32, tag="ost", name=f"ost_{i}_{cb}")
                        nc.vector.tensor_copy(out=ost[:, :], in_=pso[:, :])
                        nc.sync.dma_start(
                            out=out[i * 128 : (i + 1) * 128, cb * 512 : (cb + 1) * 512],
                            in_=ost[:, :],
                        )

                    # v tiles 0-3 (needed by the first attention block) and
                    # the remaining pair-0 rope blocks run before attention;
                    # v evacs land early in the DVE queue
                    for t in range(4):
                        emit_v(t)
                    for nbp in range(1, NB):
                        rope_block(0, nbp)
                        rope_block(4, nbp)

                    fill = []  # (level, marker_key_or_None, emitfn)
                    for t in range(4, NT):
                        fill.append((0, ("v", t), lambda t=t: emit_v(t)))
                    for p in range(1, 4):
                        # qk chain for one n-block, then its rope right away
                        # (per-block items keep DVE bursts short so the
                        # mask->exp chain is never delayed long); level p-1
                        # paces pair p's chains into head-pair p-1's loop so
                        # late head-pairs keep PE fill against the exp clock
                        for jt in (p, 4 + p):
                            for nbp in range(NB):
                                def qk_and_rope(jt=jt, nbp=nbp):
                                    emit_qk(jt, nbp)
                                    rope_block(jt, nbp)
                                fill.append(
                                    (
                                        p - 1,
                                        ("pair", p) if (jt >= 4 and nbp == NB - 1) else None,
                                        qk_and_rope,
                                    )
                                )

                    state = {"pos": 0}
                    done_markers = set()

                    def drain_one(cap):
                        if state["pos"] < len(fill):
                            lev, key, fn = fill[state["pos"]]
                            if lev > cap:
                                return
                            state["pos"] += 1
                            fn()
                            if key is not None:
                                done_markers.add(key)

                    def drain_until(key):
                        if key in done_markers:
                            return
                        while state["pos"] < len(fill):
                            _lev, k, fn = fill[state["pos"]]
                            state["pos"] += 1
                            fn()
                            if k is not None:
                                done_markers.add(k)
                            if k == key:
                                return

                    def emit_pv(nb, hp, onat, pend, last):
                        pj, ppt, poff2, pw, pooff = pend
                        r0 = pooff // 128
                        for i in range(r0, 4):
                            g, il = i // 2, i % 2
                            # column of q-tile i inside the score window
                            cs = i * 128 - pooff + poff2
                            # bank g's final write happens at the diagonal
                            # j-tile that still covers q-tile g*2+1
                            for hh in range(2):
                                h = 2 * hp + hh
                                nc.tensor.matmul(
                                    onat[g][:, il * 130 + hh * 65 : il * 130 + hh * 65 + 65],
                                    ppt[:, hh * 512 + cs : hh * 512 + cs + 128],
                                    vN[pj][:, h * 65 : h * 65 + 65],
                                    start=(pj == 0 and hh == 0 and il == 0),
                                    stop=(pj == 4 * nb + 2 * g + 1 and hh == 1 and i == g * 2 + 1),
                                    skip_group_check=True,
                                )

                    # ---------------- attention ----------------
                    for hp in range(4):
                        if hp > 0:
                            drain_until(("pair", hp))
                        for nb in range(NB):
                            if 4 * nb + 3 >= 4:
                                drain_until(("v", 4 * nb + 3))
                            # natural-layout PV accumulators: one PSUM bank
                            # per 2 q-tiles; col(i%2, h, d) = (i%2)*130+h*65+d
                            # (col 64 of each 65-group is the Z denominator)
                            onat = [
                                op.tile([128, 512], F32, tag="on", name=f"on_{nb}_{hp}_{g}")
                                for g in range(2)
                            ]
                            # software-pipelined: scores/exp for j+1 are
                            # emitted before PV of j so PE never waits on exp
                            pend = None  # (j, pt, off2, w, ooff)
                            for j in range(4 * nb + 4):
                                if j // 4 == nb:
                                    qoff = j * 128
                                    w = 512 * (nb + 1) - qoff
                                else:
                                    qoff, w = nb * 512, 512
                                # diag tiles: score/exp only the causal width
                                # w of each head's half; qbase clamp keeps the
                                # window in-bounds at the tail (nb=3), where
                                # the causal range sits at [off2, off2+w)
                                qbase = min(qoff, N - 512)
                                off2 = qoff - qbase
                                ooff = qoff - 512 * nb
                                st = sp.tile([128, 1024], F32, tag="st", name=f"st_{nb}_{hp}_{j}")
                                _lbl(f"score{hp}_{nb}_{j}")
                                dg = j // 4 == nb
                                for hh in range(2):
                                    nc.tensor.matmul(
                                        st[:, hh * 512 + off2 : hh * 512 + off2 + w],
                                        qkT[4 + hp][hh * 64 : hh * 64 + 64, j * 128 : (j + 1) * 128],
                                        qkT[hp][hh * 64 : hh * 64 + 64, qbase + off2 : qbase + off2 + w],
                                        start=True,
                                        stop=not dg,
                                    )
                                if dg:
                                    # causal mask on PE: accumulate the 0/-1e9
                                    # triangle table through an identity lhsT
                                    # (keeps DVE out of the exp chain)
                                    for hh in range(2):
                                        nc.tensor.matmul(
                                            st[:, hh * 512 + off2 : hh * 512 + off2 + 128],
                                            ident_sb[:, :],
                                            mask_sb[:, :],
                                            start=False,
                                            stop=True,
                                            skip_group_check=True,
                                        )
                                pt = ptp.tile([128, 1024], BF16, tag="pt", name=f"pt_{nb}_{hp}_{j}")
                                if w < 512:
                                    nc.scalar.activation(
                                        pt.rearrange("p (b q) -> p b q", b=2)[:, :, off2 : off2 + w],
                                        st[:, 0:1024].rearrange("p (b q) -> p b q", b=2)[:, :, off2 : off2 + w],
                                        mybir.ActivationFunctionType.Exp,
                                    )
                                else:
                                    nc.scalar.activation(
                                        pt[:, :],
                                        st[:, :],
                                        mybir.ActivationFunctionType.Exp,
                                    )
                                if pend is not None:
                                    _lbl(f"pv{hp}_{nb}_{pend[0]}")
                                    emit_pv(nb, hp, onat, pend, last=False)
                                pend = (j, pt, off2, w, ooff)
                                drain_one(hp)
                            _lbl(f"pvL{hp}_{nb}_{pend[0]}")
                            emit_pv(nb, hp, onat, pend, last=True)
                            # normalization in natural layout: per-partition
                            # 1/Z broadcast along free dim, one DVE op per
                            # two heads; the PE transposes that rebuild oT
                            # are deferred as fill items so they never block
                            # the next block's scores in the PE queue
                            for g in range(2):
                                rzq = zp.tile([128, 4], F32, tag="rz", name=f"rz_{nb}_{hp}_{g}")
                                nc.vector.reciprocal(
                                    rzq[:, :], onat[g][:, 64:260:65]
                                )
                                for il in range(2):
                                    i = g * 2 + il
                                    onsb = obp.tile(
                                        [128, 128], BF16, tag="onsb", name=f"onsb_{nb}_{hp}_{i}"
                                    )
                                    nc.vector.tensor_mul(
                                        onsb[:, :].rearrange("p (h e) -> p h e", e=64),
                                        onat[g][:, il * 130 : il * 130 + 130].rearrange(
                                            "p (h e) -> p h e", e=65
                                        )[:, :, 0:64],
                                        rzq[:, il * 2 : il * 2 + 2, None].broadcast_to([128, 2, 64]),
                                    )

                                    def tp_item(nb=nb, hp=hp, i=i, onsb=onsb):
                                        _lbl(f"tp{hp}_{nb}_{i}")
                                        tp = fp.tile([128, 128], BF16, tag="fill", name=f"tp_{nb}_{hp}_{i}")
                                        nc.tensor.transpose(tp[:, :], onsb[:, :], ident_sb[:, :])
                                        nc.vector.tensor_copy(
                                            out=oT[hp][:, nb * 512 + i * 128 : nb * 512 + (i + 1) * 128],
                                            in_=tp[:, :],
                                        )
                                    # front of the pending queue: must drain
                                    # within the next block so onat/onsb
                                    # buffers recycle on time
                                    fill.insert(state["pos"] + 2 * g + il, (0, None, tp_item))
                            if hp == 3:
                                # out-proj for q-block nb, one block behind
                                for i in range(4 * nb, 4 * nb + 4):
                                    for cb in range(2):
                                        fill.append(
                                            (0, None, lambda i=i, cb=cb: emit_outproj_i(i, cb))
                                        )
                    while state["pos"] < len(fill):
                        drain_one(99)
    nc.compile()
    return nc


def make_in_maps(x, Wqkv_w, Wqkv_b, out_w):
    """Host-side sharding/layout prep. Returns per-core input dicts."""
    in_maps = []
    # deinterleave perm within one head: even rope components then odd
    perm = np.concatenate([np.arange(0, D, 2), np.arange(1, D, 2)])
    # rope tables
    inv = 1.0 / (ROPE_THETA ** (np.arange(0, D, 2, dtype=np.float64) / D))
    ang = np.arange(N, dtype=np.float64)[:, None] * inv[None, :]  # [N, 32]
    cosT = np.cos(ang).T.astype(np.float32)  # [32, N]
    sinT = np.sin(ang).T.astype(np.float32)
    cosb = np.tile(cosT, (4, 1))  # [128, N]
    sinb = np.concatenate([sinT, -sinT, sinT, -sinT], axis=0)  # [128, N], block a holds out-block a^1's signed sin
    qc, kc = np.arange(128), np.arange(128)
    maskp = np.where(qc[None, :] >= kc[:, None], 0.0, NEG).astype(np.float32)
    identp = np.eye(128, dtype=np.float32)

    for c in range(8):
        b, g = c // 2, c % 2
        heads = np.arange(g * HPC, (g + 1) * HPC)
        qk_rows = (heads[:, None] * D + perm[None, :]).reshape(-1)  # [512]
        v_rows = (heads[:, None] * D + np.arange(D)[None, :]).reshape(-1)
        Wq = Wqkv_w[qk_rows] * SCALE
        bq = Wqkv_b[qk_rows] * SCALE
        Wk = Wqkv_w[C + qk_rows]
        bk = Wqkv_b[C + qk_rows]
        Wv = Wqkv_w[2 * C + v_rows]
        bv = Wqkv_b[2 * C + v_rows]
        Wcat = np.concatenate([Wq, Wk, Wv], axis=0)  # [1536, C]
        wt = np.ascontiguousarray(Wcat.T).reshape(CC, 128, 1536)
        xt = np.ascontiguousarray(x[b].T).reshape(CC, 128, N)
        bqk = np.ascontiguousarray(
            np.concatenate([bq, bk]).reshape(8, 128).T
        )  # [128, 8]
        owt = np.ascontiguousarray(out_w[:, g * JQK : (g + 1) * JQK].T).reshape(
            4, 128, C
        )
        import ml_dtypes
        in_maps.append(
            dict(
                onesp=np.ones((1, 128), dtype=np.float32),
                ones16=np.ones((128, 8), dtype=ml_dtypes.bfloat16),
                xt=xt.astype(ml_dtypes.bfloat16),
                wt=wt.astype(ml_dtypes.bfloat16),
                bqk=bqk.astype(np.float32),
                bv=np.ascontiguousarray(bv[None, :]).astype(np.float32),
                cosb=cosb.astype(ml_dtypes.bfloat16),
                sinb=sinb.astype(ml_dtypes.bfloat16),
                maskb=maskp.astype(ml_dtypes.bfloat16),
                identb=identp.astype(ml_dtypes.bfloat16),
                owt=owt.astype(ml_dtypes.bfloat16),
            )
        )
    return in_maps


_CACHED_NC = None


def kernel(x, Wqkv_w, Wqkv_b, out_w, out_b):
    from concourse.bass_utils import run_bass_kernel_spmd

    global _CACHED_NC
    x = np.asarray(x, dtype=np.float32)
    Wqkv_w = np.asarray(Wqkv_w, dtype=np.float32)
    Wqkv_b = np.asarray(Wqkv_b, dtype=np.float32)
    out_w = np.asarray(out_w, dtype=np.float32)
    out_b = np.asarray(out_b, dtype=np.float32)

    if _CACHED_NC is None:
        _CACHED_NC = build_nc()
    nc = _CACHED_NC
    in_maps = make_in_maps(x, Wqkv_w, Wqkv_b, out_w)
    res = run_bass_kernel_spmd(nc, in_maps, core_ids=list(range(8)))
    out = np.empty((B, N, C), dtype=np.float32)
    for b in range(B):
        out[b] = res.results[2 * b]["out"] + res.results[2 * b + 1]["out"] + out_b
    return out


# revision 26
# speedup vs baseline: 1.2357x; 1.0308x over previous
"""Trainium2 Bass kernel for a causal multi-head attention block
(fused QKV proj + RoPE + causal softmax attention + out proj).

Sharding: 8 cores = 4 batches x 2 head-groups (8 heads each), no
on-chip collectives: each core emits a partial out-projection [N, C]
(row-parallel over heads); the host sums each batch's pair of partials
and adds the output bias.

Schedule (single fused stream, PE never phase-barriers):
  - Prefix: chunk-major projection of q0/k0 (pair 0) across 8 PSUM
    banks while the wt/xt chunks stream in, then v tiles 0-3.
  - Attention runs head-pair-outer / q-block-inner. All remaining
    projection work (v4-15, q/k pairs 1-3, their RoPE) lives in a fill
    queue drained one item per k-tile iteration, so the PE pipeline
    stays dense while ACT's exp stream (the per-iteration clock) runs.
  - Scores S^T[k, q] for both heads of a pair row-packed into one
    [128,1024] PSUM tile; causal-trimmed on diagonal tiles; one wide
    exp -> bf16 P^T; P^T @ [v|1] accumulates o^T and the softmax
    denominator Z per head.
  - Normalization is entirely off the PE path: o_ps evacuates to SBUF
    (bf16) immediately (PSUM recycles in <1us), then DVE recip ->
    gpsimd partition-broadcast -> DVE multiply produce oT in bf16.
  - Out-proj (bf16) for q-block nb is enqueued as fill during the
    last head-pair, one block behind its norm, and drains at the tail.
Scores/PV/projections in bf16 (f32 PSUM accumulation); v-bias via a
K=1 ones-matmul; q pre-scaled by D^-0.5 on the host.
"""

import sys

sys.path.insert(0, "/opt/trn_rl_repo")

import numpy as np

import concourse.bass as bass
import concourse.mybir as mybir
from concourse import bacc, library_config
from concourse.tile import TileContext

F32 = mybir.dt.float32
F32R = mybir.dt.float32r
BF16 = mybir.dt.bfloat16

B, N, C = 4, 2048, 1024
H_ALL, D = 16, 64
HPC = 8  # heads per core
JQK = HPC * D  # 512 rows for q (and k) per core
ROPE_THETA = 10000.0
SCALE = D**-0.5
NEG = -1e9

NT = N // 128  # 16 n-tiles
NB = N // 512  # 4 n-blocks
CC = C // 128  # 8 contraction chunks


def r(ap):
    return ap.bitcast(F32R)


PE_LABELS = []
_CUR = ["?"]


def _lbl(s):
    _CUR[0] = s


def build_nc(reps=1):
    PE_LABELS.clear()
    nc = bacc.Bacc(None, target_bir_lowering=False)
    _orig_mm = nc.tensor.matmul

    def _mm(*a, **k):
        PE_LABELS.append(_CUR[0])
        return _orig_mm(*a, **k)

    nc.tensor.matmul = _mm

    xt = nc.declare_dram_parameter("xt", [CC, 128, N], BF16, isOutput=False)
    wt = nc.declare_dram_parameter("wt", [CC, 128, 1536], BF16, isOutput=False)
    bqk = nc.declare_dram_parameter("bqk", [128, 8], F32, isOutput=False)
    bv = nc.declare_dram_parameter("bv", [1, JQK], F32R, isOutput=False)
    cosb = nc.declare_dram_parameter("cosb", [128, N], BF16, isOutput=False)
    sinb = nc.declare_dram_parameter("sinb", [128, N], BF16, isOutput=False)
    maskb = nc.declare_dram_parameter("maskb", [128, 128], BF16, isOutput=False)
    identb = nc.declare_dram_parameter("identb", [128, 128], BF16, isOutput=False)
    owt = nc.declare_dram_parameter("owt", [4, 128, C], BF16, isOutput=False)
    onesp = nc.declare_dram_parameter("onesp", [1, 128], F32R, isOutput=False)
    ones16 = nc.declare_dram_parameter("ones16", [128, 8], BF16, isOutput=False)
    out = nc.declare_dram_parameter("out", [N, C], F32, isOutput=True)

    with TileContext(nc) as tc:
      for _rep in range(reps):
        with tc.tile_pool(name="persist", bufs=1) as pp:
            qkT = [pp.tile([128, N], BF16, tag=f"qkT{t}", name=f"qkT{t}") for t in range(8)]
            vN = [pp.tile([128, HPC * 65], BF16, tag=f"vN{t}", name=f"vN{t}") for t in range(NT)]
            oT = [pp.tile([128, N], BF16, tag=f"oT{t}", name=f"oT{t}") for t in range(4)]
            owt_sb = [pp.tile([128, C], BF16, tag=f"owt{hc}", name=f"owt{hc}") for hc in range(4)]
            cos_sb = pp.tile([128, N], BF16, tag="cos_sb", name="cos_sb")
            sin_sb = pp.tile([128, N], BF16, tag="sin_sb", name="sin_sb")
            mask_sb = pp.tile([128, 128], BF16, tag="mask_sb", name="mask_sb")
            ident_sb = pp.tile([128, 128], BF16, tag="ident_sb", name="ident_sb")
            bqk_sb = pp.tile([128, 8], F32, tag="bqk_sb", name="bqk_sb")
            bv_sb = pp.tile([1, JQK], F32R, tag="bv_sb", name="bv_sb")
            ones_sb = pp.tile([1, 128], F32R, tag="ones_sb", name="ones_sb")
            ones16_sb = pp.tile([128, 8], BF16, tag="ones16_sb", name="ones16_sb")
            xts = [pp.tile([128, N], BF16, tag=f"xt{cch}", name=f"xt{cch}") for cch in range(CC)]
            wt_sb = [pp.tile([128, 1536], BF16, tag=f"wt{cch}", name=f"wt{cch}") for cch in range(CC)]

            # input DMAs in consumption order: per chunk wt then two halves
            # of xt (half pieces advance the chunk-major prefix earlier
            # without blowing the serial HWDGE desc-gen budget); tables
            # after the chunks; owt last (first consumed ~80us in).
            for cch in range(CC):
                nc.sync.dma_start(out=wt_sb[cch][:, :], in_=wt[cch, :, :])
                for nbp in range(2):
                    nc.sync.dma_start(
                        out=xts[cch][:, nbp * 1024 : (nbp + 1) * 1024],
                        in_=xt[cch, :, nbp * 1024 : (nbp + 1) * 1024],
                    )
                if cch == 1:
                    nc.sync.dma_start(out=bqk_sb[:, :], in_=bqk[:, :])
                    nc.sync.dma_start(out=ones16_sb[:, :], in_=ones16[:, :])
                    nc.sync.dma_start(out=bv_sb[:, :], in_=bv[:, :])
                    nc.sync.dma_start(out=ones_sb[:, :], in_=onesp[:, :])
            nc.sync.dma_start(out=cos_sb[:, :], in_=cosb[:, :])
            nc.sync.dma_start(out=sin_sb[:, :], in_=sinb[:, :])
            nc.sync.dma_start(out=mask_sb[:, :], in_=maskb[:, :])
            nc.sync.dma_start(out=ident_sb[:, :], in_=identb[:, :])
            for hc in range(4):
                nc.sync.dma_start(out=owt_sb[hc][:, :], in_=owt[hc, :, :])

            # gpsimd: library + the ones column (col 64 of each head group)
            # for every v tile - independent of the v projections
            nc.gpsimd.load_library(library_config.attn)
            for t in range(NT):
                nc.gpsimd.tensor_copy(
                    out=vN[t][:, 64 : HPC * 65 : 65], in_=ones16_sb[:, :]
                )

            with tc.tile_pool(name="rope", bufs=2) as rp:
                sw_cache = {}

                def get_sw(jt):
                    # one sw tile per jt, shared by its rope block-items;
                    # 2 rotating buffers (jt usage windows are sequential)
                    if jt not in sw_cache:
                        sw_cache[jt] = rp.tile(
                            [128, N], BF16, tag="swf", name=f"swf{jt}", bufs=2
                        )
                    return sw_cache[jt]

                def rope_block(jt, nbp):
                    # r[a] = q[a]*cos[a] + q[a^1]*sinSigned[a] per 32-block,
                    # applied to one 512-wide n-block so the first consumer
                    # never waits on a full-row DVE chain
                    sw = get_sw(jt)
                    s = slice(nbp * 512, (nbp + 1) * 512)
                    for a in range(4):
                        b = (a ^ 1) * 32
                        nc.vector.tensor_mul(
                            sw[a * 32 : a * 32 + 32, s],
                            qkT[jt][b : b + 32, s],
                            sin_sb[b : b + 32, s],
                        )
                    nc.vector.tensor_mul(qkT[jt][:, s], qkT[jt][:, s], cos_sb[:, s])
                    nc.vector.tensor_add(qkT[jt][:, s], qkT[jt][:, s], sw[:, s])

                # ---- prefix: pair 0 (q=jt0, k=jt4) chunk-major across 8
                # PSUM banks so PE tracks the chunk DMA stream ----
                with tc.tile_pool(name="prefix_ps", bufs=1, space="PSUM") as pfx:
                    # 6 banks only: the other two stay untouched so the
                    # fill pool's first chains never wait on prefix evacs
                    pf = {
                        (jt, nbp): pfx.tile(
                            [128, 512], F32, tag=f"pf{jt}_{nbp}", name=f"pf{jt}_{nbp}"
                        )
                        for jt in (0, 4)
                        for nbp in range(3)
                    }
                    _lbl("prefix")
                    for cch in range(CC):
                        for nbp in range(3):
                            for jt in (0, 4):
                                nc.tensor.matmul(
                                    pf[(jt, nbp)][:, :],
                                    wt_sb[cch][:, jt * 128 : (jt + 1) * 128],
                                    xts[cch][:, nbp * 512 : (nbp + 1) * 512],
                                    start=(cch == 0),
                                    stop=(cch == CC - 1),
                                )
                    # all evacs first (each frees a PSUM bank; keeps the DVE
                    # queue short ahead of the v evacuations), then only the
                    # nb0 rope blocks -- the rest run after v0-3 below
                    for nbp in range(3):
                        for jt in (0, 4):
                            nc.vector.tensor_scalar_add(
                                out=qkT[jt][:, nbp * 512 : (nbp + 1) * 512],
                                in0=pf[(jt, nbp)][:, :],
                                scalar1=bqk_sb[:, jt : jt + 1],
                            )
                    # nb3 chain-major (all chunks present by now)
                    for jt in (0, 4):
                        ps3 = pfx.tile([128, 512], F32, tag="pf0_0", name=f"pf3_{jt}")
                        for cch in range(CC):
                            nc.tensor.matmul(
                                ps3[:, :],
                                wt_sb[cch][:, jt * 128 : (jt + 1) * 128],
                                xts[cch][:, 1536:2048],
                                start=(cch == 0),
                                stop=(cch == CC - 1),
                            )
                        nc.vector.tensor_scalar_add(
                            out=qkT[jt][:, 1536:2048],
                            in0=ps3[:, :],
                            scalar1=bqk_sb[:, jt : jt + 1],
                        )
                    rope_block(0, 0)
                    rope_block(4, 0)

                # ---- fused attention + fill stream ----
                with (
                    tc.tile_pool(name="attn_ps", bufs=2, space="PSUM") as sp,
                    tc.tile_pool(name="o_ps", bufs=2, space="PSUM") as op,
                    tc.tile_pool(name="fill_ps", bufs=2, space="PSUM") as fp,
                    tc.tile_pool(name="pt_pool", bufs=6) as ptp,
                    tc.tile_pool(name="znorm", bufs=4) as zp,
                    tc.tile_pool(name="onsb_pool", bufs=8) as obp,
                    tc.tile_pool(name="ostage", bufs=4) as osg,
                ):
                    # ---------------- fill queue machinery ----------------
                    def emit_v(t):
                        _lbl(f"fill_v{t}")
                        psv = fp.tile([128, 512], F32, tag="fill", name=f"psv_{t}")
                        for cch in range(CC):
                            nc.tensor.matmul(
                                psv[:, :],
                                xts[cch][:, t * 128 : (t + 1) * 128],
                                wt_sb[cch][:, 1024:1536],
                                start=(cch == 0),
                                stop=False,
                            )
                        nc.tensor.matmul(
                            psv[:, :],
                            r(ones_sb[:, 0:128]),
                            r(bv_sb[:, :]),
                            start=False,
                            stop=True,
                        )
                        nc.vector.tensor_copy(
                            out=vN[t].rearrange("p (h e) -> p h e", e=65)[:, :, 0:64],
                            in_=psv[:, :].rearrange("p (h d) -> p h d", d=64),
                        )

                    def emit_qk(jt, nbp):
                        _lbl(f"fill_qk{jt}_{nbp}")
                        ps = fp.tile([128, 512], F32, tag="fill", name=f"psqk_{jt}_{nbp}")
                        for cch in range(CC):
                            nc.tensor.matmul(
                                ps[:, :],
                                wt_sb[cch][:, jt * 128 : (jt + 1) * 128],
                                xts[cch][:, nbp * 512 : (nbp + 1) * 512],
                                start=(cch == 0),
                                stop=(cch == CC - 1),
                            )
                        nc.vector.tensor_scalar_add(
                            out=qkT[jt][:, nbp * 512 : (nbp + 1) * 512],
                            in0=ps[:, :],
                            scalar1=bqk_sb[:, jt : jt + 1],
                        )

                    def emit_outproj_i(i, cb):
                        _lbl(f"outproj{i}_{cb}")
                        pso = fp.tile([128, 512], F32, tag="fill", name=f"pso_{i}_{cb}")
                        for hc in range(4):
                            nc.tensor.matmul(
                                pso[:, :],
                                oT[hc][:, i * 128 : (i + 1) * 128],
                                owt_sb[hc][:, cb * 512 : (cb + 1) * 512],
                                start=(hc == 0),
                                stop=(hc == 3),
                            )
                        ost = osg.tile([128, 512], F32, tag="ost", name=f"ost_{i}_{cb}")
                        nc.vector.tensor_copy(out=ost[:, :], in_=pso[:, :])
                        nc.sync.dma_start(
                            out=out[i * 128 : (i + 1) * 128, cb * 512 : (cb + 1) * 512],
                            in_=ost[:, :],
                        )

                    # v tiles 0-3 (needed by the first attention block) and
                    # the remaining pair-0 rope blocks run before attention;
                    # v evacs land early in the DVE queue
                    for t in range(4):
                        emit_v(t)
                    for nbp in range(1, NB):
                        rope_block(0, nbp)
                        rope_block(4, nbp)

                    fill = []  # (level, marker_key_or_None, emitfn)
                    for t in range(4, NT):
                        fill.append((0, ("v", t), lambda t=t: emit_v(t)))
                    for p in range(1, 4):
                        # qk chain for one n-block, then its rope right away
                        # (per-block items keep DVE bursts short so the
                        # mask->exp chain is never delayed long); level p-1
                        # paces pair p's chains into head-pair p-1's loop so
                        # late head-pairs keep PE fill against the exp clock
                        for jt in (p, 4 + p):
                            for nbp in range(NB):
                                def qk_and_rope(jt=jt, nbp=nbp):
                                    emit_qk(jt, nbp)
                                    rope_block(jt, nbp)
                                fill.append(
                                    (
                                        p - 1,
                                        ("pair", p) if (jt >= 4 and nbp == NB - 1) else None,
                                        qk_and_rope,
                                    )
                                )

                    state = {"pos": 0}
                    done_markers = set()

                    def drain_one(cap):
                        if state["pos"] < len(fill):
                            lev, key, fn = fill[state["pos"]]
                            if lev > cap:
                                return
                            state["pos"] += 1
                            fn()
                            if key is not None:
                                done_markers.add(key)

                    def drain_until(key):
                        if key in done_markers:
                            return
                        while state["pos"] < len(fill):
                            _lev, k, fn = fill[state["pos"]]
                            state["pos"] += 1
                            fn()
                            if k is not None:
                                done_markers.add(k)
                            if k == key:
                                return

                    def emit_pv(nb, hp, onat, pend, last):
                        pj, ppt, poff2, pw, pooff = pend
                        r0 = pooff // 128
                        for i in range(r0, 4):
                            g, il = i // 2, i % 2
                            # column of q-tile i inside the score window
                            cs = i * 128 - pooff + poff2
                            # bank g's final write happens at the diagonal
                            # j-tile that still covers q-tile g*2+1
                            for hh in range(2):
                                h = 2 * hp + hh
                                nc.tensor.matmul(
                                    onat[g][:, il * 130 + hh * 65 : il * 130 + hh * 65 + 65],
                                    ppt[:, hh * 512 + cs : hh * 512 + cs + 128],
                                    vN[pj][:, h * 65 : h * 65 + 65],
                                    start=(pj == 0 and hh == 0 and il == 0),
                                    stop=(pj == 4 * nb + 2 * g + 1 and hh == 1 and i == g * 2 + 1),
                                    skip_group_check=True,
                                )

                    # ---------------- attention ----------------
                    # pass 1: hp0 then hp1 across all blocks (projection fill
                    # drains here); pass 2 interleaves hp2/hp3 per block so
                    # each block's out-proj becomes fill right after its hp3
                    schedule = [(0, nb) for nb in range(NB)]
                    schedule += [(1, nb) for nb in range(NB)]
                    for nb in range(NB):
                        schedule += [(2, nb), (3, nb)]
                    for hp, nb in schedule:
                        if hp > 0:
                            drain_until(("pair", hp))
                        if True:
                            if 4 * nb + 3 >= 4:
                                drain_until(("v", 4 * nb + 3))
                            # natural-layout PV accumulators: one PSUM bank
                            # per 2 q-tiles; col(i%2, h, d) = (i%2)*130+h*65+d
                            # (col 64 of each 65-group is the Z denominator)
                            onat = [
                                op.tile([128, 512], F32, tag="on", name=f"on_{nb}_{hp}_{g}")
                                for g in range(2)
                            ]
                            # software-pipelined: scores/exp for j+1 are
                            # emitted before PV of j so PE never waits on exp
                            pend = None  # (j, pt, off2, w, ooff)
                            for j in range(4 * nb + 4):
                                if j // 4 == nb:
                                    qoff = j * 128
                                    w = 512 * (nb + 1) - qoff
                                else:
                                    qoff, w = nb * 512, 512
                                # diag tiles: score/exp only the causal width
                                # w of each head's half; qbase clamp keeps the
                                # window in-bounds at the tail (nb=3), where
                                # the causal range sits at [off2, off2+w)
                                qbase = min(qoff, N - 512)
                                off2 = qoff - qbase
                                ooff = qoff - 512 * nb
                                st = sp.tile([128, 1024], F32, tag="st", name=f"st_{nb}_{hp}_{j}")
                                _lbl(f"score{hp}_{nb}_{j}")
                                dg = j // 4 == nb
                                for hh in range(2):
                                    nc.tensor.matmul(
                                        st[:, hh * 512 + off2 : hh * 512 + off2 + w],
                                        qkT[4 + hp][hh * 64 : hh * 64 + 64, j * 128 : (j + 1) * 128],
                                        qkT[hp][hh * 64 : hh * 64 + 64, qbase + off2 : qbase + off2 + w],
                                        start=True,
                                        stop=not dg,
                                    )
                                if dg:
                                    # causal mask on PE: accumulate the 0/-1e9
                                    # triangle table through an identity lhsT
                                    # (keeps DVE out of the exp chain)
                                    for hh in range(2):
                                        nc.tensor.matmul(
                                            st[:, hh * 512 + off2 : hh * 512 + off2 + 128],
                                            ident_sb[:, :],
                                            mask_sb[:, :],
                                            start=False,
                                            stop=True,
                                            skip_group_check=True,
                                        )
                                pt = ptp.tile([128, 1024], BF16, tag="pt", name=f"pt_{nb}_{hp}_{j}")
                                if w < 512:
                                    nc.scalar.activation(
                                        pt.rearrange("p (b q) -> p b q", b=2)[:, :, off2 : off2 + w],
                                        st[:, 0:1024].rearrange("p (b q) -> p b q", b=2)[:, :, off2 : off2 + w],
                                        mybir.ActivationFunctionType.Exp,
                                    )
                                else:
                                    nc.scalar.activation(
                                        pt[:, :],
                                        st[:, :],
                                        mybir.ActivationFunctionType.Exp,
                                    )
                                if pend is not None:
                                    _lbl(f"pv{hp}_{nb}_{pend[0]}")
                                    emit_pv(nb, hp, onat, pend, last=False)
                                pend = (j, pt, off2, w, ooff)
                                drain_one(0 if hp == 0 else 99)
                            _lbl(f"pvL{hp}_{nb}_{pend[0]}")
                            emit_pv(nb, hp, onat, pend, last=True)
                            # normalization in natural layout: per-partition
                            # 1/Z broadcast along free dim, one DVE op per
                            # two heads; the PE transposes that rebuild oT
                            # are deferred as fill items so they never block
                            # the next block's scores in the PE queue
                            for g in range(2):
                                rzq = zp.tile([128, 4], F32, tag="rz", name=f"rz_{nb}_{hp}_{g}")
                                nc.vector.reciprocal(
                                    rzq[:, :], onat[g][:, 64:260:65]
                                )
                                for il in range(2):
                                    i = g * 2 + il
                                    onsb = obp.tile(
                                        [128, 128], BF16, tag="onsb", name=f"onsb_{nb}_{hp}_{i}"
                                    )
                                    nc.vector.tensor_mul(
                                        onsb[:, :].rearrange("p (h e) -> p h e", e=64),
                                        onat[g][:, il * 130 : il * 130 + 130].rearrange(
                                            "p (h e) -> p h e", e=65
                                        )[:, :, 0:64],
                                        rzq[:, il * 2 : il * 2 + 2, None].broadcast_to([128, 2, 64]),
                                    )

                                    def tp_item(nb=nb, hp=hp, i=i, onsb=onsb):
                                        _lbl(f"tp{hp}_{nb}_{i}")
                                        tp = fp.tile([128, 128], BF16, tag="fill", name=f"tp_{nb}_{hp}_{i}")
                                        nc.tensor.transpose(tp[:, :], onsb[:, :], ident_sb[:, :])
                                        nc.vector.tensor_copy(
                                            out=oT[hp][:, nb * 512 + i * 128 : nb * 512 + (i + 1) * 128],
                                            in_=tp[:, :],
                                        )
                                    # front of the pending queue: must drain
                                    # within the next block so onat/onsb
                                    # buffers recycle on time
                                    fill.insert(state["pos"] + 2 * g + il, (0, None, tp_item))
                            if hp == 3:
                                # out-proj for q-block nb, one block behind
                                for i in range(4 * nb, 4 * nb + 4):
                                    for cb in range(2):
                                        fill.append(
                                            (0, None, lambda i=i, cb=cb: emit_outproj_i(i, cb))
                                        )
                    while state["pos"] < len(fill):
                        drain_one(99)
    nc.compile()
    return nc


def make_in_maps(x, Wqkv_w, Wqkv_b, out_w):
    """Host-side sharding/layout prep. Returns per-core input dicts."""
    in_maps = []
    # deinterleave perm within one head: even rope components then odd
    perm = np.concatenate([np.arange(0, D, 2), np.arange(1, D, 2)])
    # rope tables
    inv = 1.0 / (ROPE_THETA ** (np.arange(0, D, 2, dtype=np.float64) / D))
    ang = np.arange(N, dtype=np.float64)[:, None] * inv[None, :]  # [N, 32]
    cosT = np.cos(ang).T.astype(np.float32)  # [32, N]
    sinT = np.sin(ang).T.astype(np.float32)
    cosb = np.tile(cosT, (4, 1))  # [128, N]
    sinb = np.concatenate([sinT, -sinT, sinT, -sinT], axis=0)  # [128, N], block a holds out-block a^1's signed sin
    qc, kc = np.arange(128), np.arange(128)
    maskp = np.where(qc[None, :] >= kc[:, None], 0.0, NEG).astype(np.float32)
    identp = np.eye(128, dtype=np.float32)

    for c in range(8):
        b, g = c // 2, c % 2
        heads = np.arange(g * HPC, (g + 1) * HPC)
        qk_rows = (heads[:, None] * D + perm[None, :]).reshape(-1)  # [512]
        v_rows = (heads[:, None] * D + np.arange(D)[None, :]).reshape(-1)
        Wq = Wqkv_w[qk_rows] * SCALE
        bq = Wqkv_b[qk_rows] * SCALE
        Wk = Wqkv_w[C + qk_rows]
        bk = Wqkv_b[C + qk_rows]
        Wv = Wqkv_w[2 * C + v_rows]
        bv = Wqkv_b[2 * C + v_rows]
        Wcat = np.concatenate([Wq, Wk, Wv], axis=0)  # [1536, C]
        wt = np.ascontiguousarray(Wcat.T).reshape(CC, 128, 1536)
        xt = np.ascontiguousarray(x[b].T).reshape(CC, 128, N)
        bqk = np.ascontiguousarray(
            np.concatenate([bq, bk]).reshape(8, 128).T
        )  # [128, 8]
        owt = np.ascontiguousarray(out_w[:, g * JQK : (g + 1) * JQK].T).reshape(
            4, 128, C
        )
        import ml_dtypes
        in_maps.append(
            dict(
                onesp=np.ones((1, 128), dtype=np.float32),
                ones16=np.ones((128, 8), dtype=ml_dtypes.bfloat16),
                xt=xt.astype(ml_dtypes.bfloat16),
                wt=wt.astype(ml_dtypes.bfloat16),
                bqk=bqk.astype(np.float32),
                bv=np.ascontiguousarray(bv[None, :]).astype(np.float32),
                cosb=cosb.astype(ml_dtypes.bfloat16),
                sinb=sinb.astype(ml_dtypes.bfloat16),
                maskb=maskp.astype(ml_dtypes.bfloat16),
                identb=identp.astype(ml_dtypes.bfloat16),
                owt=owt.astype(ml_dtypes.bfloat16),
            )
        )
    return in_maps


_CACHED_NC = None


def kernel(x, Wqkv_w, Wqkv_b, out_w, out_b):
    from concourse.bass_utils import run_bass_kernel_spmd

    global _CACHED_NC
    x = np.asarray(x, dtype=np.float32)
    Wqkv_w = np.asarray(Wqkv_w, dtype=np.float32)
    Wqkv_b = np.asarray(Wqkv_b, dtype=np.float32)
    out_w = np.asarray(out_w, dtype=np.float32)
    out_b = np.asarray(out_b, dtype=np.float32)

    if _CACHED_NC is None:
        _CACHED_NC = build_nc()
    nc = _CACHED_NC
    in_maps = make_in_maps(x, Wqkv_w, Wqkv_b, out_w)
    res = run_bass_kernel_spmd(nc, in_maps, core_ids=list(range(8)))
    out = np.empty((B, N, C), dtype=np.float32)
    for b in range(B):
        out[b] = res.results[2 * b]["out"] + res.results[2 * b + 1]["out"] + out_b
    return out


# revision 30
# speedup vs baseline: 1.2765x; 1.0330x over previous
"""Trainium2 Bass kernel for a causal multi-head attention block
(fused QKV proj + RoPE + causal softmax attention + out proj).

Sharding: 8 cores = 4 batches x 2 head-groups (8 heads each), no
on-chip collectives: each core emits a partial out-projection [N, C]
(row-parallel over heads); the host sums each batch's pair of partials
and adds the output bias.

Schedule (single fused stream, PE never phase-barriers):
  - Prefix: chunk-major projection of q0/k0 (pair 0) across 8 PSUM
    banks while the wt/xt chunks stream in, then v tiles 0-3.
  - Attention runs head-pair-outer / q-block-inner. All remaining
    projection work (v4-15, q/k pairs 1-3, their RoPE) lives in a fill
    queue drained one item per k-tile iteration, so the PE pipeline
    stays dense while ACT's exp stream (the per-iteration clock) runs.
  - Scores S^T[k, q] for both heads of a pair row-packed into one
    [128,1024] PSUM tile; causal-trimmed on diagonal tiles; one wide
    exp -> bf16 P^T; P^T @ [v|1] accumulates o^T and the softmax
    denominator Z per head.
  - Normalization is entirely off the PE path: o_ps evacuates to SBUF
    (bf16) immediately (PSUM recycles in <1us), then DVE recip ->
    gpsimd partition-broadcast -> DVE multiply produce oT in bf16.
  - Out-proj (bf16) for q-block nb is enqueued as fill during the
    last head-pair, one block behind its norm, and drains at the tail.
Scores/PV/projections in bf16 (f32 PSUM accumulation); v-bias via a
K=1 ones-matmul; q pre-scaled by D^-0.5 on the host.
"""

import sys

sys.path.insert(0, "/opt/trn_rl_repo")

import numpy as np

import concourse.bass as bass
import concourse.mybir as mybir
from concourse import bacc, library_config
from concourse.tile import TileContext

F32 = mybir.dt.float32
F32R = mybir.dt.float32r
BF16 = mybir.dt.bfloat16

B, N, C = 4, 2048, 1024
H_ALL, D = 16, 64
HPC = 8  # heads per core
JQK = HPC * D  # 512 rows for q (and k) per core
ROPE_THETA = 10000.0
SCALE = D**-0.5
NEG = -1e9

NT = N // 128  # 16 n-tiles
NB = N // 512  # 4 n-blocks
CC = C // 128  # 8 contraction chunks


def r(ap):
    return ap.bitcast(F32R)


PE_LABELS = []
_CUR = ["?"]


def _lbl(s):
    _CUR[0] = s


def build_nc(reps=1):
    PE_LABELS.clear()
    nc = bacc.Bacc(None, target_bir_lowering=False)
    _orig_mm = nc.tensor.matmul

    def _mm(*a, **k):
        PE_LABELS.append(_CUR[0])
        return _orig_mm(*a, **k)

    nc.tensor.matmul = _mm

    xt = nc.declare_dram_parameter("xt", [CC, 128, N], BF16, isOutput=False)
    wt = nc.declare_dram_parameter("wt", [CC, 128, 1536], BF16, isOutput=False)
    bqk = nc.declare_dram_parameter("bqk", [128, 8], F32, isOutput=False)
    bv = nc.declare_dram_parameter("bv", [1, JQK], F32R, isOutput=False)
    cosb = nc.declare_dram_parameter("cosb", [128, N], BF16, isOutput=False)
    sinb = nc.declare_dram_parameter("sinb", [128, N], BF16, isOutput=False)
    maskb = nc.declare_dram_parameter("maskb", [128, 128], BF16, isOutput=False)
    identb = nc.declare_dram_parameter("identb", [128, 128], BF16, isOutput=False)
    owt = nc.declare_dram_parameter("owt", [4, 128, C], BF16, isOutput=False)
    onesp = nc.declare_dram_parameter("onesp", [1, 128], F32R, isOutput=False)
    ones16 = nc.declare_dram_parameter("ones16", [128, 8], BF16, isOutput=False)
    out = nc.declare_dram_parameter("out", [N, C], F32, isOutput=True)

    with TileContext(nc) as tc:
      for _rep in range(reps):
        with tc.tile_pool(name="persist", bufs=1) as pp:
            qkT = [pp.tile([128, N], BF16, tag=f"qkT{t}", name=f"qkT{t}") for t in range(8)]
            vN = [pp.tile([128, HPC * 65], BF16, tag=f"vN{t}", name=f"vN{t}") for t in range(NT)]
            oT = [pp.tile([128, N], BF16, tag=f"oT{t}", name=f"oT{t}") for t in range(4)]
            owt_sb = [pp.tile([128, C], BF16, tag=f"owt{hc}", name=f"owt{hc}") for hc in range(4)]
            cos_sb = pp.tile([128, N], BF16, tag="cos_sb", name="cos_sb")
            sin_sb = pp.tile([128, N], BF16, tag="sin_sb", name="sin_sb")
            mask_sb = pp.tile([128, 128], BF16, tag="mask_sb", name="mask_sb")
            ident_sb = pp.tile([128, 128], BF16, tag="ident_sb", name="ident_sb")
            bqk_sb = pp.tile([128, 8], F32, tag="bqk_sb", name="bqk_sb")
            bv_sb = pp.tile([1, JQK], F32R, tag="bv_sb", name="bv_sb")
            ones_sb = pp.tile([1, 128], F32R, tag="ones_sb", name="ones_sb")
            ones16_sb = pp.tile([128, 8], BF16, tag="ones16_sb", name="ones16_sb")
            xts = [pp.tile([128, N], BF16, tag=f"xt{cch}", name=f"xt{cch}") for cch in range(CC)]
            wt_sb = [pp.tile([128, 1536], BF16, tag=f"wt{cch}", name=f"wt{cch}") for cch in range(CC)]

            # input DMAs in consumption order: per chunk wt then two halves
            # of xt (half pieces advance the chunk-major prefix earlier
            # without blowing the serial HWDGE desc-gen budget); tables
            # after the chunks; owt last (first consumed ~80us in).
            for cch in range(CC):
                nc.sync.dma_start(out=wt_sb[cch][:, :], in_=wt[cch, :, :])
                for nbp in range(2):
                    nc.sync.dma_start(
                        out=xts[cch][:, nbp * 1024 : (nbp + 1) * 1024],
                        in_=xt[cch, :, nbp * 1024 : (nbp + 1) * 1024],
                    )
                if cch == 1:
                    nc.sync.dma_start(out=bqk_sb[:, :], in_=bqk[:, :])
                    nc.sync.dma_start(out=ones16_sb[:, :], in_=ones16[:, :])
                    nc.sync.dma_start(out=bv_sb[:, :], in_=bv[:, :])
                    nc.sync.dma_start(out=ones_sb[:, :], in_=onesp[:, :])
            nc.sync.dma_start(out=cos_sb[:, :], in_=cosb[:, :])
            nc.sync.dma_start(out=sin_sb[:, :], in_=sinb[:, :])
            nc.sync.dma_start(out=mask_sb[:, :], in_=maskb[:, :])
            nc.sync.dma_start(out=ident_sb[:, :], in_=identb[:, :])
            for hc in range(4):
                nc.sync.dma_start(out=owt_sb[hc][:, :], in_=owt[hc, :, :])

            # gpsimd: library + the ones column (col 64 of each head group)
            # for every v tile - independent of the v projections
            nc.gpsimd.load_library(library_config.attn)
            for t in range(NT):
                nc.gpsimd.tensor_copy(
                    out=vN[t][:, 64 : HPC * 65 : 65], in_=ones16_sb[:, :]
                )

            with tc.tile_pool(name="rope", bufs=2) as rp:
                sw_cache = {}

                def get_sw(jt):
                    # one sw tile per jt, shared by its rope block-items;
                    # 2 rotating buffers (jt usage windows are sequential)
                    if jt not in sw_cache:
                        sw_cache[jt] = rp.tile(
                            [128, N], BF16, tag="swf", name=f"swf{jt}", bufs=2
                        )
                    return sw_cache[jt]

                def rope_block(jt, nbp):
                    # r[a] = q[a]*cos[a] + q[a^1]*sinSigned[a] per 32-block,
                    # applied to one 512-wide n-block so the first consumer
                    # never waits on a full-row DVE chain
                    sw = get_sw(jt)
                    s = slice(nbp * 512, (nbp + 1) * 512)
                    for a in range(4):
                        b = (a ^ 1) * 32
                        nc.vector.tensor_mul(
                            sw[a * 32 : a * 32 + 32, s],
                            qkT[jt][b : b + 32, s],
                            sin_sb[b : b + 32, s],
                        )
                    nc.vector.tensor_mul(qkT[jt][:, s], qkT[jt][:, s], cos_sb[:, s])
                    nc.vector.tensor_add(qkT[jt][:, s], qkT[jt][:, s], sw[:, s])

                # ---- prefix: pair 0 (q=jt0, k=jt4) chunk-major across 8
                # PSUM banks so PE tracks the chunk DMA stream ----
                with tc.tile_pool(name="prefix_ps", bufs=1, space="PSUM") as pfx:
                    # 6 banks only: the other two stay untouched so the
                    # fill pool's first chains never wait on prefix evacs
                    pf = {
                        (jt, nbp): pfx.tile(
                            [128, 512], F32, tag=f"pf{jt}_{nbp}", name=f"pf{jt}_{nbp}"
                        )
                        for jt in (0, 4)
                        for nbp in range(3)
                    }
                    _lbl("prefix")
                    for cch in range(CC):
                        for nbp in range(3):
                            for jt in (0, 4):
                                nc.tensor.matmul(
                                    pf[(jt, nbp)][:, :],
                                    wt_sb[cch][:, jt * 128 : (jt + 1) * 128],
                                    xts[cch][:, nbp * 512 : (nbp + 1) * 512],
                                    start=(cch == 0),
                                    stop=(cch == CC - 1),
                                )
                    # all evacs first (each frees a PSUM bank; keeps the DVE
                    # queue short ahead of the v evacuations), then only the
                    # nb0 rope blocks -- the rest run after v0-3 below
                    for nbp in range(3):
                        for jt in (0, 4):
                            nc.scalar.activation(
                                qkT[jt][:, nbp * 512 : (nbp + 1) * 512],
                                pf[(jt, nbp)][:, :],
                                mybir.ActivationFunctionType.Identity,
                                bias=bqk_sb[:, jt : jt + 1],
                            )
                    # nb3 chain-major (all chunks present by now)
                    for jt in (0, 4):
                        ps3 = pfx.tile([128, 512], F32, tag="pf0_0", name=f"pf3_{jt}")
                        for cch in range(CC):
                            nc.tensor.matmul(
                                ps3[:, :],
                                wt_sb[cch][:, jt * 128 : (jt + 1) * 128],
                                xts[cch][:, 1536:2048],
                                start=(cch == 0),
                                stop=(cch == CC - 1),
                            )
                        nc.scalar.activation(
                            qkT[jt][:, 1536:2048],
                            ps3[:, :],
                            mybir.ActivationFunctionType.Identity,
                            bias=bqk_sb[:, jt : jt + 1],
                        )
                    rope_block(0, 0)
                    rope_block(4, 0)

                # ---- fused attention + fill stream ----
                with (
                    tc.tile_pool(name="attn_ps", bufs=2, space="PSUM") as sp,
                    tc.tile_pool(name="o_ps", bufs=2, space="PSUM") as op,
                    tc.tile_pool(name="fill_ps", bufs=2, space="PSUM") as fp,
                    tc.tile_pool(name="pt_pool", bufs=6) as ptp,
                    tc.tile_pool(name="znorm", bufs=4) as zp,
                    tc.tile_pool(name="onsb_pool", bufs=8) as obp,
                    tc.tile_pool(name="ostage", bufs=4) as osg,
                ):
                    # ---------------- fill queue machinery ----------------
                    def emit_v(t):
                        _lbl(f"fill_v{t}")
                        psv = fp.tile([128, 512], F32, tag="fill", name=f"psv_{t}")
                        for cch in range(CC):
                            nc.tensor.matmul(
                                psv[:, :],
                                xts[cch][:, t * 128 : (t + 1) * 128],
                                wt_sb[cch][:, 1024:1536],
                                start=(cch == 0),
                                stop=False,
                            )
                        nc.tensor.matmul(
                            psv[:, :],
                            r(ones_sb[:, 0:128]),
                            r(bv_sb[:, :]),
                            start=False,
                            stop=True,
                        )
                        nc.scalar.copy(
                            vN[t].rearrange("p (h e) -> p h e", e=65)[:, :, 0:64],
                            psv[:, :].rearrange("p (h d) -> p h d", d=64),
                        )

                    def emit_qk(jt, nbp):
                        _lbl(f"fill_qk{jt}_{nbp}")
                        ps = fp.tile([128, 512], F32, tag="fill", name=f"psqk_{jt}_{nbp}")
                        for cch in range(CC):
                            nc.tensor.matmul(
                                ps[:, :],
                                wt_sb[cch][:, jt * 128 : (jt + 1) * 128],
                                xts[cch][:, nbp * 512 : (nbp + 1) * 512],
                                start=(cch == 0),
                                stop=(cch == CC - 1),
                            )
                        nc.scalar.activation(
                            qkT[jt][:, nbp * 512 : (nbp + 1) * 512],
                            ps[:, :],
                            mybir.ActivationFunctionType.Identity,
                            bias=bqk_sb[:, jt : jt + 1],
                        )

                    def emit_outproj_i(i, cb):
                        _lbl(f"outproj{i}_{cb}")
                        pso = fp.tile([128, 512], F32, tag="fill", name=f"pso_{i}_{cb}")
                        for hc in range(4):
                            nc.tensor.matmul(
                                pso[:, :],
                                oT[hc][:, i * 128 : (i + 1) * 128],
                                owt_sb[hc][:, cb * 512 : (cb + 1) * 512],
                                start=(hc == 0),
                                stop=(hc == 3),
                            )
                        ost = osg.tile([128, 512], F32, tag="ost", name=f"ost_{i}_{cb}")
                        nc.vector.tensor_copy(out=ost[:, :], in_=pso[:, :])
                        nc.sync.dma_start(
                            out=out[i * 128 : (i + 1) * 128, cb * 512 : (cb + 1) * 512],
                            in_=ost[:, :],
                        )

                    # v tiles 0-3 (needed by the first attention block) and
                    # the remaining pair-0 rope blocks run before attention;
                    # v evacs land early in the DVE queue
                    for t in range(4):
                        emit_v(t)
                    for nbp in range(1, NB):
                        rope_block(0, nbp)
                        rope_block(4, nbp)

                    fill = []  # (level, marker_key_or_None, emitfn)
                    for t in range(4, NT):
                        fill.append((0, ("v", t), lambda t=t: emit_v(t)))
                    for p in range(1, 4):
                        # qk chain for one n-block, then its rope right away
                        # (per-block items keep DVE bursts short so the
                        # mask->exp chain is never delayed long); level p-1
                        # paces pair p's chains into head-pair p-1's loop so
                        # late head-pairs keep PE fill against the exp clock
                        for jt in (p, 4 + p):
                            for nbp in range(NB):
                                def qk_and_rope(jt=jt, nbp=nbp):
                                    emit_qk(jt, nbp)
                                    rope_block(jt, nbp)
                                fill.append(
                                    (
                                        p - 1,
                                        ("pair", p) if (jt >= 4 and nbp == NB - 1) else None,
                                        qk_and_rope,
                                    )
                                )

                    state = {"pos": 0}
                    done_markers = set()

                    def drain_one(cap):
                        if state["pos"] < len(fill):
                            lev, key, fn = fill[state["pos"]]
                            if lev > cap:
                                return
                            state["pos"] += 1
                            fn()
                            if key is not None:
                                done_markers.add(key)

                    def drain_until(key):
                        if key in done_markers:
                            return
                        while state["pos"] < len(fill):
                            _lev, k, fn = fill[state["pos"]]
                            state["pos"] += 1
                            fn()
                            if k is not None:
                                done_markers.add(k)
                            if k == key:
                                return

                    def emit_pv(nb, hp, onat, pend, last):
                        pj, ppt, poff2, pw, pooff = pend
                        r0 = pooff // 128
                        for i in range(r0, 4):
                            g, il = i // 2, i % 2
                            # column of q-tile i inside the score window
                            cs = i * 128 - pooff + poff2
                            # bank g's final write happens at the diagonal
                            # j-tile that still covers q-tile g*2+1
                            for hh in range(2):
                                h = 2 * hp + hh
                                nc.tensor.matmul(
                                    onat[g][:, il * 130 + hh * 65 : il * 130 + hh * 65 + 65],
                                    ppt[:, hh * 512 + cs : hh * 512 + cs + 128],
                                    vN[pj][:, h * 65 : h * 65 + 65],
                                    start=(pj == 0 and hh == 0 and il == 0),
                                    stop=(pj == 4 * nb + 2 * g + 1 and hh == 1 and i == g * 2 + 1),
                                    skip_group_check=True,
                                )

                    # ---------------- attention ----------------
                    # pass 1: hp0 then hp1 across all blocks (projection fill
                    # drains here); pass 2 interleaves hp2/hp3 per block so
                    # each block's out-proj becomes fill right after its hp3
                    schedule = [(0, nb) for nb in range(NB)]
                    schedule += [(1, nb) for nb in range(NB)]
                    for nb in range(NB):
                        schedule += [(2, nb), (3, nb)]
                    def flush_pend(pend):
                        # PV for the pending iteration; when it closes a
                        # block, emit that block's normalization too (this
                        # runs AFTER the next block's first scores, so the
                        # exp stream never drains at block boundaries)
                        onat, nb, hp, pj, ppt, poff2, pw, pooff, is_last = pend
                        _lbl(f"pv{hp}_{nb}_{pj}")
                        emit_pv(nb, hp, onat, (pj, ppt, poff2, pw, pooff), last=is_last)
                        if not is_last:
                            return
                        # normalization in natural layout: per-partition 1/Z
                        # broadcast along free dim; the PE transposes that
                        # rebuild oT are deferred as fill items
                        for g in range(2):
                            rzq = zp.tile([128, 4], F32, tag="rz", name=f"rz_{nb}_{hp}_{g}")
                            nc.vector.reciprocal(
                                rzq[:, :], onat[g][:, 64:260:65]
                            )
                            for il in range(2):
                                i = g * 2 + il
                                onsb = obp.tile(
                                    [128, 128], BF16, tag="onsb", name=f"onsb_{nb}_{hp}_{i}"
                                )
                                nc.vector.tensor_mul(
                                    onsb[:, :].rearrange("p (h e) -> p h e", e=64),
                                    onat[g][:, il * 130 : il * 130 + 130].rearrange(
                                        "p (h e) -> p h e", e=65
                                    )[:, :, 0:64],
                                    rzq[:, il * 2 : il * 2 + 2, None].broadcast_to([128, 2, 64]),
                                )

                                def tp_item(nb=nb, hp=hp, i=i, onsb=onsb):
                                    _lbl(f"tp{hp}_{nb}_{i}")
                                    tp = fp.tile([128, 128], BF16, tag="fill", name=f"tp_{nb}_{hp}_{i}")
                                    nc.tensor.transpose(tp[:, :], onsb[:, :], ident_sb[:, :])
                                    nc.vector.tensor_copy(
                                        out=oT[hp][:, nb * 512 + i * 128 : nb * 512 + (i + 1) * 128],
                                        in_=tp[:, :],
                                    )
                                # front of the pending queue: must drain
                                # within the next block so onat/onsb
                                # buffers recycle on time
                                fill.insert(state["pos"] + 2 * g + il, (0, None, tp_item))
                        if hp == 3:
                            # out-proj for q-block nb, one block behind
                            for i in range(4 * nb, 4 * nb + 4):
                                for cb in range(2):
                                    fill.append(
                                        (0, None, lambda i=i, cb=cb: emit_outproj_i(i, cb))
                                    )

                    pend = None  # carries the score->exp->PV pipeline
                    # across block boundaries
                    for hp, nb in schedule:
                        if hp > 0:
                            drain_until(("pair", hp))
                        if True:
                            if 4 * nb + 3 >= 4:
                                drain_until(("v", 4 * nb + 3))
                            # natural-layout PV accumulators: one PSUM bank
                            # per 2 q-tiles; col(i%2, h, d) = (i%2)*130+h*65+d
                            # (col 64 of each 65-group is the Z denominator)
                            onat = [
                                op.tile([128, 512], F32, tag="on", name=f"on_{nb}_{hp}_{g}")
                                for g in range(2)
                            ]
                            for j in range(4 * nb + 4):
                                if j // 4 == nb:
                                    qoff = j * 128
                                    w = 512 * (nb + 1) - qoff
                                else:
                                    qoff, w = nb * 512, 512
                                # diag tiles: score/exp only the causal width
                                # w of each head's half; qbase clamp keeps the
                                # window in-bounds at the tail (nb=3), where
                                # the causal range sits at [off2, off2+w)
                                qbase = min(qoff, N - 512)
                                off2 = qoff - qbase
                                ooff = qoff - 512 * nb
                                st = sp.tile([128, 1024], F32, tag="st", name=f"st_{nb}_{hp}_{j}")
                                _lbl(f"score{hp}_{nb}_{j}")
                                dg = j // 4 == nb
                                for hh in range(2):
                                    nc.tensor.matmul(
                                        st[:, hh * 512 + off2 : hh * 512 + off2 + w],
                                        qkT[4 + hp][hh * 64 : hh * 64 + 64, j * 128 : (j + 1) * 128],
                                        qkT[hp][hh * 64 : hh * 64 + 64, qbase + off2 : qbase + off2 + w],
                                        start=True,
                                        stop=not dg,
                                    )
                                if dg:
                                    # causal mask on PE: accumulate the 0/-1e9
                                    # triangle table through an identity lhsT
                                    # (keeps DVE out of the exp chain)
                                    for hh in range(2):
                                        nc.tensor.matmul(
                                            st[:, hh * 512 + off2 : hh * 512 + off2 + 128],
                                            ident_sb[:, :],
                                            mask_sb[:, :],
                                            start=False,
                                            stop=True,
                                            skip_group_check=True,
                                        )
                                pt = ptp.tile([128, 1024], BF16, tag="pt", name=f"pt_{nb}_{hp}_{j}")
                                if w < 512:
                                    nc.scalar.activation(
                                        pt.rearrange("p (b q) -> p b q", b=2)[:, :, off2 : off2 + w],
                                        st[:, 0:1024].rearrange("p (b q) -> p b q", b=2)[:, :, off2 : off2 + w],
                                        mybir.ActivationFunctionType.Exp,
                                    )
                                else:
                                    nc.scalar.activation(
                                        pt[:, :],
                                        st[:, :],
                                        mybir.ActivationFunctionType.Exp,
                                    )
                                if pend is not None:
                                    flush_pend(pend)
                                pend = (onat, nb, hp, j, pt, off2, w, ooff, j == 4 * nb + 3)
                                drain_one(0 if hp == 0 else 99)
                    flush_pend(pend)
                    while state["pos"] < len(fill):
                        drain_one(99)
    nc.compile()
    return nc


def make_in_maps(x, Wqkv_w, Wqkv_b, out_w):
    """Host-side sharding/layout prep. Returns per-core input dicts."""
    in_maps = []
    # deinterleave perm within one head: even rope components then odd
    perm = np.concatenate([np.arange(0, D, 2), np.arange(1, D, 2)])
    # rope tables
    inv = 1.0 / (ROPE_THETA ** (np.arange(0, D, 2, dtype=np.float64) / D))
    ang = np.arange(N, dtype=np.float64)[:, None] * inv[None, :]  # [N, 32]
    cosT = np.cos(ang).T.astype(np.float32)  # [32, N]
    sinT = np.sin(ang).T.astype(np.float32)
    cosb = np.tile(cosT, (4, 1))  # [128, N]
    sinb = np.concatenate([sinT, -sinT, sinT, -sinT], axis=0)  # [128, N], block a holds out-block a^1's signed sin
    qc, kc = np.arange(128), np.arange(128)
    maskp = np.where(qc[None, :] >= kc[:, None], 0.0, NEG).astype(np.float32)
    identp = np.eye(128, dtype=np.float32)

    for c in range(8):
        b, g = c // 2, c % 2
        heads = np.arange(g * HPC, (g + 1) * HPC)
        qk_rows = (heads[:, None] * D + perm[None, :]).reshape(-1)  # [512]
        v_rows = (heads[:, None] * D + np.arange(D)[None, :]).reshape(-1)
        Wq = Wqkv_w[qk_rows] * SCALE
        bq = Wqkv_b[qk_rows] * SCALE
        Wk = Wqkv_w[C + qk_rows]
        bk = Wqkv_b[C + qk_rows]
        Wv = Wqkv_w[2 * C + v_rows]
        bv = Wqkv_b[2 * C + v_rows]
        Wcat = np.concatenate([Wq, Wk, Wv], axis=0)  # [1536, C]
        wt = np.ascontiguousarray(Wcat.T).reshape(CC, 128, 1536)
        xt = np.ascontiguousarray(x[b].T).reshape(CC, 128, N)
        bqk = np.ascontiguousarray(
            np.concatenate([bq, bk]).reshape(8, 128).T
        )  # [128, 8]
        owt = np.ascontiguousarray(out_w[:, g * JQK : (g + 1) * JQK].T).reshape(
            4, 128, C
        )
        import ml_dtypes
        in_maps.append(
            dict(
                onesp=np.ones((1, 128), dtype=np.float32),
                ones16=np.ones((128, 8), dtype=ml_dtypes.bfloat16),
                xt=xt.astype(ml_dtypes.bfloat16),
                wt=wt.astype(ml_dtypes.bfloat16),
                bqk=bqk.astype(np.float32),
                bv=np.ascontiguousarray(bv[None, :]).astype(np.float32),
                cosb=cosb.astype(ml_dtypes.bfloat16),
                sinb=sinb.astype(ml_dtypes.bfloat16),
                maskb=maskp.astype(ml_dtypes.bfloat16),
                identb=identp.astype(ml_dtypes.bfloat16),
                owt=owt.astype(ml_dtypes.bfloat16),
            )
        )
    return in_maps


_CACHED_NC = None


def kernel(x, Wqkv_w, Wqkv_b, out_w, out_b):
    from concourse.bass_utils import run_bass_kernel_spmd

    global _CACHED_NC
    x = np.asarray(x, dtype=np.float32)
    Wqkv_w = np.asarray(Wqkv_w, dtype=np.float32)
    Wqkv_b = np.asarray(Wqkv_b, dtype=np.float32)
    out_w = np.asarray(out_w, dtype=np.float32)
    out_b = np.asarray(out_b, dtype=np.float32)

    if _CACHED_NC is None:
        _CACHED_NC = build_nc()
    nc = _CACHED_NC
    in_maps = make_in_maps(x, Wqkv_w, Wqkv_b, out_w)
    res = run_bass_kernel_spmd(nc, in_maps, core_ids=list(range(8)))
    out = np.empty((B, N, C), dtype=np.float32)
    for b in range(B):
        out[b] = res.results[2 * b]["out"] + res.results[2 * b + 1]["out"] + out_b
    return out


# revision 40
# speedup vs baseline: 1.2782x; 1.0014x over previous
"""Trainium2 Bass kernel for a causal multi-head attention block
(fused QKV proj + RoPE + causal softmax attention + out proj).

Sharding: 8 cores = 4 batches x 2 head-groups (8 heads each), no
on-chip collectives: each core emits a partial out-projection [N, C]
(row-parallel over heads); the host sums each batch's pair of partials
and adds the output bias.

Schedule (single fused stream, PE never phase-barriers):
  - Prefix: chunk-major projection of q0/k0 (pair 0) across 8 PSUM
    banks while the wt/xt chunks stream in, then v tiles 0-3.
  - Attention runs head-pair-outer / q-block-inner. All remaining
    projection work (v4-15, q/k pairs 1-3, their RoPE) lives in a fill
    queue drained one item per k-tile iteration, so the PE pipeline
    stays dense while ACT's exp stream (the per-iteration clock) runs.
  - Scores S^T[k, q] for both heads of a pair row-packed into one
    [128,1024] PSUM tile; causal-trimmed on diagonal tiles; one wide
    exp -> bf16 P^T; P^T @ [v|1] accumulates o^T and the softmax
    denominator Z per head.
  - Normalization is entirely off the PE path: o_ps evacuates to SBUF
    (bf16) immediately (PSUM recycles in <1us), then DVE recip ->
    gpsimd partition-broadcast -> DVE multiply produce oT in bf16.
  - Out-proj (bf16) for q-block nb is enqueued as fill during the
    last head-pair, one block behind its norm, and drains at the tail.
Scores/PV/projections in bf16 (f32 PSUM accumulation); v-bias via a
K=1 ones-matmul; q pre-scaled by D^-0.5 on the host.
"""

import sys

sys.path.insert(0, "/opt/trn_rl_repo")

import numpy as np

import concourse.bass as bass
import concourse.mybir as mybir
from concourse import bacc, library_config
from concourse.tile import TileContext

F32 = mybir.dt.float32
F32R = mybir.dt.float32r
BF16 = mybir.dt.bfloat16

B, N, C = 4, 2048, 1024
H_ALL, D = 16, 64
HPC = 8  # heads per core
JQK = HPC * D  # 512 rows for q (and k) per core
ROPE_THETA = 10000.0
SCALE = D**-0.5
NEG = -1e9

NT = N // 128  # 16 n-tiles
NB = N // 512  # 4 n-blocks
CC = C // 128  # 8 contraction chunks


def r(ap):
    return ap.bitcast(F32R)


PE_LABELS = []
_CUR = ["?"]


def _lbl(s):
    _CUR[0] = s


def build_nc(reps=1):
    PE_LABELS.clear()
    nc = bacc.Bacc(None, target_bir_lowering=False)
    _orig_mm = nc.tensor.matmul

    def _mm(*a, **k):
        PE_LABELS.append(_CUR[0])
        return _orig_mm(*a, **k)

    nc.tensor.matmul = _mm

    xt = nc.declare_dram_parameter("xt", [CC, 128, N], BF16, isOutput=False)
    wt = nc.declare_dram_parameter("wt", [CC, 128, 1536], BF16, isOutput=False)
    bqk = nc.declare_dram_parameter("bqk", [128, 8], F32, isOutput=False)
    bv = nc.declare_dram_parameter("bv", [1, JQK], F32R, isOutput=False)
    cosb = nc.declare_dram_parameter("cosb", [128, N], BF16, isOutput=False)
    sinb = nc.declare_dram_parameter("sinb", [128, N], BF16, isOutput=False)
    maskb = nc.declare_dram_parameter("maskb", [128, 128], BF16, isOutput=False)
    identb = nc.declare_dram_parameter("identb", [128, 128], BF16, isOutput=False)
    owt = nc.declare_dram_parameter("owt", [4, 128, C], BF16, isOutput=False)
    onesp = nc.declare_dram_parameter("onesp", [1, 128], F32R, isOutput=False)
    ones16 = nc.declare_dram_parameter("ones16", [128, 8], BF16, isOutput=False)
    out = nc.declare_dram_parameter("out", [N, C], F32, isOutput=True)

    with TileContext(nc) as tc:
      for _rep in range(reps):
        with tc.tile_pool(name="persist", bufs=1) as pp:
            qkT = [pp.tile([128, N], BF16, tag=f"qkT{t}", name=f"qkT{t}") for t in range(8)]
            vN = [pp.tile([128, HPC * 65], BF16, tag=f"vN{t}", name=f"vN{t}") for t in range(NT)]
            oT = [pp.tile([128, N], BF16, tag=f"oT{t}", name=f"oT{t}") for t in range(4)]
            owt_sb = [pp.tile([128, C], BF16, tag=f"owt{hc}", name=f"owt{hc}") for hc in range(4)]
            cos_sb = pp.tile([128, N], BF16, tag="cos_sb", name="cos_sb")
            sin_sb = pp.tile([128, N], BF16, tag="sin_sb", name="sin_sb")
            mask_sb = pp.tile([128, 128], BF16, tag="mask_sb", name="mask_sb")
            ident_sb = pp.tile([128, 128], BF16, tag="ident_sb", name="ident_sb")
            bqk_sb = pp.tile([128, 8], F32, tag="bqk_sb", name="bqk_sb")
            bv_sb = pp.tile([1, JQK], F32R, tag="bv_sb", name="bv_sb")
            ones_sb = pp.tile([1, 128], F32R, tag="ones_sb", name="ones_sb")
            ones16_sb = pp.tile([128, 8], BF16, tag="ones16_sb", name="ones16_sb")
            xts = [pp.tile([128, N], BF16, tag=f"xt{cch}", name=f"xt{cch}") for cch in range(CC)]
            wt_sb = [pp.tile([128, 1536], BF16, tag=f"wt{cch}", name=f"wt{cch}") for cch in range(CC)]

            # input DMAs in consumption order: per chunk wt then two halves
            # of xt (half pieces advance the chunk-major prefix earlier
            # without blowing the serial HWDGE desc-gen budget); tables
            # after the chunks; owt last (first consumed ~80us in).
            for cch in range(CC):
                nc.sync.dma_start(out=wt_sb[cch][:, :], in_=wt[cch, :, :])
                for nbp in range(2):
                    nc.sync.dma_start(
                        out=xts[cch][:, nbp * 1024 : (nbp + 1) * 1024],
                        in_=xt[cch, :, nbp * 1024 : (nbp + 1) * 1024],
                    )
                if cch == 1:
                    nc.sync.dma_start(out=bqk_sb[:, :], in_=bqk[:, :])
                    nc.sync.dma_start(out=ones16_sb[:, :], in_=ones16[:, :])
                    nc.sync.dma_start(out=bv_sb[:, :], in_=bv[:, :])
                    nc.sync.dma_start(out=ones_sb[:, :], in_=onesp[:, :])
            nc.sync.dma_start(out=cos_sb[:, :], in_=cosb[:, :])
            nc.sync.dma_start(out=sin_sb[:, :], in_=sinb[:, :])
            nc.sync.dma_start(out=mask_sb[:, :], in_=maskb[:, :])
            nc.sync.dma_start(out=ident_sb[:, :], in_=identb[:, :])
            for hc in range(4):
                nc.sync.dma_start(out=owt_sb[hc][:, :], in_=owt[hc, :, :])

            # gpsimd: library + the ones column (col 64 of each head group)
            # for every v tile - independent of the v projections
            nc.gpsimd.load_library(library_config.attn)
            for t in range(NT):
                nc.gpsimd.tensor_copy(
                    out=vN[t][:, 64 : HPC * 65 : 65], in_=ones16_sb[:, :]
                )

            with tc.tile_pool(name="rope", bufs=2) as rp:
                sw_cache = {}

                def get_sw(jt):
                    # one sw tile per jt, shared by its rope block-items;
                    # 2 rotating buffers (jt usage windows are sequential)
                    if jt not in sw_cache:
                        sw_cache[jt] = rp.tile(
                            [128, N], BF16, tag="swf", name=f"swf{jt}", bufs=3
                        )
                    return sw_cache[jt]

                def rope_block(jt, nbp):
                    # r[a] = q[a]*cos[a] + q[a^1]*sinSigned[a] per 32-block,
                    # applied to one 512-wide n-block so the first consumer
                    # never waits on a full-row DVE chain
                    sw = get_sw(jt)
                    s = slice(nbp * 512, (nbp + 1) * 512)
                    for a in range(4):
                        b = (a ^ 1) * 32
                        nc.vector.tensor_mul(
                            sw[a * 32 : a * 32 + 32, s],
                            qkT[jt][b : b + 32, s],
                            sin_sb[b : b + 32, s],
                        )
                    nc.vector.tensor_mul(qkT[jt][:, s], qkT[jt][:, s], cos_sb[:, s])
                    nc.vector.tensor_add(qkT[jt][:, s], qkT[jt][:, s], sw[:, s])

                # ---- prefix: pair 0 (q=jt0, k=jt4) chunk-major across 8
                # PSUM banks so PE tracks the chunk DMA stream ----
                with tc.tile_pool(name="prefix_ps", bufs=1, space="PSUM") as pfx:
                    # 6 banks only: the other two stay untouched so the
                    # fill pool's first chains never wait on prefix evacs
                    pf = {
                        (jt, nbp): pfx.tile(
                            [128, 512], F32, tag=f"pf{jt}_{nbp}", name=f"pf{jt}_{nbp}"
                        )
                        for jt in (0, 4)
                        for nbp in range(3)
                    }
                    _lbl("prefix")
                    for cch in range(CC):
                        for nbp in range(3):
                            for jt in (0, 4):
                                nc.tensor.matmul(
                                    pf[(jt, nbp)][:, :],
                                    wt_sb[cch][:, jt * 128 : (jt + 1) * 128],
                                    xts[cch][:, nbp * 512 : (nbp + 1) * 512],
                                    start=(cch == 0),
                                    stop=(cch == CC - 1),
                                )
                    # all evacs first (each frees a PSUM bank; keeps the DVE
                    # queue short ahead of the v evacuations), then only the
                    # nb0 rope blocks -- the rest run after v0-3 below
                    for nbp in range(3):
                        for jt in (0, 4):
                            nc.scalar.activation(
                                qkT[jt][:, nbp * 512 : (nbp + 1) * 512],
                                pf[(jt, nbp)][:, :],
                                mybir.ActivationFunctionType.Identity,
                                bias=bqk_sb[:, jt : jt + 1],
                            )
                    # nb3 chain-major (all chunks present by now)
                    for jt in (0, 4):
                        ps3 = pfx.tile([128, 512], F32, tag="pf0_0", name=f"pf3_{jt}")
                        for cch in range(CC):
                            nc.tensor.matmul(
                                ps3[:, :],
                                wt_sb[cch][:, jt * 128 : (jt + 1) * 128],
                                xts[cch][:, 1536:2048],
                                start=(cch == 0),
                                stop=(cch == CC - 1),
                            )
                        nc.scalar.activation(
                            qkT[jt][:, 1536:2048],
                            ps3[:, :],
                            mybir.ActivationFunctionType.Identity,
                            bias=bqk_sb[:, jt : jt + 1],
                        )
                    rope_block(0, 0)
                    rope_block(4, 0)

                # ---- fused attention + fill stream ----
                with (
                    tc.tile_pool(name="attn_ps", bufs=2, space="PSUM") as sp,
                    tc.tile_pool(name="o_ps", bufs=2, space="PSUM") as op,
                    tc.tile_pool(name="fill_ps", bufs=2, space="PSUM") as fp,
                    tc.tile_pool(name="pt_pool", bufs=8) as ptp,
                    tc.tile_pool(name="znorm", bufs=4) as zp,
                    tc.tile_pool(name="onsb_pool", bufs=12) as obp,
                    tc.tile_pool(name="ostage", bufs=6) as osg,
                ):
                    # ---------------- fill queue machinery ----------------
                    def emit_v(t):
                        _lbl(f"fill_v{t}")
                        psv = fp.tile([128, 512], F32, tag="fill", name=f"psv_{t}")
                        for cch in range(CC):
                            nc.tensor.matmul(
                                psv[:, :],
                                xts[cch][:, t * 128 : (t + 1) * 128],
                                wt_sb[cch][:, 1024:1536],
                                start=(cch == 0),
                                stop=False,
                            )
                        nc.tensor.matmul(
                            psv[:, :],
                            r(ones_sb[:, 0:128]),
                            r(bv_sb[:, :]),
                            start=False,
                            stop=True,
                        )
                        nc.scalar.copy(
                            vN[t].rearrange("p (h e) -> p h e", e=65)[:, :, 0:64],
                            psv[:, :].rearrange("p (h d) -> p h d", d=64),
                        )

                    def emit_qk(jt, nbp):
                        _lbl(f"fill_qk{jt}_{nbp}")
                        ps = fp.tile([128, 512], F32, tag="fill", name=f"psqk_{jt}_{nbp}")
                        for cch in range(CC):
                            nc.tensor.matmul(
                                ps[:, :],
                                wt_sb[cch][:, jt * 128 : (jt + 1) * 128],
                                xts[cch][:, nbp * 512 : (nbp + 1) * 512],
                                start=(cch == 0),
                                stop=(cch == CC - 1),
                            )
                        nc.scalar.activation(
                            qkT[jt][:, nbp * 512 : (nbp + 1) * 512],
                            ps[:, :],
                            mybir.ActivationFunctionType.Identity,
                            bias=bqk_sb[:, jt : jt + 1],
                        )

                    def emit_outproj_i(i, cb):
                        _lbl(f"outproj{i}_{cb}")
                        pso = fp.tile([128, 512], F32, tag="fill", name=f"pso_{i}_{cb}")
                        for hc in range(4):
                            nc.tensor.matmul(
                                pso[:, :],
                                oT[hc][:, i * 128 : (i + 1) * 128],
                                owt_sb[hc][:, cb * 512 : (cb + 1) * 512],
                                start=(hc == 0),
                                stop=(hc == 3),
                            )
                        ost = osg.tile([128, 512], F32, tag="ost", name=f"ost_{i}_{cb}")
                        nc.vector.tensor_copy(out=ost[:, :], in_=pso[:, :])
                        nc.sync.dma_start(
                            out=out[i * 128 : (i + 1) * 128, cb * 512 : (cb + 1) * 512],
                            in_=ost[:, :],
                        )

                    # v tiles 0-3 (needed by the first attention block) and
                    # the remaining pair-0 rope blocks run before attention;
                    # v evacs land early in the DVE queue
                    for t in range(4):
                        emit_v(t)
                    for nbp in range(1, NB):
                        rope_block(0, nbp)
                        rope_block(4, nbp)

                    fill = []  # (level, marker_key_or_None, emitfn)
                    for t in range(4, NT):
                        fill.append((0, ("v", t), lambda t=t: emit_v(t)))
                    for p in range(1, 4):
                        # qk chain for one n-block, then its rope right away
                        # (per-block items keep DVE bursts short so the
                        # mask->exp chain is never delayed long); level p-1
                        # paces pair p's chains into head-pair p-1's loop so
                        # late head-pairs keep PE fill against the exp clock
                        for jt in (p, 4 + p):
                            for nbp in range(NB):
                                def qk_and_rope(jt=jt, nbp=nbp):
                                    emit_qk(jt, nbp)
                                    rope_block(jt, nbp)
                                fill.append(
                                    (
                                        p - 1,
                                        ("pair", p) if (jt >= 4 and nbp == NB - 1) else None,
                                        qk_and_rope,
                                    )
                                )

                    state = {"pos": 0}
                    done_markers = set()

                    def drain_one(cap):
                        if state["pos"] < len(fill):
                            lev, key, fn = fill[state["pos"]]
                            if lev > cap:
                                return
                            state["pos"] += 1
                            fn()
                            if key is not None:
                                done_markers.add(key)

                    def drain_until(key):
                        if key in done_markers:
                            return
                        while state["pos"] < len(fill):
                            _lev, k, fn = fill[state["pos"]]
                            state["pos"] += 1
                            fn()
                            if k is not None:
                                done_markers.add(k)
                            if k == key:
                                return

                    def emit_pv(nb, hp, onat, pend, last):
                        pj, ppt, poff2, pw, pooff = pend
                        r0 = pooff // 128
                        for i in range(r0, 4):
                            g, il = i // 2, i % 2
                            # column of q-tile i inside the score window
                            cs = i * 128 - pooff + poff2
                            # bank g's final write happens at the diagonal
                            # j-tile that still covers q-tile g*2+1
                            for hh in range(2):
                                h = 2 * hp + hh
                                nc.tensor.matmul(
                                    onat[g][:, il * 130 + hh * 65 : il * 130 + hh * 65 + 65],
                                    ppt[:, hh * 512 + cs : hh * 512 + cs + 128],
                                    vN[pj][:, h * 65 : h * 65 + 65],
                                    start=(pj == 0 and hh == 0 and il == 0),
                                    stop=(pj == 4 * nb + 2 * g + 1 and hh == 1 and i == g * 2 + 1),
                                    skip_group_check=True,
                                )

                    # ---------------- attention ----------------
                    # pass 1: hp0 then hp1 across all blocks (projection fill
                    # drains here); pass 2 interleaves hp2/hp3 per block so
                    # each block's out-proj becomes fill right after its hp3
                    schedule = [(0, nb) for nb in range(NB)]
                    schedule += [(1, nb) for nb in range(NB)]
                    for nb in range(NB):
                        schedule += [(2, nb), (3, nb)]
                    def flush_pend(pend):
                        # PV for the pending iteration; when it closes a
                        # block, emit that block's normalization too (this
                        # runs AFTER the next block's first scores, so the
                        # exp stream never drains at block boundaries)
                        onat, nb, hp, pj, ppt, poff2, pw, pooff, is_last = pend
                        _lbl(f"pv{hp}_{nb}_{pj}")
                        emit_pv(nb, hp, onat, (pj, ppt, poff2, pw, pooff), last=is_last)
                        if not is_last:
                            return
                        # normalization in natural layout: per-partition 1/Z
                        # broadcast along free dim; the PE transposes that
                        # rebuild oT are deferred as fill items
                        for g in range(2):
                            rzq = zp.tile([128, 4], F32, tag="rz", name=f"rz_{nb}_{hp}_{g}")
                            nc.vector.reciprocal(
                                rzq[:, :], onat[g][:, 64:260:65]
                            )
                            for il in range(2):
                                i = g * 2 + il
                                onsb = obp.tile(
                                    [128, 128], BF16, tag="onsb", name=f"onsb_{nb}_{hp}_{i}"
                                )
                                nc.vector.tensor_mul(
                                    onsb[:, :].rearrange("p (h e) -> p h e", e=64),
                                    onat[g][:, il * 130 : il * 130 + 130].rearrange(
                                        "p (h e) -> p h e", e=65
                                    )[:, :, 0:64],
                                    rzq[:, il * 2 : il * 2 + 2, None].broadcast_to([128, 2, 64]),
                                )

                                def tp_item(nb=nb, hp=hp, i=i, onsb=onsb):
                                    _lbl(f"tp{hp}_{nb}_{i}")
                                    tp = fp.tile([128, 128], BF16, tag="fill", name=f"tp_{nb}_{hp}_{i}")
                                    nc.tensor.transpose(tp[:, :], onsb[:, :], ident_sb[:, :])
                                    nc.vector.tensor_copy(
                                        out=oT[hp][:, nb * 512 + i * 128 : nb * 512 + (i + 1) * 128],
                                        in_=tp[:, :],
                                    )
                                # front of the pending queue: must drain
                                # within the next block so onat/onsb
                                # buffers recycle on time
                                fill.insert(state["pos"] + 2 * g + il, (0, None, tp_item))
                        if hp == 3:
                            # out-proj for q-block nb, one block behind
                            for i in range(4 * nb, 4 * nb + 4):
                                for cb in range(2):
                                    fill.append(
                                        (0, None, lambda i=i, cb=cb: emit_outproj_i(i, cb))
                                    )

                    pend = None  # carries the score->exp->PV pipeline
                    # across block boundaries
                    for hp, nb in schedule:
                        if hp > 0:
                            drain_until(("pair", hp))
                        if True:
                            if 4 * nb + 3 >= 4:
                                drain_until(("v", 4 * nb + 3))
                            # natural-layout PV accumulators: one PSUM bank
                            # per 2 q-tiles; col(i%2, h, d) = (i%2)*130+h*65+d
                            # (col 64 of each 65-group is the Z denominator)
                            onat = [
                                op.tile([128, 512], F32, tag="on", name=f"on_{nb}_{hp}_{g}")
                                for g in range(2)
                            ]
                            for j in range(4 * nb + 4):
                                if j // 4 == nb:
                                    qoff = j * 128
                                    w = 512 * (nb + 1) - qoff
                                else:
                                    qoff, w = nb * 512, 512
                                # diag tiles: score/exp only the causal width
                                # w of each head's half; qbase clamp keeps the
                                # window in-bounds at the tail (nb=3), where
                                # the causal range sits at [off2, off2+w)
                                qbase = min(qoff, N - 512)
                                off2 = qoff - qbase
                                ooff = qoff - 512 * nb
                                st = sp.tile([128, 1024], F32, tag="st", name=f"st_{nb}_{hp}_{j}")
                                _lbl(f"score{hp}_{nb}_{j}")
                                dg = j // 4 == nb
                                for hh in range(2):
                                    nc.tensor.matmul(
                                        st[:, hh * 512 + off2 : hh * 512 + off2 + w],
                                        qkT[4 + hp][hh * 64 : hh * 64 + 64, j * 128 : (j + 1) * 128],
                                        qkT[hp][hh * 64 : hh * 64 + 64, qbase + off2 : qbase + off2 + w],
                                        start=True,
                                        stop=not dg,
                                    )
                                if dg:
                                    # causal mask on PE: accumulate the 0/-1e9
                                    # triangle table through an identity lhsT
                                    # (keeps DVE out of the exp chain)
                                    for hh in range(2):
                                        nc.tensor.matmul(
                                            st[:, hh * 512 + off2 : hh * 512 + off2 + 128],
                                            ident_sb[:, :],
                                            mask_sb[:, :],
                                            start=False,
                                            stop=True,
                                            skip_group_check=True,
                                        )
                                pt = ptp.tile([128, 1024], BF16, tag="pt", name=f"pt_{nb}_{hp}_{j}")
                                if w < 512:
                                    nc.scalar.activation(
                                        pt.rearrange("p (b q) -> p b q", b=2)[:, :, off2 : off2 + w],
                                        st[:, 0:1024].rearrange("p (b q) -> p b q", b=2)[:, :, off2 : off2 + w],
                                        mybir.ActivationFunctionType.Exp,
                                    )
                                else:
                                    nc.scalar.activation(
                                        pt[:, :],
                                        st[:, :],
                                        mybir.ActivationFunctionType.Exp,
                                    )
                                if pend is not None:
                                    flush_pend(pend)
                                pend = (onat, nb, hp, j, pt, off2, w, ooff, j == 4 * nb + 3)
                                drain_one(0 if hp == 0 else 99)
                    flush_pend(pend)
                    while state["pos"] < len(fill):
                        drain_one(99)
    nc.compile()
    return nc


def make_in_maps(x, Wqkv_w, Wqkv_b, out_w):
    """Host-side sharding/layout prep. Returns per-core input dicts."""
    in_maps = []
    # deinterleave perm within one head: even rope components then odd
    perm = np.concatenate([np.arange(0, D, 2), np.arange(1, D, 2)])
    # rope tables
    inv = 1.0 / (ROPE_THETA ** (np.arange(0, D, 2, dtype=np.float64) / D))
    ang = np.arange(N, dtype=np.float64)[:, None] * inv[None, :]  # [N, 32]
    cosT = np.cos(ang).T.astype(np.float32)  # [32, N]
    sinT = np.sin(ang).T.astype(np.float32)
    cosb = np.tile(cosT, (4, 1))  # [128, N]
    sinb = np.concatenate([sinT, -sinT, sinT, -sinT], axis=0)  # [128, N], block a holds out-block a^1's signed sin
    qc, kc = np.arange(128), np.arange(128)
    maskp = np.where(qc[None, :] >= kc[:, None], 0.0, NEG).astype(np.float32)
    identp = np.eye(128, dtype=np.float32)

    for c in range(8):
        b, g = c // 2, c % 2
        heads = np.arange(g * HPC, (g + 1) * HPC)
        qk_rows = (heads[:, None] * D + perm[None, :]).reshape(-1)  # [512]
        v_rows = (heads[:, None] * D + np.arange(D)[None, :]).reshape(-1)
        Wq = Wqkv_w[qk_rows] * SCALE
        bq = Wqkv_b[qk_rows] * SCALE
        Wk = Wqkv_w[C + qk_rows]
        bk = Wqkv_b[C + qk_rows]
        Wv = Wqkv_w[2 * C + v_rows]
        bv = Wqkv_b[2 * C + v_rows]
        Wcat = np.concatenate([Wq, Wk, Wv], axis=0)  # [1536, C]
        wt = np.ascontiguousarray(Wcat.T).reshape(CC, 128, 1536)
        xt = np.ascontiguousarray(x[b].T).reshape(CC, 128, N)
        bqk = np.ascontiguousarray(
            np.concatenate([bq, bk]).reshape(8, 128).T
        )  # [128, 8]
        owt = np.ascontiguousarray(out_w[:, g * JQK : (g + 1) * JQK].T).reshape(
            4, 128, C
        )
        import ml_dtypes
        in_maps.append(
            dict(
                onesp=np.ones((1, 128), dtype=np.float32),
                ones16=np.ones((128, 8), dtype=ml_dtypes.bfloat16),
                xt=xt.astype(ml_dtypes.bfloat16),
                wt=wt.astype(ml_dtypes.bfloat16),
                bqk=bqk.astype(np.float32),
                bv=np.ascontiguousarray(bv[None, :]).astype(np.float32),
                cosb=cosb.astype(ml_dtypes.bfloat16),
                sinb=sinb.astype(ml_dtypes.bfloat16),
                maskb=maskp.astype(ml_dtypes.bfloat16),
                identb=identp.astype(ml_dtypes.bfloat16),
                owt=owt.astype(ml_dtypes.bfloat16),
            )
        )
    return in_maps


_CACHED_NC = None


def kernel(x, Wqkv_w, Wqkv_b, out_w, out_b):
    from concourse.bass_utils import run_bass_kernel_spmd

    global _CACHED_NC
    x = np.asarray(x, dtype=np.float32)
    Wqkv_w = np.asarray(Wqkv_w, dtype=np.float32)
    Wqkv_b = np.asarray(Wqkv_b, dtype=np.float32)
    out_w = np.asarray(out_w, dtype=np.float32)
    out_b = np.asarray(out_b, dtype=np.float32)

    if _CACHED_NC is None:
        _CACHED_NC = build_nc()
    nc = _CACHED_NC
    in_maps = make_in_maps(x, Wqkv_w, Wqkv_b, out_w)
    res = run_bass_kernel_spmd(nc, in_maps, core_ids=list(range(8)))
    out = np.empty((B, N, C), dtype=np.float32)
    for b in range(B):
        out[b] = res.results[2 * b]["out"] + res.results[2 * b + 1]["out"] + out_b
    return out


# revision 45
# speedup vs baseline: 1.3061x; 1.0218x over previous
"""Trainium2 Bass kernel for a causal multi-head attention block
(fused QKV proj + RoPE + causal softmax attention + out proj).

Sharding: 8 cores = 4 batches x 2 head-groups (8 heads each), no
on-chip collectives: each core emits a partial out-projection [N, C]
(row-parallel over heads); the host sums each batch's pair of partials
and adds the output bias.

Schedule (single fused stream, PE never phase-barriers):
  - Prefix: chunk-major projection of q0/k0 (pair 0) across 8 PSUM
    banks while the wt/xt chunks stream in, then v tiles 0-3.
  - Attention runs head-pair-outer / q-block-inner. All remaining
    projection work (v4-15, q/k pairs 1-3, their RoPE) lives in a fill
    queue drained one item per k-tile iteration, so the PE pipeline
    stays dense while ACT's exp stream (the per-iteration clock) runs.
  - Scores S^T[k, q] for both heads of a pair row-packed into one
    [128,1024] PSUM tile; causal-trimmed on diagonal tiles; one wide
    exp -> bf16 P^T; P^T @ [v|1] accumulates o^T and the softmax
    denominator Z per head.
  - Normalization is entirely off the PE path: o_ps evacuates to SBUF
    (bf16) immediately (PSUM recycles in <1us), then DVE recip ->
    gpsimd partition-broadcast -> DVE multiply produce oT in bf16.
  - Out-proj (bf16) for q-block nb is enqueued as fill during the
    last head-pair, one block behind its norm, and drains at the tail.
Scores/PV/projections in bf16 (f32 PSUM accumulation); v-bias via a
K=1 ones-matmul; q pre-scaled by D^-0.5 on the host.
"""

import sys

sys.path.insert(0, "/opt/trn_rl_repo")

import numpy as np

import concourse.bass as bass
import concourse.mybir as mybir
from concourse import bacc, library_config
from concourse.tile import TileContext

F32 = mybir.dt.float32
F32R = mybir.dt.float32r
BF16 = mybir.dt.bfloat16

B, N, C = 4, 2048, 1024
H_ALL, D = 16, 64
HPC = 8  # heads per core
JQK = HPC * D  # 512 rows for q (and k) per core
ROPE_THETA = 10000.0
SCALE = D**-0.5
NEG = -1e9

NT = N // 128  # 16 n-tiles
NB = N // 512  # 4 n-blocks
CC = C // 128  # 8 contraction chunks


def r(ap):
    return ap.bitcast(F32R)


PE_LABELS = []
_CUR = ["?"]


def _lbl(s):
    _CUR[0] = s


def build_nc(reps=1):
    PE_LABELS.clear()
    nc = bacc.Bacc(None, target_bir_lowering=False)
    _orig_mm = nc.tensor.matmul

    def _mm(*a, **k):
        PE_LABELS.append(_CUR[0])
        return _orig_mm(*a, **k)

    nc.tensor.matmul = _mm

    xt = nc.declare_dram_parameter("xt", [CC, 128, N], BF16, isOutput=False)
    wt = nc.declare_dram_parameter("wt", [CC, 128, 1536], BF16, isOutput=False)
    bqk = nc.declare_dram_parameter("bqk", [128, 8], F32, isOutput=False)
    bv = nc.declare_dram_parameter("bv", [1, JQK], F32R, isOutput=False)
    cosb = nc.declare_dram_parameter("cosb", [128, N], BF16, isOutput=False)
    sinb = nc.declare_dram_parameter("sinb", [128, N], BF16, isOutput=False)
    maskb = nc.declare_dram_parameter("maskb", [128, 128], BF16, isOutput=False)
    identb = nc.declare_dram_parameter("identb", [128, 128], BF16, isOutput=False)
    owt = nc.declare_dram_parameter("owt", [4, 128, C], BF16, isOutput=False)
    onesp = nc.declare_dram_parameter("onesp", [1, 128], F32R, isOutput=False)
    ones16 = nc.declare_dram_parameter("ones16", [128, 8], BF16, isOutput=False)
    out = nc.declare_dram_parameter("out", [N, C], F32, isOutput=True)

    with TileContext(nc) as tc:
      for _rep in range(reps):
        with tc.tile_pool(name="persist", bufs=1) as pp:
            qkT = [pp.tile([128, N], BF16, tag=f"qkT{t}", name=f"qkT{t}") for t in range(8)]
            vN = [pp.tile([128, HPC * 65], BF16, tag=f"vN{t}", name=f"vN{t}") for t in range(NT)]
            oT = [pp.tile([128, N], BF16, tag=f"oT{t}", name=f"oT{t}") for t in range(4)]
            owt_sb = [pp.tile([128, C], BF16, tag=f"owt{hc}", name=f"owt{hc}") for hc in range(4)]
            cos_sb = pp.tile([128, N], BF16, tag="cos_sb", name="cos_sb")
            sin_sb = pp.tile([128, N], BF16, tag="sin_sb", name="sin_sb")
            mask_sb = pp.tile([128, 128], BF16, tag="mask_sb", name="mask_sb")
            ident_sb = pp.tile([128, 128], BF16, tag="ident_sb", name="ident_sb")
            bqk_sb = pp.tile([128, 8], F32, tag="bqk_sb", name="bqk_sb")
            bv_sb = pp.tile([1, JQK], F32R, tag="bv_sb", name="bv_sb")
            ones_sb = pp.tile([1, 128], F32R, tag="ones_sb", name="ones_sb")
            ones16_sb = pp.tile([128, 8], BF16, tag="ones16_sb", name="ones16_sb")
            xts = [pp.tile([128, N], BF16, tag=f"xt{cch}", name=f"xt{cch}") for cch in range(CC)]
            wt_sb = [pp.tile([128, 1536], BF16, tag=f"wt{cch}", name=f"wt{cch}") for cch in range(CC)]

            # input DMAs in consumption order: per chunk wt then two halves
            # of xt (half pieces advance the chunk-major prefix earlier
            # without blowing the serial HWDGE desc-gen budget); tables
            # after the chunks; owt last (first consumed ~80us in).
            for cch in range(CC):
                nc.sync.dma_start(out=wt_sb[cch][:, :], in_=wt[cch, :, :])
                for nbp in range(2):
                    nc.sync.dma_start(
                        out=xts[cch][:, nbp * 1024 : (nbp + 1) * 1024],
                        in_=xt[cch, :, nbp * 1024 : (nbp + 1) * 1024],
                    )
                if cch == 1:
                    nc.sync.dma_start(out=bqk_sb[:, :], in_=bqk[:, :])
                    nc.sync.dma_start(out=ones16_sb[:, :], in_=ones16[:, :])
                    nc.sync.dma_start(out=bv_sb[:, :], in_=bv[:, :])
                    nc.sync.dma_start(out=ones_sb[:, :], in_=onesp[:, :])
            nc.sync.dma_start(out=cos_sb[:, :], in_=cosb[:, :])
            nc.sync.dma_start(out=sin_sb[:, :], in_=sinb[:, :])
            nc.sync.dma_start(out=mask_sb[:, :], in_=maskb[:, :])
            nc.sync.dma_start(out=ident_sb[:, :], in_=identb[:, :])
            for hc in range(4):
                nc.sync.dma_start(out=owt_sb[hc][:, :], in_=owt[hc, :, :])

            # gpsimd: library + the ones column (col 64 of each head group)
            # for every v tile - independent of the v projections
            nc.gpsimd.load_library(library_config.attn)
            for t in range(NT):
                nc.gpsimd.tensor_copy(
                    out=vN[t][:, 64 : HPC * 65 : 65], in_=ones16_sb[:, :]
                )

            with tc.tile_pool(name="rope", bufs=2) as rp:
                sw_cache = {}

                def get_sw(jt):
                    # one sw tile per jt, shared by its rope block-items;
                    # 2 rotating buffers (jt usage windows are sequential)
                    if jt not in sw_cache:
                        sw_cache[jt] = rp.tile(
                            [128, N], BF16, tag="swf", name=f"swf{jt}", bufs=3
                        )
                    return sw_cache[jt]

                def rope_block(jt, nbp):
                    # r[a] = q[a]*cos[a] + q[a^1]*sinSigned[a] per 32-block,
                    # applied to one 512-wide n-block so the first consumer
                    # never waits on a full-row DVE chain
                    sw = get_sw(jt)
                    s = slice(nbp * 512, (nbp + 1) * 512)
                    for a in range(4):
                        b = (a ^ 1) * 32
                        nc.vector.tensor_mul(
                            sw[a * 32 : a * 32 + 32, s],
                            qkT[jt][b : b + 32, s],
                            sin_sb[b : b + 32, s],
                        )
                    nc.vector.tensor_mul(qkT[jt][:, s], qkT[jt][:, s], cos_sb[:, s])
                    nc.vector.tensor_add(qkT[jt][:, s], qkT[jt][:, s], sw[:, s])

                # ---- prefix: pair 0 (q=jt0, k=jt4) chunk-major across 8
                # PSUM banks so PE tracks the chunk DMA stream ----
                with tc.tile_pool(name="prefix_ps", bufs=1, space="PSUM") as pfx:
                    # 6 banks only: the other two stay untouched so the
                    # fill pool's first chains never wait on prefix evacs
                    pf = {
                        (jt, nbp): pfx.tile(
                            [128, 512], F32, tag=f"pf{jt}_{nbp}", name=f"pf{jt}_{nbp}"
                        )
                        for jt in (0, 4)
                        for nbp in range(3)
                    }
                    _lbl("prefix")
                    for cch in range(CC):
                        for nbp in range(3):
                            for jt in (0, 4):
                                nc.tensor.matmul(
                                    pf[(jt, nbp)][:, :],
                                    wt_sb[cch][:, jt * 128 : (jt + 1) * 128],
                                    xts[cch][:, nbp * 512 : (nbp + 1) * 512],
                                    start=(cch == 0),
                                    stop=(cch == CC - 1),
                                )
                    # all evacs first (each frees a PSUM bank; keeps the DVE
                    # queue short ahead of the v evacuations), then only the
                    # nb0 rope blocks -- the rest run after v0-3 below
                    for nbp in range(3):
                        for jt in (0, 4):
                            nc.scalar.activation(
                                qkT[jt][:, nbp * 512 : (nbp + 1) * 512],
                                pf[(jt, nbp)][:, :],
                                mybir.ActivationFunctionType.Identity,
                                bias=bqk_sb[:, jt : jt + 1],
                            )
                    # nb3 chain-major (all chunks present by now)
                    for jt in (0, 4):
                        ps3 = pfx.tile([128, 512], F32, tag="pf0_0", name=f"pf3_{jt}")
                        for cch in range(CC):
                            nc.tensor.matmul(
                                ps3[:, :],
                                wt_sb[cch][:, jt * 128 : (jt + 1) * 128],
                                xts[cch][:, 1536:2048],
                                start=(cch == 0),
                                stop=(cch == CC - 1),
                            )
                        nc.scalar.activation(
                            qkT[jt][:, 1536:2048],
                            ps3[:, :],
                            mybir.ActivationFunctionType.Identity,
                            bias=bqk_sb[:, jt : jt + 1],
                        )
                    rope_block(0, 0)
                    rope_block(4, 0)

                # ---- fused attention + fill stream ----
                with (
                    tc.tile_pool(name="attn_ps", bufs=2, space="PSUM") as sp,
                    tc.tile_pool(name="o_ps", bufs=2, space="PSUM") as op,
                    tc.tile_pool(name="fill_ps", bufs=2, space="PSUM") as fp,
                    tc.tile_pool(name="pt_pool", bufs=8) as ptp,
                    tc.tile_pool(name="znorm", bufs=4) as zp,
                    tc.tile_pool(name="onsb_pool", bufs=12) as obp,
                    tc.tile_pool(name="ostage", bufs=6) as osg,
                ):
                    # ---------------- fill queue machinery ----------------
                    v_cache = {}

                    def emit_v_half(t, half):
                        _lbl(f"fill_v{t}")
                        if half == 0:
                            v_cache[t] = fp.tile([128, 512], F32, tag="fill", name=f"psv_{t}")
                        psv = v_cache[t]
                        for cch in range(4 * half, 4 * half + 4):
                            nc.tensor.matmul(
                                psv[:, :],
                                xts[cch][:, t * 128 : (t + 1) * 128],
                                wt_sb[cch][:, 1024:1536],
                                start=(cch == 0),
                                stop=False,
                            )
                        if half == 0:
                            return
                        nc.tensor.matmul(
                            psv[:, :],
                            r(ones_sb[:, 0:128]),
                            r(bv_sb[:, :]),
                            start=False,
                            stop=True,
                        )
                        nc.scalar.copy(
                            vN[t].rearrange("p (h e) -> p h e", e=65)[:, :, 0:64],
                            psv[:, :].rearrange("p (h d) -> p h d", d=64),
                        )

                    def emit_v(t):
                        emit_v_half(t, 0)
                        emit_v_half(t, 1)

                    def emit_qk(jt, nbp):
                        _lbl(f"fill_qk{jt}_{nbp}")
                        ps = fp.tile([128, 512], F32, tag="fill", name=f"psqk_{jt}_{nbp}")
                        for cch in range(CC):
                            nc.tensor.matmul(
                                ps[:, :],
                                wt_sb[cch][:, jt * 128 : (jt + 1) * 128],
                                xts[cch][:, nbp * 512 : (nbp + 1) * 512],
                                start=(cch == 0),
                                stop=(cch == CC - 1),
                            )
                        nc.scalar.activation(
                            qkT[jt][:, nbp * 512 : (nbp + 1) * 512],
                            ps[:, :],
                            mybir.ActivationFunctionType.Identity,
                            bias=bqk_sb[:, jt : jt + 1],
                        )

                    opart = {}

                    def emit_outproj_i(i, cb, mode="full"):
                        # mode="part": accumulate head-groups 0-2 into SBUF
                        # early; "fin": tail does only the hc3 matmul + add
                        _lbl(f"outproj{i}_{cb}")
                        pso = fp.tile([128, 512], F32, tag="fill", name=f"pso_{i}_{cb}_{mode}")
                        hcs = {"full": (0, 4), "part": (0, 3), "fin": (3, 4)}[mode]
                        for hc in range(*hcs):
                            nc.tensor.matmul(
                                pso[:, :],
                                oT[hc][:, i * 128 : (i + 1) * 128],
                                owt_sb[hc][:, cb * 512 : (cb + 1) * 512],
                                start=(hc == hcs[0]),
                                stop=(hc == hcs[1] - 1),
                            )
                        if mode == "part":
                            pa = pp.tile([128, 512], F32, tag=f"opart{i % 4}_{cb}", name=f"opart_{i}_{cb}")
                            nc.vector.tensor_copy(out=pa[:, :], in_=pso[:, :])
                            opart[(i, cb)] = pa
                            return
                        ost = osg.tile([128, 512], F32, tag="ost", name=f"ost_{i}_{cb}")
                        if mode == "fin":
                            nc.vector.tensor_add(ost[:, :], pso[:, :], opart[(i, cb)][:, :])
                        else:
                            nc.vector.tensor_copy(out=ost[:, :], in_=pso[:, :])
                        nc.sync.dma_start(
                            out=out[i * 128 : (i + 1) * 128, cb * 512 : (cb + 1) * 512],
                            in_=ost[:, :],
                        )

                    # v tiles 0-3 (needed by the first attention block) and
                    # the remaining pair-0 rope blocks run before attention;
                    # v evacs land early in the DVE queue
                    for t in range(4):
                        emit_v(t)
                    for nbp in range(1, NB):
                        rope_block(0, nbp)
                        rope_block(4, nbp)

                    fill = []  # (level, marker_key_or_None, emitfn)
                    qk_cache = {}

                    def emit_qk_half(jt, nbp, half):
                        # half-chains (4 chunks) share one accumulating pso
                        # bank; finer items track the exp clock more smoothly
                        _lbl(f"fill_qk{jt}_{nbp}")
                        if half == 0:
                            qk_cache[(jt, nbp)] = fp.tile(
                                [128, 512], F32, tag="fill", name=f"psqk_{jt}_{nbp}"
                            )
                        ps = qk_cache[(jt, nbp)]
                        for cch in range(4 * half, 4 * half + 4):
                            nc.tensor.matmul(
                                ps[:, :],
                                wt_sb[cch][:, jt * 128 : (jt + 1) * 128],
                                xts[cch][:, nbp * 512 : (nbp + 1) * 512],
                                start=(cch == 0),
                                stop=(cch == CC - 1),
                            )
                        if half == 1:
                            nc.scalar.activation(
                                qkT[jt][:, nbp * 512 : (nbp + 1) * 512],
                                ps[:, :],
                                mybir.ActivationFunctionType.Identity,
                                bias=bqk_sb[:, jt : jt + 1],
                            )

                    for t in range(4, NT):
                        fill.append((0, None, lambda t=t: emit_v_half(t, 0)))
                        fill.append((0, ("v", t), lambda t=t: emit_v_half(t, 1)))
                    for p in range(1, 4):
                        # qk chain for one n-block, then its rope right away
                        # (per-block items keep DVE bursts short so the
                        # mask->exp chain is never delayed long); level p-1
                        # paces pair p's chains into head-pair p-1's loop so
                        # late head-pairs keep PE fill against the exp clock
                        for jt in (p, 4 + p):
                            for nbp in range(NB):
                                fill.append(
                                    (p - 1, None, lambda jt=jt, nbp=nbp: emit_qk_half(jt, nbp, 0))
                                )
                                def qk_fin(jt=jt, nbp=nbp):
                                    emit_qk_half(jt, nbp, 1)
                                    rope_block(jt, nbp)
                                fill.append(
                                    (
                                        p - 1,
                                        ("pair", p) if (jt >= 4 and nbp == NB - 1) else None,
                                        qk_fin,
                                    )
                                )

                    state = {"pos": 0}
                    done_markers = set()

                    def drain_one(cap):
                        if state["pos"] < len(fill):
                            lev, key, fn = fill[state["pos"]]
                            if lev > cap:
                                return
                            state["pos"] += 1
                            fn()
                            if key is not None:
                                done_markers.add(key)

                    def drain_until(key):
                        if key in done_markers:
                            return
                        while state["pos"] < len(fill):
                            _lev, k, fn = fill[state["pos"]]
                            state["pos"] += 1
                            fn()
                            if k is not None:
                                done_markers.add(k)
                            if k == key:
                                return

                    def emit_pv(nb, hp, onat, pend, last):
                        pj, ppt, poff2, pw, pooff = pend
                        r0 = pooff // 128
                        for i in range(r0, 4):
                            g, il = i // 2, i % 2
                            # column of q-tile i inside the score window
                            cs = i * 128 - pooff + poff2
                            # bank g's final write happens at the diagonal
                            # j-tile that still covers q-tile g*2+1
                            for hh in range(2):
                                h = 2 * hp + hh
                                nc.tensor.matmul(
                                    onat[g][:, il * 130 + hh * 65 : il * 130 + hh * 65 + 65],
                                    ppt[:, hh * 512 + cs : hh * 512 + cs + 128],
                                    vN[pj][:, h * 65 : h * 65 + 65],
                                    start=(pj == 0 and hh == 0 and il == 0),
                                    stop=(pj == 4 * nb + 2 * g + 1 and hh == 1 and i == g * 2 + 1),
                                    skip_group_check=True,
                                )

                    # ---------------- attention ----------------
                    # pass 1: hp0 then hp1 across all blocks (projection fill
                    # drains here); pass 2 interleaves hp2/hp3 per block so
                    # each block's out-proj becomes fill right after its hp3
                    schedule = [(0, nb) for nb in range(NB)]
                    schedule += [(1, nb) for nb in range(NB)]
                    for nb in range(NB):
                        schedule += [(2, nb), (3, nb)]
                    def flush_pend(pend):
                        # PV for the pending iteration; when it closes a
                        # block, emit that block's normalization too (this
                        # runs AFTER the next block's first scores, so the
                        # exp stream never drains at block boundaries)
                        onat, nb, hp, pj, ppt, poff2, pw, pooff, is_last = pend
                        _lbl(f"pv{hp}_{nb}_{pj}")
                        emit_pv(nb, hp, onat, (pj, ppt, poff2, pw, pooff), last=is_last)
                        if not is_last:
                            return
                        # normalization in natural layout: per-partition 1/Z
                        # broadcast along free dim; the PE transposes that
                        # rebuild oT are deferred as fill items
                        for g in range(2):
                            rzq = zp.tile([128, 4], F32, tag="rz", name=f"rz_{nb}_{hp}_{g}")
                            nc.vector.reciprocal(
                                rzq[:, :], onat[g][:, 64:260:65]
                            )
                            for il in range(2):
                                i = g * 2 + il
                                onsb = obp.tile(
                                    [128, 128], BF16, tag="onsb", name=f"onsb_{nb}_{hp}_{i}"
                                )
                                nc.vector.tensor_mul(
                                    onsb[:, :].rearrange("p (h e) -> p h e", e=64),
                                    onat[g][:, il * 130 : il * 130 + 130].rearrange(
                                        "p (h e) -> p h e", e=65
                                    )[:, :, 0:64],
                                    rzq[:, il * 2 : il * 2 + 2, None].broadcast_to([128, 2, 64]),
                                )

                                def tp_item(nb=nb, hp=hp, i=i, onsb=onsb):
                                    _lbl(f"tp{hp}_{nb}_{i}")
                                    tp = fp.tile([128, 128], BF16, tag="fill", name=f"tp_{nb}_{hp}_{i}")
                                    nc.tensor.transpose(tp[:, :], onsb[:, :], ident_sb[:, :])
                                    nc.vector.tensor_copy(
                                        out=oT[hp][:, nb * 512 + i * 128 : nb * 512 + (i + 1) * 128],
                                        in_=tp[:, :],
                                    )
                                # front of the pending queue: must drain
                                # within the next block so onat/onsb
                                # buffers recycle on time
                                fill.insert(state["pos"] + 2 * g + il, (0, None, tp_item))
                        if hp == 2 and nb == 3:
                            # last q-block: pre-accumulate head-groups 0-2 so
                            # the tail needs only one matmul + add per chain
                            for i in range(12, 16):
                                for cb in range(2):
                                    fill.append(
                                        (0, None, lambda i=i, cb=cb: emit_outproj_i(i, cb, "part"))
                                    )
                        if hp == 3:
                            # out-proj for q-block nb, one block behind
                            for i in range(4 * nb, 4 * nb + 4):
                                for cb in range(2):
                                    mode = "fin" if nb == 3 else "full"
                                    fill.append(
                                        (0, None, lambda i=i, cb=cb, mode=mode: emit_outproj_i(i, cb, mode))
                                    )

                    pend = None  # carries the score->exp->PV pipeline
                    # across block boundaries
                    for hp, nb in schedule:
                        if hp > 0:
                            drain_until(("pair", hp))
                        if True:
                            if 4 * nb + 3 >= 4:
                                drain_until(("v", 4 * nb + 3))
                            # natural-layout PV accumulators: one PSUM bank
                            # per 2 q-tiles; col(i%2, h, d) = (i%2)*130+h*65+d
                            # (col 64 of each 65-group is the Z denominator)
                            onat = [
                                op.tile([128, 512], F32, tag="on", name=f"on_{nb}_{hp}_{g}")
                                for g in range(2)
                            ]
                            for j in range(4 * nb + 4):
                                if j // 4 == nb:
                                    qoff = j * 128
                                    w = 512 * (nb + 1) - qoff
                                else:
                                    qoff, w = nb * 512, 512
                                # diag tiles: score/exp only the causal width
                                # w of each head's half; qbase clamp keeps the
                                # window in-bounds at the tail (nb=3), where
                                # the causal range sits at [off2, off2+w)
                                qbase = min(qoff, N - 512)
                                off2 = qoff - qbase
                                ooff = qoff - 512 * nb
                                st = sp.tile([128, 1024], F32, tag="st", name=f"st_{nb}_{hp}_{j}")
                                _lbl(f"score{hp}_{nb}_{j}")
                                dg = j // 4 == nb
                                for hh in range(2):
                                    nc.tensor.matmul(
                                        st[:, hh * 512 + off2 : hh * 512 + off2 + w],
                                        qkT[4 + hp][hh * 64 : hh * 64 + 64, j * 128 : (j + 1) * 128],
                                        qkT[hp][hh * 64 : hh * 64 + 64, qbase + off2 : qbase + off2 + w],
                                        start=True,
                                        stop=not dg,
                                    )
                                if dg:
                                    # causal mask on PE: accumulate the 0/-1e9
                                    # triangle table through an identity lhsT
                                    # (keeps DVE out of the exp chain)
                                    for hh in range(2):
                                        nc.tensor.matmul(
                                            st[:, hh * 512 + off2 : hh * 512 + off2 + 128],
                                            ident_sb[:, :],
                                            mask_sb[:, :],
                                            start=False,
                                            stop=True,
                                            skip_group_check=True,
                                        )
                                pt = ptp.tile([128, 1024], BF16, tag="pt", name=f"pt_{nb}_{hp}_{j}")
                                if w < 512:
                                    nc.scalar.activation(
                                        pt.rearrange("p (b q) -> p b q", b=2)[:, :, off2 : off2 + w],
                                        st[:, 0:1024].rearrange("p (b q) -> p b q", b=2)[:, :, off2 : off2 + w],
                                        mybir.ActivationFunctionType.Exp,
                                    )
                                else:
                                    nc.scalar.activation(
                                        pt[:, :],
                                        st[:, :],
                                        mybir.ActivationFunctionType.Exp,
                                    )
                                if pend is not None:
                                    flush_pend(pend)
                                pend = (onat, nb, hp, j, pt, off2, w, ooff, j == 4 * nb + 3)
                                drain_one(0 if hp == 0 else 99)
                    flush_pend(pend)
                    while state["pos"] < len(fill):
                        drain_one(99)
    nc.compile()
    return nc


def make_in_maps(x, Wqkv_w, Wqkv_b, out_w):
    """Host-side sharding/layout prep. Returns per-core input dicts."""
    in_maps = []
    # deinterleave perm within one head: even rope components then odd
    perm = np.concatenate([np.arange(0, D, 2), np.arange(1, D, 2)])
    # rope tables
    inv = 1.0 / (ROPE_THETA ** (np.arange(0, D, 2, dtype=np.float64) / D))
    ang = np.arange(N, dtype=np.float64)[:, None] * inv[None, :]  # [N, 32]
    cosT = np.cos(ang).T.astype(np.float32)  # [32, N]
    sinT = np.sin(ang).T.astype(np.float32)
    cosb = np.tile(cosT, (4, 1))  # [128, N]
    sinb = np.concatenate([sinT, -sinT, sinT, -sinT], axis=0)  # [128, N], block a holds out-block a^1's signed sin
    qc, kc = np.arange(128), np.arange(128)
    maskp = np.where(qc[None, :] >= kc[:, None], 0.0, NEG).astype(np.float32)
    identp = np.eye(128, dtype=np.float32)

    for c in range(8):
        b, g = c // 2, c % 2
        heads = np.arange(g * HPC, (g + 1) * HPC)
        qk_rows = (heads[:, None] * D + perm[None, :]).reshape(-1)  # [512]
        v_rows = (heads[:, None] * D + np.arange(D)[None, :]).reshape(-1)
        Wq = Wqkv_w[qk_rows] * SCALE
        bq = Wqkv_b[qk_rows] * SCALE
        Wk = Wqkv_w[C + qk_rows]
        bk = Wqkv_b[C + qk_rows]
        Wv = Wqkv_w[2 * C + v_rows]
        bv = Wqkv_b[2 * C + v_rows]
        Wcat = np.concatenate([Wq, Wk, Wv], axis=0)  # [1536, C]
        wt = np.ascontiguousarray(Wcat.T).reshape(CC, 128, 1536)
        xt = np.ascontiguousarray(x[b].T).reshape(CC, 128, N)
        bqk = np.ascontiguousarray(
            np.concatenate([bq, bk]).reshape(8, 128).T
        )  # [128, 8]
        owt = np.ascontiguousarray(out_w[:, g * JQK : (g + 1) * JQK].T).reshape(
            4, 128, C
        )
        import ml_dtypes
        in_maps.append(
            dict(
                onesp=np.ones((1, 128), dtype=np.float32),
                ones16=np.ones((128, 8), dtype=ml_dtypes.bfloat16),
                xt=xt.astype(ml_dtypes.bfloat16),
                wt=wt.astype(ml_dtypes.bfloat16),
                bqk=bqk.astype(np.float32),
                bv=np.ascontiguousarray(bv[None, :]).astype(np.float32),
                cosb=cosb.astype(ml_dtypes.bfloat16),
                sinb=sinb.astype(ml_dtypes.bfloat16),
                maskb=maskp.astype(ml_dtypes.bfloat16),
                identb=identp.astype(ml_dtypes.bfloat16),
                owt=owt.astype(ml_dtypes.bfloat16),
            )
        )
    return in_maps


_CACHED_NC = None


def kernel(x, Wqkv_w, Wqkv_b, out_w, out_b):
    from concourse.bass_utils import run_bass_kernel_spmd

    global _CACHED_NC
    x = np.asarray(x, dtype=np.float32)
    Wqkv_w = np.asarray(Wqkv_w, dtype=np.float32)
    Wqkv_b = np.asarray(Wqkv_b, dtype=np.float32)
    out_w = np.asarray(out_w, dtype=np.float32)
    out_b = np.asarray(out_b, dtype=np.float32)

    if _CACHED_NC is None:
        _CACHED_NC = build_nc()
    nc = _CACHED_NC
    in_maps = make_in_maps(x, Wqkv_w, Wqkv_b, out_w)
    res = run_bass_kernel_spmd(nc, in_maps, core_ids=list(range(8)))
    out = np.empty((B, N, C), dtype=np.float32)
    for b in range(B):
        out[b] = res.results[2 * b]["out"] + res.results[2 * b + 1]["out"] + out_b
    return out


# revision 49
# speedup vs baseline: 1.3552x; 1.0376x over previous
"""Trainium2 Bass kernel for a causal multi-head attention block
(fused QKV proj + RoPE + causal softmax attention + out proj).

Sharding: 8 cores = 4 batches x 2 head-groups (8 heads each), no
on-chip collectives: each core emits a partial out-projection [N, C]
(row-parallel over heads); the host sums each batch's pair of partials
and adds the output bias.

Schedule (single fused stream, PE never phase-barriers):
  - Prefix: chunk-major projection of q0/k0 (pair 0) across 8 PSUM
    banks while the wt/xt chunks stream in, then v tiles 0-3.
  - Attention runs head-pair-outer / q-block-inner. All remaining
    projection work (v4-15, q/k pairs 1-3, their RoPE) lives in a fill
    queue drained one item per k-tile iteration, so the PE pipeline
    stays dense while ACT's exp stream (the per-iteration clock) runs.
  - Scores S^T[k, q] for both heads of a pair row-packed into one
    [128,1024] PSUM tile; causal-trimmed on diagonal tiles; one wide
    exp -> bf16 P^T; P^T @ [v|1] accumulates o^T and the softmax
    denominator Z per head.
  - Normalization is entirely off the PE path: o_ps evacuates to SBUF
    (bf16) immediately (PSUM recycles in <1us), then DVE recip ->
    gpsimd partition-broadcast -> DVE multiply produce oT in bf16.
  - Out-proj (bf16) for q-block nb is enqueued as fill during the
    last head-pair, one block behind its norm, and drains at the tail.
Scores/PV/projections in bf16 (f32 PSUM accumulation); v-bias via a
K=1 ones-matmul; q pre-scaled by D^-0.5 on the host.
"""

import sys

sys.path.insert(0, "/opt/trn_rl_repo")

import numpy as np

import concourse.bass as bass
import concourse.mybir as mybir
from concourse import bacc, library_config
from concourse.tile import TileContext

F32 = mybir.dt.float32
F32R = mybir.dt.float32r
BF16 = mybir.dt.bfloat16

B, N, C = 4, 2048, 1024
H_ALL, D = 16, 64
HPC = 8  # heads per core
JQK = HPC * D  # 512 rows for q (and k) per core
ROPE_THETA = 10000.0
SCALE = D**-0.5
NEG = -1e9

NT = N // 128  # 16 n-tiles
NB = N // 512  # 4 n-blocks
CC = C // 128  # 8 contraction chunks


def r(ap):
    return ap.bitcast(F32R)


PE_LABELS = []
_CUR = ["?"]


def _lbl(s):
    _CUR[0] = s


def build_nc(reps=1):
    PE_LABELS.clear()
    nc = bacc.Bacc(None, target_bir_lowering=False)
    _orig_mm = nc.tensor.matmul

    def _mm(*a, **k):
        PE_LABELS.append(_CUR[0])
        return _orig_mm(*a, **k)

    nc.tensor.matmul = _mm

    xt = nc.declare_dram_parameter("xt", [CC, 128, N], BF16, isOutput=False)
    wt = nc.declare_dram_parameter("wt", [CC, 128, 1536], BF16, isOutput=False)
    bqk = nc.declare_dram_parameter("bqk", [128, 8], F32, isOutput=False)
    bv = nc.declare_dram_parameter("bv", [1, JQK], F32R, isOutput=False)
    cosb = nc.declare_dram_parameter("cosb", [128, N], BF16, isOutput=False)
    sinb = nc.declare_dram_parameter("sinb", [128, N], BF16, isOutput=False)
    maskb = nc.declare_dram_parameter("maskb", [128, 128], BF16, isOutput=False)
    identb = nc.declare_dram_parameter("identb", [128, 128], BF16, isOutput=False)
    owt = nc.declare_dram_parameter("owt", [4, 128, C], BF16, isOutput=False)
    onesp = nc.declare_dram_parameter("onesp", [1, 128], F32R, isOutput=False)
    ones16 = nc.declare_dram_parameter("ones16", [128, 8], BF16, isOutput=False)
    out = nc.declare_dram_parameter("out", [N, C], F32, isOutput=True)

    with TileContext(nc) as tc:
      for _rep in range(reps):
        with tc.tile_pool(name="persist", bufs=1) as pp:
            qkT = [pp.tile([128, N], BF16, tag=f"qkT{t}", name=f"qkT{t}") for t in range(8)]
            vN = [pp.tile([128, HPC * 65], BF16, tag=f"vN{t}", name=f"vN{t}") for t in range(NT)]
            oT = [pp.tile([128, N], BF16, tag=f"oT{t}", name=f"oT{t}") for t in range(4)]
            owt_sb = [pp.tile([128, C], BF16, tag=f"owt{hc}", name=f"owt{hc}") for hc in range(4)]
            cos_sb = pp.tile([128, N], BF16, tag="cos_sb", name="cos_sb")
            sin_sb = pp.tile([128, N], BF16, tag="sin_sb", name="sin_sb")
            mask_sb = pp.tile([128, 128], BF16, tag="mask_sb", name="mask_sb")
            ident_sb = pp.tile([128, 128], BF16, tag="ident_sb", name="ident_sb")
            bqk_sb = pp.tile([128, 8], F32, tag="bqk_sb", name="bqk_sb")
            bv_sb = pp.tile([1, JQK], F32R, tag="bv_sb", name="bv_sb")
            ones_sb = pp.tile([1, 128], F32R, tag="ones_sb", name="ones_sb")
            ones16_sb = pp.tile([128, 8], BF16, tag="ones16_sb", name="ones16_sb")
            xts = [pp.tile([128, N], BF16, tag=f"xt{cch}", name=f"xt{cch}") for cch in range(CC)]
            wt_sb = [pp.tile([128, 1536], BF16, tag=f"wt{cch}", name=f"wt{cch}") for cch in range(CC)]

            # input DMAs in consumption order: per chunk wt then two halves
            # of xt (half pieces advance the chunk-major prefix earlier
            # without blowing the serial HWDGE desc-gen budget); tables
            # after the chunks; owt last (first consumed ~80us in).
            for cch in range(CC):
                nc.sync.dma_start(out=wt_sb[cch][:, :], in_=wt[cch, :, :])
                for nbp in range(2):
                    nc.sync.dma_start(
                        out=xts[cch][:, nbp * 1024 : (nbp + 1) * 1024],
                        in_=xt[cch, :, nbp * 1024 : (nbp + 1) * 1024],
                    )
                if cch == 1:
                    nc.sync.dma_start(out=bqk_sb[:, :], in_=bqk[:, :])
                    nc.sync.dma_start(out=ones16_sb[:, :], in_=ones16[:, :])
                    nc.sync.dma_start(out=bv_sb[:, :], in_=bv[:, :])
                    nc.sync.dma_start(out=ones_sb[:, :], in_=onesp[:, :])
            nc.sync.dma_start(out=cos_sb[:, :], in_=cosb[:, :])
            nc.sync.dma_start(out=sin_sb[:, :], in_=sinb[:, :])
            nc.sync.dma_start(out=mask_sb[:, :], in_=maskb[:, :])
            nc.sync.dma_start(out=ident_sb[:, :], in_=identb[:, :])
            for hc in range(4):
                nc.sync.dma_start(out=owt_sb[hc][:, :], in_=owt[hc, :, :])

            # gpsimd: library + the ones column (col 64 of each head group)
            # for every v tile - independent of the v projections
            nc.gpsimd.load_library(library_config.attn)
            for t in range(NT):
                nc.gpsimd.tensor_copy(
                    out=vN[t][:, 64 : HPC * 65 : 65], in_=ones16_sb[:, :]
                )

            with tc.tile_pool(name="rope", bufs=2) as rp:
                sw_cache = {}

                def get_sw(jt):
                    # one sw tile per jt, shared by its rope block-items;
                    # 2 rotating buffers (jt usage windows are sequential)
                    if jt not in sw_cache:
                        sw_cache[jt] = rp.tile(
                            [128, N], BF16, tag="swf", name=f"swf{jt}", bufs=3
                        )
                    return sw_cache[jt]

                def rope_block(jt, nbp):
                    # r[a] = q[a]*cos[a] + q[a^1]*sinSigned[a] per 32-block,
                    # applied to one 512-wide n-block so the first consumer
                    # never waits on a full-row DVE chain
                    sw = get_sw(jt)
                    s = slice(nbp * 512, (nbp + 1) * 512)
                    for a in range(4):
                        b = (a ^ 1) * 32
                        nc.vector.tensor_mul(
                            sw[a * 32 : a * 32 + 32, s],
                            qkT[jt][b : b + 32, s],
                            sin_sb[b : b + 32, s],
                        )
                    nc.vector.tensor_mul(qkT[jt][:, s], qkT[jt][:, s], cos_sb[:, s])
                    nc.vector.tensor_add(qkT[jt][:, s], qkT[jt][:, s], sw[:, s])

                # ---- prefix: pair 0 (q=jt0, k=jt4) chunk-major across 8
                # PSUM banks so PE tracks the chunk DMA stream ----
                with tc.tile_pool(name="prefix_ps", bufs=1, space="PSUM") as pfx:
                    # 6 banks only: the other two stay untouched so the
                    # fill pool's first chains never wait on prefix evacs
                    pf = {
                        (jt, nbp): pfx.tile(
                            [128, 512], F32, tag=f"pf{jt}_{nbp}", name=f"pf{jt}_{nbp}"
                        )
                        for jt in (0, 4)
                        for nbp in range(3)
                    }
                    _lbl("prefix")
                    for cch in range(CC):
                        for nbp in range(3):
                            for jt in (0, 4):
                                nc.tensor.matmul(
                                    pf[(jt, nbp)][:, :],
                                    wt_sb[cch][:, jt * 128 : (jt + 1) * 128],
                                    xts[cch][:, nbp * 512 : (nbp + 1) * 512],
                                    start=(cch == 0),
                                    stop=(cch == CC - 1),
                                )
                    # all evacs first (each frees a PSUM bank; keeps the DVE
                    # queue short ahead of the v evacuations), then only the
                    # nb0 rope blocks -- the rest run after v0-3 below
                    for nbp in range(3):
                        for jt in (0, 4):
                            nc.scalar.activation(
                                qkT[jt][:, nbp * 512 : (nbp + 1) * 512],
                                pf[(jt, nbp)][:, :],
                                mybir.ActivationFunctionType.Identity,
                                bias=bqk_sb[:, jt : jt + 1],
                            )
                    # nb3 chain-major (all chunks present by now)
                    for jt in (0, 4):
                        ps3 = pfx.tile([128, 512], F32, tag="pf0_0", name=f"pf3_{jt}")
                        for cch in range(CC):
                            nc.tensor.matmul(
                                ps3[:, :],
                                wt_sb[cch][:, jt * 128 : (jt + 1) * 128],
                                xts[cch][:, 1536:2048],
                                start=(cch == 0),
                                stop=(cch == CC - 1),
                            )
                        nc.scalar.activation(
                            qkT[jt][:, 1536:2048],
                            ps3[:, :],
                            mybir.ActivationFunctionType.Identity,
                            bias=bqk_sb[:, jt : jt + 1],
                        )
                    rope_block(0, 0)
                    rope_block(4, 0)

                # ---- fused attention + fill stream ----
                with (
                    tc.tile_pool(name="attn_ps", bufs=2, space="PSUM") as sp,
                    tc.tile_pool(name="o_ps", bufs=2, space="PSUM") as op,
                    tc.tile_pool(name="fill_ps", bufs=2, space="PSUM") as fp,
                    tc.tile_pool(name="pt_pool", bufs=8) as ptp,
                    tc.tile_pool(name="znorm", bufs=4) as zp,
                    tc.tile_pool(name="onsb_pool", bufs=12) as obp,
                    tc.tile_pool(name="ostage", bufs=6) as osg,
                ):
                    # ---------------- fill queue machinery ----------------
                    v_cache = {}

                    def emit_v_half(t, half):
                        _lbl(f"fill_v{t}")
                        if half == 0:
                            v_cache[t] = fp.tile([128, 512], F32, tag="fill", name=f"psv_{t}")
                        psv = v_cache[t]
                        for cch in range(4 * half, 4 * half + 4):
                            nc.tensor.matmul(
                                psv[:, :],
                                xts[cch][:, t * 128 : (t + 1) * 128],
                                wt_sb[cch][:, 1024:1536],
                                start=(cch == 0),
                                stop=False,
                            )
                        if half == 0:
                            return
                        nc.tensor.matmul(
                            psv[:, :],
                            r(ones_sb[:, 0:128]),
                            r(bv_sb[:, :]),
                            start=False,
                            stop=True,
                        )
                        nc.scalar.copy(
                            vN[t].rearrange("p (h e) -> p h e", e=65)[:, :, 0:64],
                            psv[:, :].rearrange("p (h d) -> p h d", d=64),
                        )

                    def emit_v(t):
                        emit_v_half(t, 0)
                        emit_v_half(t, 1)

                    def emit_qk(jt, nbp):
                        _lbl(f"fill_qk{jt}_{nbp}")
                        ps = fp.tile([128, 512], F32, tag="fill", name=f"psqk_{jt}_{nbp}")
                        for cch in range(CC):
                            nc.tensor.matmul(
                                ps[:, :],
                                wt_sb[cch][:, jt * 128 : (jt + 1) * 128],
                                xts[cch][:, nbp * 512 : (nbp + 1) * 512],
                                start=(cch == 0),
                                stop=(cch == CC - 1),
                            )
                        nc.scalar.activation(
                            qkT[jt][:, nbp * 512 : (nbp + 1) * 512],
                            ps[:, :],
                            mybir.ActivationFunctionType.Identity,
                            bias=bqk_sb[:, jt : jt + 1],
                        )

                    opart = {}
                    op_cache = {}

                    def emit_outproj_i(i, cb, mode="full"):
                        # mode="part": accumulate head-groups 0-2 into SBUF
                        # early; "fin": tail does only the hc3 matmul + add;
                        # "fullA"/"fullB" split a full chain into two items
                        _lbl(f"outproj{i}_{cb}")
                        if mode == "fullA":
                            op_cache[(i, cb)] = fp.tile(
                                [128, 512], F32, tag="fill", name=f"pso_{i}_{cb}"
                            )
                            pso = op_cache[(i, cb)]
                            for hc in range(2):
                                nc.tensor.matmul(
                                    pso[:, :],
                                    oT[hc][:, i * 128 : (i + 1) * 128],
                                    owt_sb[hc][:, cb * 512 : (cb + 1) * 512],
                                    start=(hc == 0),
                                    stop=False,
                                )
                            return
                        if mode == "fullB":
                            pso = op_cache[(i, cb)]
                            for hc in range(2, 4):
                                nc.tensor.matmul(
                                    pso[:, :],
                                    oT[hc][:, i * 128 : (i + 1) * 128],
                                    owt_sb[hc][:, cb * 512 : (cb + 1) * 512],
                                    start=False,
                                    stop=(hc == 3),
                                )
                            ost = osg.tile([128, 512], F32, tag="ost", name=f"ost_{i}_{cb}")
                            nc.vector.tensor_copy(out=ost[:, :], in_=pso[:, :])
                            nc.sync.dma_start(
                                out=out[i * 128 : (i + 1) * 128, cb * 512 : (cb + 1) * 512],
                                in_=ost[:, :],
                            )
                            return
                        pso = fp.tile([128, 512], F32, tag="fill", name=f"pso_{i}_{cb}_{mode}")
                        hcs = {"full": (0, 4), "part": (0, 3), "fin": (3, 4)}[mode]
                        for hc in range(*hcs):
                            nc.tensor.matmul(
                                pso[:, :],
                                oT[hc][:, i * 128 : (i + 1) * 128],
                                owt_sb[hc][:, cb * 512 : (cb + 1) * 512],
                                start=(hc == hcs[0]),
                                stop=(hc == hcs[1] - 1),
                            )
                        if mode == "part":
                            pa = pp.tile([128, 512], F32, tag=f"opart{i % 4}_{cb}", name=f"opart_{i}_{cb}")
                            nc.vector.tensor_copy(out=pa[:, :], in_=pso[:, :])
                            opart[(i, cb)] = pa
                            return
                        ost = osg.tile([128, 512], F32, tag="ost", name=f"ost_{i}_{cb}")
                        if mode == "fin":
                            nc.vector.tensor_add(ost[:, :], pso[:, :], opart[(i, cb)][:, :])
                        else:
                            nc.vector.tensor_copy(out=ost[:, :], in_=pso[:, :])
                        nc.sync.dma_start(
                            out=out[i * 128 : (i + 1) * 128, cb * 512 : (cb + 1) * 512],
                            in_=ost[:, :],
                        )

                    # v tiles 0-3 (needed by the first attention block) and
                    # the remaining pair-0 rope blocks run before attention;
                    # v evacs land early in the DVE queue
                    for t in range(4):
                        emit_v(t)
                    for nbp in range(1, NB):
                        rope_block(0, nbp)
                        rope_block(4, nbp)

                    fill = []  # (level, marker_key_or_None, emitfn)
                    qk_cache = {}

                    def emit_qk_half(jt, nbp, half):
                        # half-chains (4 chunks) share one accumulating pso
                        # bank; finer items track the exp clock more smoothly
                        _lbl(f"fill_qk{jt}_{nbp}")
                        if half == 0:
                            qk_cache[(jt, nbp)] = fp.tile(
                                [128, 512], F32, tag="fill", name=f"psqk_{jt}_{nbp}"
                            )
                        ps = qk_cache[(jt, nbp)]
                        for cch in range(4 * half, 4 * half + 4):
                            nc.tensor.matmul(
                                ps[:, :],
                                wt_sb[cch][:, jt * 128 : (jt + 1) * 128],
                                xts[cch][:, nbp * 512 : (nbp + 1) * 512],
                                start=(cch == 0),
                                stop=(cch == CC - 1),
                            )
                        if half == 1:
                            nc.scalar.activation(
                                qkT[jt][:, nbp * 512 : (nbp + 1) * 512],
                                ps[:, :],
                                mybir.ActivationFunctionType.Identity,
                                bias=bqk_sb[:, jt : jt + 1],
                            )

                    for t in range(4, NT):
                        fill.append((0, None, lambda t=t: emit_v_half(t, 0)))
                        fill.append((0, ("v", t), lambda t=t: emit_v_half(t, 1)))
                    for p in range(1, 4):
                        # qk chain for one n-block, then its rope right away
                        # (per-block items keep DVE bursts short so the
                        # mask->exp chain is never delayed long); level p-1
                        # paces pair p's chains into head-pair p-1's loop so
                        # late head-pairs keep PE fill against the exp clock
                        for jt in (p, 4 + p):
                            for nbp in range(NB):
                                fill.append(
                                    (p - 1, None, lambda jt=jt, nbp=nbp: emit_qk_half(jt, nbp, 0))
                                )
                                def qk_fin(jt=jt, nbp=nbp):
                                    emit_qk_half(jt, nbp, 1)
                                    rope_block(jt, nbp)
                                fill.append(
                                    (
                                        p - 1,
                                        ("pair", p) if (jt >= 4 and nbp == NB - 1) else None,
                                        qk_fin,
                                    )
                                )

                    state = {"pos": 0}
                    done_markers = set()

                    def drain_one(cap):
                        if state["pos"] < len(fill):
                            lev, key, fn = fill[state["pos"]]
                            if lev > cap:
                                return
                            state["pos"] += 1
                            fn()
                            if key is not None:
                                done_markers.add(key)

                    def drain_until(key):
                        if key in done_markers:
                            return
                        while state["pos"] < len(fill):
                            _lev, k, fn = fill[state["pos"]]
                            state["pos"] += 1
                            fn()
                            if k is not None:
                                done_markers.add(k)
                            if k == key:
                                return

                    def emit_pv(nb, hp, onat, pend, last):
                        pj, ppt, poff2, pw, pooff = pend
                        r0 = pooff // 128
                        for i in range(r0, 4):
                            g, il = i // 2, i % 2
                            # column of q-tile i inside the score window
                            cs = i * 128 - pooff + poff2
                            # bank g's final write happens at the diagonal
                            # j-tile that still covers q-tile g*2+1
                            for hh in range(2):
                                h = 2 * hp + hh
                                nc.tensor.matmul(
                                    onat[g][:, il * 130 + hh * 65 : il * 130 + hh * 65 + 65],
                                    ppt[:, hh * 512 + cs : hh * 512 + cs + 128],
                                    vN[pj][:, h * 65 : h * 65 + 65],
                                    start=(pj == 0 and hh == 0 and il == 0),
                                    stop=(pj == 4 * nb + 2 * g + 1 and hh == 1 and i == g * 2 + 1),
                                    skip_group_check=True,
                                )

                    # ---------------- attention ----------------
                    # pass 1: hp0 then hp1 across all blocks (projection fill
                    # drains here); pass 2 interleaves hp2/hp3 per block so
                    # each block's out-proj becomes fill right after its hp3
                    schedule = [(0, nb) for nb in range(NB)]
                    schedule += [(1, nb) for nb in range(NB)]
                    for nb in range(NB):
                        schedule += [(2, nb), (3, nb)]
                    def flush_pend(pend):
                        # PV for the pending iteration; when it closes a
                        # block, emit that block's normalization too (this
                        # runs AFTER the next block's first scores, so the
                        # exp stream never drains at block boundaries)
                        onat, nb, hp, pj, ppt, poff2, pw, pooff, is_last = pend
                        _lbl(f"pv{hp}_{nb}_{pj}")
                        emit_pv(nb, hp, onat, (pj, ppt, poff2, pw, pooff), last=is_last)
                        if not is_last:
                            return
                        # normalization in natural layout: per-partition 1/Z
                        # broadcast along free dim; the PE transposes that
                        # rebuild oT are deferred as fill items
                        for g in range(2):
                            rzq = zp.tile([128, 4], F32, tag="rz", name=f"rz_{nb}_{hp}_{g}")
                            nc.vector.reciprocal(
                                rzq[:, :], onat[g][:, 64:260:65]
                            )
                            for il in range(2):
                                i = g * 2 + il
                                onsb = obp.tile(
                                    [128, 128], BF16, tag="onsb", name=f"onsb_{nb}_{hp}_{i}"
                                )
                                nc.vector.tensor_mul(
                                    onsb[:, :].rearrange("p (h e) -> p h e", e=64),
                                    onat[g][:, il * 130 : il * 130 + 130].rearrange(
                                        "p (h e) -> p h e", e=65
                                    )[:, :, 0:64],
                                    rzq[:, il * 2 : il * 2 + 2, None].broadcast_to([128, 2, 64]),
                                )

                                def tp_item(nb=nb, hp=hp, i=i, onsb=onsb):
                                    _lbl(f"tp{hp}_{nb}_{i}")
                                    tp = fp.tile([128, 128], BF16, tag="fill", name=f"tp_{nb}_{hp}_{i}")
                                    nc.tensor.transpose(tp[:, :], onsb[:, :], ident_sb[:, :])
                                    nc.vector.tensor_copy(
                                        out=oT[hp][:, nb * 512 + i * 128 : nb * 512 + (i + 1) * 128],
                                        in_=tp[:, :],
                                    )
                                # front of the pending queue: must drain
                                # within the next block so onat/onsb
                                # buffers recycle on time
                                fill.insert(state["pos"] + 2 * g + il, (0, None, tp_item))
                        if hp == 2 and nb == 3:
                            # last q-block: pre-accumulate head-groups 0-2 so
                            # the tail needs only one matmul + add per chain
                            for i in range(12, 16):
                                for cb in range(2):
                                    fill.append(
                                        (0, None, lambda i=i, cb=cb: emit_outproj_i(i, cb, "part"))
                                    )
                        if hp == 3:
                            # out-proj for q-block nb, one block behind
                            for i in range(4 * nb, 4 * nb + 4):
                                for cb in range(2):
                                    if nb == 3:
                                        fill.append(
                                            (0, None, lambda i=i, cb=cb: emit_outproj_i(i, cb, "fin"))
                                        )
                                    else:
                                        fill.append(
                                            (0, None, lambda i=i, cb=cb: emit_outproj_i(i, cb, "fullA"))
                                        )
                                        fill.append(
                                            (0, None, lambda i=i, cb=cb: emit_outproj_i(i, cb, "fullB"))
                                        )

                    pend = None  # carries the score->exp->PV pipeline
                    # across block boundaries
                    for hp, nb in schedule:
                        if hp > 0:
                            drain_until(("pair", hp))
                        if True:
                            if 4 * nb + 3 >= 4:
                                drain_until(("v", 4 * nb + 3))
                            # natural-layout PV accumulators: one PSUM bank
                            # per 2 q-tiles; col(i%2, h, d) = (i%2)*130+h*65+d
                            # (col 64 of each 65-group is the Z denominator)
                            onat = [
                                op.tile([128, 512], F32, tag="on", name=f"on_{nb}_{hp}_{g}")
                                for g in range(2)
                            ]
                            for j in range(4 * nb + 4):
                                if j // 4 == nb:
                                    qoff = j * 128
                                    w = 512 * (nb + 1) - qoff
                                else:
                                    qoff, w = nb * 512, 512
                                # diag tiles: score/exp only the causal width
                                # w of each head's half; qbase clamp keeps the
                                # window in-bounds at the tail (nb=3), where
                                # the causal range sits at [off2, off2+w)
                                qbase = min(qoff, N - 512)
                                off2 = qoff - qbase
                                ooff = qoff - 512 * nb
                                st = sp.tile([128, 1024], F32, tag="st", name=f"st_{nb}_{hp}_{j}")
                                _lbl(f"score{hp}_{nb}_{j}")
                                dg = j // 4 == nb
                                for hh in range(2):
                                    nc.tensor.matmul(
                                        st[:, hh * 512 + off2 : hh * 512 + off2 + w],
                                        qkT[4 + hp][hh * 64 : hh * 64 + 64, j * 128 : (j + 1) * 128],
                                        qkT[hp][hh * 64 : hh * 64 + 64, qbase + off2 : qbase + off2 + w],
                                        start=True,
                                        stop=not dg,
                                    )
                                if dg:
                                    # causal mask on PE: accumulate the 0/-1e9
                                    # triangle table through an identity lhsT
                                    # (keeps DVE out of the exp chain)
                                    for hh in range(2):
                                        nc.tensor.matmul(
                                            st[:, hh * 512 + off2 : hh * 512 + off2 + 128],
                                            ident_sb[:, :],
                                            mask_sb[:, :],
                                            start=False,
                                            stop=True,
                                            skip_group_check=True,
                                        )
                                pt = ptp.tile([128, 1024], BF16, tag="pt", name=f"pt_{nb}_{hp}_{j}")
                                if w < 512:
                                    nc.scalar.activation(
                                        pt.rearrange("p (b q) -> p b q", b=2)[:, :, off2 : off2 + w],
                                        st[:, 0:1024].rearrange("p (b q) -> p b q", b=2)[:, :, off2 : off2 + w],
                                        mybir.ActivationFunctionType.Exp,
                                    )
                                else:
                                    nc.scalar.activation(
                                        pt[:, :],
                                        st[:, :],
                                        mybir.ActivationFunctionType.Exp,
                                    )
                                # fill BEFORE the PV flush: the fill chain
                                # absorbs the exp wait instead of the PE
                                # head-of-line stalling on it
                                drain_one(0 if hp == 0 else 99)
                                if pend is not None:
                                    flush_pend(pend)
                                pend = (onat, nb, hp, j, pt, off2, w, ooff, j == 4 * nb + 3)
                    flush_pend(pend)
                    while state["pos"] < len(fill):
                        drain_one(99)
    nc.compile()
    return nc


def make_in_maps(x, Wqkv_w, Wqkv_b, out_w):
    """Host-side sharding/layout prep. Returns per-core input dicts."""
    in_maps = []
    # deinterleave perm within one head: even rope components then odd
    perm = np.concatenate([np.arange(0, D, 2), np.arange(1, D, 2)])
    # rope tables
    inv = 1.0 / (ROPE_THETA ** (np.arange(0, D, 2, dtype=np.float64) / D))
    ang = np.arange(N, dtype=np.float64)[:, None] * inv[None, :]  # [N, 32]
    cosT = np.cos(ang).T.astype(np.float32)  # [32, N]
    sinT = np.sin(ang).T.astype(np.float32)
    cosb = np.tile(cosT, (4, 1))  # [128, N]
    sinb = np.concatenate([sinT, -sinT, sinT, -sinT], axis=0)  # [128, N], block a holds out-block a^1's signed sin
    qc, kc = np.arange(128), np.arange(128)
    maskp = np.where(qc[None, :] >= kc[:, None], 0.0, NEG).astype(np.float32)
    identp = np.eye(128, dtype=np.float32)

    for c in range(8):
        b, g = c // 2, c % 2
        heads = np.arange(g * HPC, (g + 1) * HPC)
        qk_rows = (heads[:, None] * D + perm[None, :]).reshape(-1)  # [512]
        v_rows = (heads[:, None] * D + np.arange(D)[None, :]).reshape(-1)
        Wq = Wqkv_w[qk_rows] * SCALE
        bq = Wqkv_b[qk_rows] * SCALE
        Wk = Wqkv_w[C + qk_rows]
        bk = Wqkv_b[C + qk_rows]
        Wv = Wqkv_w[2 * C + v_rows]
        bv = Wqkv_b[2 * C + v_rows]
        Wcat = np.concatenate([Wq, Wk, Wv], axis=0)  # [1536, C]
        wt = np.ascontiguousarray(Wcat.T).reshape(CC, 128, 1536)
        xt = np.ascontiguousarray(x[b].T).reshape(CC, 128, N)
        bqk = np.ascontiguousarray(
            np.concatenate([bq, bk]).reshape(8, 128).T
        )  # [128, 8]
        owt = np.ascontiguousarray(out_w[:, g * JQK : (g + 1) * JQK].T).reshape(
            4, 128, C
        )
        import ml_dtypes
        in_maps.append(
            dict(
                onesp=np.ones((1, 128), dtype=np.float32),
                ones16=np.ones((128, 8), dtype=ml_dtypes.bfloat16),
                xt=xt.astype(ml_dtypes.bfloat16),
                wt=wt.astype(ml_dtypes.bfloat16),
                bqk=bqk.astype(np.float32),
                bv=np.ascontiguousarray(bv[None, :]).astype(np.float32),
                cosb=cosb.astype(ml_dtypes.bfloat16),
                sinb=sinb.astype(ml_dtypes.bfloat16),
                maskb=maskp.astype(ml_dtypes.bfloat16),
                identb=identp.astype(ml_dtypes.bfloat16),
                owt=owt.astype(ml_dtypes.bfloat16),
            )
        )
    return in_maps


_CACHED_NC = None


def kernel(x, Wqkv_w, Wqkv_b, out_w, out_b):
    from concourse.bass_utils import run_bass_kernel_spmd

    global _CACHED_NC
    x = np.asarray(x, dtype=np.float32)
    Wqkv_w = np.asarray(Wqkv_w, dtype=np.float32)
    Wqkv_b = np.asarray(Wqkv_b, dtype=np.float32)
    out_w = np.asarray(out_w, dtype=np.float32)
    out_b = np.asarray(out_b, dtype=np.float32)

    if _CACHED_NC is None:
        _CACHED_NC = build_nc()
    nc = _CACHED_NC
    in_maps = make_in_maps(x, Wqkv_w, Wqkv_b, out_w)
    res = run_bass_kernel_spmd(nc, in_maps, core_ids=list(range(8)))
    out = np.empty((B, N, C), dtype=np.float32)
    for b in range(B):
        out[b] = res.results[2 * b]["out"] + res.results[2 * b + 1]["out"] + out_b
    return out


# revision 50
# speedup vs baseline: 1.3577x; 1.0018x over previous
"""Trainium2 Bass kernel for a causal multi-head attention block
(fused QKV proj + RoPE + causal softmax attention + out proj).

Sharding: 8 cores = 4 batches x 2 head-groups (8 heads each), no
on-chip collectives: each core emits a partial out-projection [N, C]
(row-parallel over heads); the host sums each batch's pair of partials
and adds the output bias.

Schedule (single fused stream, PE never phase-barriers):
  - Prefix: chunk-major projection of q0/k0 (pair 0) across 8 PSUM
    banks while the wt/xt chunks stream in, then v tiles 0-3.
  - Attention runs head-pair-outer / q-block-inner. All remaining
    projection work (v4-15, q/k pairs 1-3, their RoPE) lives in a fill
    queue drained one item per k-tile iteration, so the PE pipeline
    stays dense while ACT's exp stream (the per-iteration clock) runs.
  - Scores S^T[k, q] for both heads of a pair row-packed into one
    [128,1024] PSUM tile; causal-trimmed on diagonal tiles; one wide
    exp -> bf16 P^T; P^T @ [v|1] accumulates o^T and the softmax
    denominator Z per head.
  - Normalization is entirely off the PE path: o_ps evacuates to SBUF
    (bf16) immediately (PSUM recycles in <1us), then DVE recip ->
    gpsimd partition-broadcast -> DVE multiply produce oT in bf16.
  - Out-proj (bf16) for q-block nb is enqueued as fill during the
    last head-pair, one block behind its norm, and drains at the tail.
Scores/PV/projections in bf16 (f32 PSUM accumulation); v-bias via a
K=1 ones-matmul; q pre-scaled by D^-0.5 on the host.
"""

import sys

sys.path.insert(0, "/opt/trn_rl_repo")

import numpy as np

import concourse.bass as bass
import concourse.mybir as mybir
from concourse import bacc, library_config
from concourse.tile import TileContext

F32 = mybir.dt.float32
F32R = mybir.dt.float32r
BF16 = mybir.dt.bfloat16

B, N, C = 4, 2048, 1024
H_ALL, D = 16, 64
HPC = 8  # heads per core
JQK = HPC * D  # 512 rows for q (and k) per core
ROPE_THETA = 10000.0
SCALE = D**-0.5
NEG = -1e9

NT = N // 128  # 16 n-tiles
NB = N // 512  # 4 n-blocks
CC = C // 128  # 8 contraction chunks


def r(ap):
    return ap.bitcast(F32R)


PE_LABELS = []
_CUR = ["?"]


def _lbl(s):
    _CUR[0] = s


def build_nc(reps=1):
    PE_LABELS.clear()
    nc = bacc.Bacc(None, target_bir_lowering=False)
    _orig_mm = nc.tensor.matmul

    def _mm(*a, **k):
        PE_LABELS.append(_CUR[0])
        return _orig_mm(*a, **k)

    nc.tensor.matmul = _mm

    xt = nc.declare_dram_parameter("xt", [CC, 128, N], BF16, isOutput=False)
    wt = nc.declare_dram_parameter("wt", [CC, 128, 1536], BF16, isOutput=False)
    bqk = nc.declare_dram_parameter("bqk", [128, 8], F32, isOutput=False)
    bv = nc.declare_dram_parameter("bv", [1, JQK], F32R, isOutput=False)
    cosb = nc.declare_dram_parameter("cosb", [128, N], BF16, isOutput=False)
    sinb = nc.declare_dram_parameter("sinb", [128, N], BF16, isOutput=False)
    maskb = nc.declare_dram_parameter("maskb", [128, 128], BF16, isOutput=False)
    identb = nc.declare_dram_parameter("identb", [128, 128], BF16, isOutput=False)
    owt = nc.declare_dram_parameter("owt", [4, 128, C], BF16, isOutput=False)
    onesp = nc.declare_dram_parameter("onesp", [1, 128], F32R, isOutput=False)
    ones16 = nc.declare_dram_parameter("ones16", [128, 8], BF16, isOutput=False)
    out = nc.declare_dram_parameter("out", [N, C], F32, isOutput=True)

    with TileContext(nc) as tc:
      for _rep in range(reps):
        with tc.tile_pool(name="persist", bufs=1) as pp:
            qkT = [pp.tile([128, N], BF16, tag=f"qkT{t}", name=f"qkT{t}") for t in range(8)]
            vN = [pp.tile([128, HPC * 65], BF16, tag=f"vN{t}", name=f"vN{t}") for t in range(NT)]
            oT = [pp.tile([128, N], BF16, tag=f"oT{t}", name=f"oT{t}") for t in range(4)]
            owt_sb = [pp.tile([128, C], BF16, tag=f"owt{hc}", name=f"owt{hc}") for hc in range(4)]
            cos_sb = pp.tile([128, N], BF16, tag="cos_sb", name="cos_sb")
            sin_sb = pp.tile([128, N], BF16, tag="sin_sb", name="sin_sb")
            mask_sb = pp.tile([128, 128], BF16, tag="mask_sb", name="mask_sb")
            ident_sb = pp.tile([128, 128], BF16, tag="ident_sb", name="ident_sb")
            bqk_sb = pp.tile([128, 8], F32, tag="bqk_sb", name="bqk_sb")
            bv_sb = pp.tile([1, JQK], F32R, tag="bv_sb", name="bv_sb")
            ones_sb = pp.tile([1, 128], F32R, tag="ones_sb", name="ones_sb")
            ones16_sb = pp.tile([128, 8], BF16, tag="ones16_sb", name="ones16_sb")
            xts = [pp.tile([128, N], BF16, tag=f"xt{cch}", name=f"xt{cch}") for cch in range(CC)]
            wt_sb = [pp.tile([128, 1536], BF16, tag=f"wt{cch}", name=f"wt{cch}") for cch in range(CC)]

            # input DMAs in consumption order: per chunk wt then two halves
            # of xt (half pieces advance the chunk-major prefix earlier
            # without blowing the serial HWDGE desc-gen budget); tables
            # after the chunks; owt last (first consumed ~80us in).
            for cch in range(CC):
                nc.sync.dma_start(out=wt_sb[cch][:, :], in_=wt[cch, :, :])
                for nbp in range(2):
                    nc.sync.dma_start(
                        out=xts[cch][:, nbp * 1024 : (nbp + 1) * 1024],
                        in_=xt[cch, :, nbp * 1024 : (nbp + 1) * 1024],
                    )
                if cch == 1:
                    nc.sync.dma_start(out=bqk_sb[:, :], in_=bqk[:, :])
                    nc.sync.dma_start(out=ones16_sb[:, :], in_=ones16[:, :])
                    nc.sync.dma_start(out=bv_sb[:, :], in_=bv[:, :])
                    nc.sync.dma_start(out=ones_sb[:, :], in_=onesp[:, :])
            nc.sync.dma_start(out=cos_sb[:, :], in_=cosb[:, :])
            nc.sync.dma_start(out=sin_sb[:, :], in_=sinb[:, :])
            nc.sync.dma_start(out=mask_sb[:, :], in_=maskb[:, :])
            nc.sync.dma_start(out=ident_sb[:, :], in_=identb[:, :])
            for hc in range(4):
                nc.sync.dma_start(out=owt_sb[hc][:, :], in_=owt[hc, :, :])

            # gpsimd: library + the ones column (col 64 of each head group)
            # for every v tile - independent of the v projections
            nc.gpsimd.load_library(library_config.attn)
            for t in range(NT):
                nc.gpsimd.tensor_copy(
                    out=vN[t][:, 64 : HPC * 65 : 65], in_=ones16_sb[:, :]
                )

            with tc.tile_pool(name="rope", bufs=2) as rp:
                sw_cache = {}

                def get_sw(jt):
                    # one sw tile per jt, shared by its rope block-items;
                    # 2 rotating buffers (jt usage windows are sequential)
                    if jt not in sw_cache:
                        sw_cache[jt] = rp.tile(
                            [128, N], BF16, tag="swf", name=f"swf{jt}", bufs=3
                        )
                    return sw_cache[jt]

                def rope_block(jt, nbp):
                    # r[a] = q[a]*cos[a] + q[a^1]*sinSigned[a] per 32-block,
                    # applied to one 512-wide n-block so the first consumer
                    # never waits on a full-row DVE chain
                    sw = get_sw(jt)
                    s = slice(nbp * 512, (nbp + 1) * 512)
                    for a in range(4):
                        b = (a ^ 1) * 32
                        nc.vector.tensor_mul(
                            sw[a * 32 : a * 32 + 32, s],
                            qkT[jt][b : b + 32, s],
                            sin_sb[b : b + 32, s],
                        )
                    nc.vector.tensor_mul(qkT[jt][:, s], qkT[jt][:, s], cos_sb[:, s])
                    nc.vector.tensor_add(qkT[jt][:, s], qkT[jt][:, s], sw[:, s])

                # ---- prefix: pair 0 (q=jt0, k=jt4) chunk-major across 8
                # PSUM banks so PE tracks the chunk DMA stream ----
                with tc.tile_pool(name="prefix_ps", bufs=1, space="PSUM") as pfx:
                    # 6 banks only: the other two stay untouched so the
                    # fill pool's first chains never wait on prefix evacs
                    pf = {
                        (jt, nbp): pfx.tile(
                            [128, 512], F32, tag=f"pf{jt}_{nbp}", name=f"pf{jt}_{nbp}"
                        )
                        for jt in (0, 4)
                        for nbp in range(3)
                    }
                    _lbl("prefix")
                    for cch in range(CC):
                        for nbp in range(3):
                            for jt in (0, 4):
                                nc.tensor.matmul(
                                    pf[(jt, nbp)][:, :],
                                    wt_sb[cch][:, jt * 128 : (jt + 1) * 128],
                                    xts[cch][:, nbp * 512 : (nbp + 1) * 512],
                                    start=(cch == 0),
                                    stop=(cch == CC - 1),
                                )
                    # all evacs first (each frees a PSUM bank; keeps the DVE
                    # queue short ahead of the v evacuations), then only the
                    # nb0 rope blocks -- the rest run after v0-3 below
                    for nbp in range(3):
                        for jt in (0, 4):
                            nc.scalar.activation(
                                qkT[jt][:, nbp * 512 : (nbp + 1) * 512],
                                pf[(jt, nbp)][:, :],
                                mybir.ActivationFunctionType.Identity,
                                bias=bqk_sb[:, jt : jt + 1],
                            )
                    # nb3 chain-major (all chunks present by now)
                    for jt in (0, 4):
                        ps3 = pfx.tile([128, 512], F32, tag="pf0_0", name=f"pf3_{jt}")
                        for cch in range(CC):
                            nc.tensor.matmul(
                                ps3[:, :],
                                wt_sb[cch][:, jt * 128 : (jt + 1) * 128],
                                xts[cch][:, 1536:2048],
                                start=(cch == 0),
                                stop=(cch == CC - 1),
                            )
                        nc.scalar.activation(
                            qkT[jt][:, 1536:2048],
                            ps3[:, :],
                            mybir.ActivationFunctionType.Identity,
                            bias=bqk_sb[:, jt : jt + 1],
                        )
                    rope_block(0, 0)
                    rope_block(4, 0)

                # ---- fused attention + fill stream ----
                with (
                    tc.tile_pool(name="attn_ps", bufs=2, space="PSUM") as sp,
                    tc.tile_pool(name="o_ps", bufs=2, space="PSUM") as op,
                    tc.tile_pool(name="fill_ps", bufs=2, space="PSUM") as fp,
                    tc.tile_pool(name="pt_pool", bufs=8) as ptp,
                    tc.tile_pool(name="znorm", bufs=4) as zp,
                    tc.tile_pool(name="onsb_pool", bufs=12) as obp,
                    tc.tile_pool(name="ostage", bufs=6) as osg,
                ):
                    # ---------------- fill queue machinery ----------------
                    v_cache = {}

                    def emit_v_half(t, half):
                        _lbl(f"fill_v{t}")
                        if half == 0:
                            v_cache[t] = fp.tile([128, 512], F32, tag="fill", name=f"psv_{t}")
                        psv = v_cache[t]
                        for cch in range(4 * half, 4 * half + 4):
                            nc.tensor.matmul(
                                psv[:, :],
                                xts[cch][:, t * 128 : (t + 1) * 128],
                                wt_sb[cch][:, 1024:1536],
                                start=(cch == 0),
                                stop=False,
                            )
                        if half == 0:
                            return
                        nc.tensor.matmul(
                            psv[:, :],
                            r(ones_sb[:, 0:128]),
                            r(bv_sb[:, :]),
                            start=False,
                            stop=True,
                        )
                        nc.scalar.copy(
                            vN[t].rearrange("p (h e) -> p h e", e=65)[:, :, 0:64],
                            psv[:, :].rearrange("p (h d) -> p h d", d=64),
                        )

                    def emit_v(t):
                        emit_v_half(t, 0)
                        emit_v_half(t, 1)

                    def emit_qk(jt, nbp):
                        _lbl(f"fill_qk{jt}_{nbp}")
                        ps = fp.tile([128, 512], F32, tag="fill", name=f"psqk_{jt}_{nbp}")
                        for cch in range(CC):
                            nc.tensor.matmul(
                                ps[:, :],
                                wt_sb[cch][:, jt * 128 : (jt + 1) * 128],
                                xts[cch][:, nbp * 512 : (nbp + 1) * 512],
                                start=(cch == 0),
                                stop=(cch == CC - 1),
                            )
                        nc.scalar.activation(
                            qkT[jt][:, nbp * 512 : (nbp + 1) * 512],
                            ps[:, :],
                            mybir.ActivationFunctionType.Identity,
                            bias=bqk_sb[:, jt : jt + 1],
                        )

                    opart = {}
                    op_cache = {}

                    def emit_outproj_i(i, cb, mode="full"):
                        # mode="part": accumulate head-groups 0-2 into SBUF
                        # early; "fin": tail does only the hc3 matmul + add;
                        # "fullA"/"fullB" split a full chain into two items
                        _lbl(f"outproj{i}_{cb}")
                        if mode == "fullA":
                            op_cache[(i, cb)] = fp.tile(
                                [128, 512], F32, tag="fill", name=f"pso_{i}_{cb}"
                            )
                            pso = op_cache[(i, cb)]
                            for hc in range(2):
                                nc.tensor.matmul(
                                    pso[:, :],
                                    oT[hc][:, i * 128 : (i + 1) * 128],
                                    owt_sb[hc][:, cb * 512 : (cb + 1) * 512],
                                    start=(hc == 0),
                                    stop=False,
                                )
                            return
                        if mode == "fullB":
                            pso = op_cache[(i, cb)]
                            for hc in range(2, 4):
                                nc.tensor.matmul(
                                    pso[:, :],
                                    oT[hc][:, i * 128 : (i + 1) * 128],
                                    owt_sb[hc][:, cb * 512 : (cb + 1) * 512],
                                    start=False,
                                    stop=(hc == 3),
                                )
                            ost = osg.tile([128, 512], F32, tag="ost", name=f"ost_{i}_{cb}")
                            nc.vector.tensor_copy(out=ost[:, :], in_=pso[:, :])
                            nc.sync.dma_start(
                                out=out[i * 128 : (i + 1) * 128, cb * 512 : (cb + 1) * 512],
                                in_=ost[:, :],
                            )
                            return
                        pso = fp.tile([128, 512], F32, tag="fill", name=f"pso_{i}_{cb}_{mode}")
                        hcs = {"full": (0, 4), "part": (0, 3), "fin": (3, 4),
                               "pAB": (0, 2), "fCD": (2, 4)}[mode]
                        for hc in range(*hcs):
                            nc.tensor.matmul(
                                pso[:, :],
                                oT[hc][:, i * 128 : (i + 1) * 128],
                                owt_sb[hc][:, cb * 512 : (cb + 1) * 512],
                                start=(hc == hcs[0]),
                                stop=(hc == hcs[1] - 1),
                            )
                        if mode in ("part", "pAB"):
                            pa = pp.tile([128, 512], F32, tag=f"opart{i % 4}_{cb}", name=f"opart_{i}_{cb}")
                            nc.vector.tensor_copy(out=pa[:, :], in_=pso[:, :])
                            opart[(i, cb)] = pa
                            return
                        ost = osg.tile([128, 512], F32, tag="ost", name=f"ost_{i}_{cb}")
                        if mode in ("fin", "fCD"):
                            nc.vector.tensor_add(ost[:, :], pso[:, :], opart[(i, cb)][:, :])
                        else:
                            nc.vector.tensor_copy(out=ost[:, :], in_=pso[:, :])
                        nc.sync.dma_start(
                            out=out[i * 128 : (i + 1) * 128, cb * 512 : (cb + 1) * 512],
                            in_=ost[:, :],
                        )

                    # v tiles 0-3 (needed by the first attention block) and
                    # the remaining pair-0 rope blocks run before attention;
                    # v evacs land early in the DVE queue
                    for t in range(4):
                        emit_v(t)
                    for nbp in range(1, NB):
                        rope_block(0, nbp)
                        rope_block(4, nbp)

                    fill = []  # (level, marker_key_or_None, emitfn)
                    qk_cache = {}

                    def emit_qk_half(jt, nbp, half):
                        # half-chains (4 chunks) share one accumulating pso
                        # bank; finer items track the exp clock more smoothly
                        _lbl(f"fill_qk{jt}_{nbp}")
                        if half == 0:
                            qk_cache[(jt, nbp)] = fp.tile(
                                [128, 512], F32, tag="fill", name=f"psqk_{jt}_{nbp}"
                            )
                        ps = qk_cache[(jt, nbp)]
                        for cch in range(4 * half, 4 * half + 4):
                            nc.tensor.matmul(
                                ps[:, :],
                                wt_sb[cch][:, jt * 128 : (jt + 1) * 128],
                                xts[cch][:, nbp * 512 : (nbp + 1) * 512],
                                start=(cch == 0),
                                stop=(cch == CC - 1),
                            )
                        if half == 1:
                            nc.scalar.activation(
                                qkT[jt][:, nbp * 512 : (nbp + 1) * 512],
                                ps[:, :],
                                mybir.ActivationFunctionType.Identity,
                                bias=bqk_sb[:, jt : jt + 1],
                            )

                    for t in range(4, NT):
                        fill.append((0, None, lambda t=t: emit_v_half(t, 0)))
                        fill.append((0, ("v", t), lambda t=t: emit_v_half(t, 1)))
                    for p in range(1, 4):
                        # qk chain for one n-block, then its rope right away
                        # (per-block items keep DVE bursts short so the
                        # mask->exp chain is never delayed long); level p-1
                        # paces pair p's chains into head-pair p-1's loop so
                        # late head-pairs keep PE fill against the exp clock
                        for jt in (p, 4 + p):
                            for nbp in range(NB):
                                fill.append(
                                    (p - 1, None, lambda jt=jt, nbp=nbp: emit_qk_half(jt, nbp, 0))
                                )
                                def qk_fin(jt=jt, nbp=nbp):
                                    emit_qk_half(jt, nbp, 1)
                                    rope_block(jt, nbp)
                                fill.append(
                                    (
                                        p - 1,
                                        ("pair", p) if (jt >= 4 and nbp == NB - 1) else None,
                                        qk_fin,
                                    )
                                )

                    state = {"pos": 0}
                    done_markers = set()

                    def drain_one(cap):
                        if state["pos"] < len(fill):
                            lev, key, fn = fill[state["pos"]]
                            if lev > cap:
                                return
                            state["pos"] += 1
                            fn()
                            if key is not None:
                                done_markers.add(key)

                    def drain_until(key):
                        if key in done_markers:
                            return
                        while state["pos"] < len(fill):
                            _lev, k, fn = fill[state["pos"]]
                            state["pos"] += 1
                            fn()
                            if k is not None:
                                done_markers.add(k)
                            if k == key:
                                return

                    def emit_pv(nb, hp, onat, pend, last):
                        pj, ppt, poff2, pw, pooff = pend
                        r0 = pooff // 128
                        for i in range(r0, 4):
                            g, il = i // 2, i % 2
                            # column of q-tile i inside the score window
                            cs = i * 128 - pooff + poff2
                            # bank g's final write happens at the diagonal
                            # j-tile that still covers q-tile g*2+1
                            for hh in range(2):
                                h = 2 * hp + hh
                                nc.tensor.matmul(
                                    onat[g][:, il * 130 + hh * 65 : il * 130 + hh * 65 + 65],
                                    ppt[:, hh * 512 + cs : hh * 512 + cs + 128],
                                    vN[pj][:, h * 65 : h * 65 + 65],
                                    start=(pj == 0 and hh == 0 and il == 0),
                                    stop=(pj == 4 * nb + 2 * g + 1 and hh == 1 and i == g * 2 + 1),
                                    skip_group_check=True,
                                )

                    # ---------------- attention ----------------
                    # pass 1: hp0 then hp1 across all blocks (projection fill
                    # drains here); pass 2 interleaves hp2/hp3 per block so
                    # each block's out-proj becomes fill right after its hp3
                    schedule = [(0, nb) for nb in range(NB)]
                    schedule += [(1, nb) for nb in range(NB)]
                    for nb in range(NB):
                        schedule += [(2, nb), (3, nb)]
                    def flush_pend(pend):
                        # PV for the pending iteration; when it closes a
                        # block, emit that block's normalization too (this
                        # runs AFTER the next block's first scores, so the
                        # exp stream never drains at block boundaries)
                        onat, nb, hp, pj, ppt, poff2, pw, pooff, is_last = pend
                        _lbl(f"pv{hp}_{nb}_{pj}")
                        emit_pv(nb, hp, onat, (pj, ppt, poff2, pw, pooff), last=is_last)
                        if not is_last:
                            return
                        # normalization in natural layout: per-partition 1/Z
                        # broadcast along free dim; the PE transposes that
                        # rebuild oT are deferred as fill items
                        for g in range(2):
                            rzq = zp.tile([128, 4], F32, tag="rz", name=f"rz_{nb}_{hp}_{g}")
                            nc.vector.reciprocal(
                                rzq[:, :], onat[g][:, 64:260:65]
                            )
                            for il in range(2):
                                i = g * 2 + il
                                onsb = obp.tile(
                                    [128, 128], BF16, tag="onsb", name=f"onsb_{nb}_{hp}_{i}"
                                )
                                nc.vector.tensor_mul(
                                    onsb[:, :].rearrange("p (h e) -> p h e", e=64),
                                    onat[g][:, il * 130 : il * 130 + 130].rearrange(
                                        "p (h e) -> p h e", e=65
                                    )[:, :, 0:64],
                                    rzq[:, il * 2 : il * 2 + 2, None].broadcast_to([128, 2, 64]),
                                )

                                def tp_item(nb=nb, hp=hp, i=i, onsb=onsb):
                                    _lbl(f"tp{hp}_{nb}_{i}")
                                    tp = fp.tile([128, 128], BF16, tag="fill", name=f"tp_{nb}_{hp}_{i}")
                                    nc.tensor.transpose(tp[:, :], onsb[:, :], ident_sb[:, :])
                                    nc.vector.tensor_copy(
                                        out=oT[hp][:, nb * 512 + i * 128 : nb * 512 + (i + 1) * 128],
                                        in_=tp[:, :],
                                    )
                                # front of the pending queue: must drain
                                # within the next block so onat/onsb
                                # buffers recycle on time
                                fill.insert(state["pos"] + 2 * g + il, (0, None, tp_item))
                        if hp == 1 and nb == 3:
                            # block-0 out-proj head-groups 0-1 become fill for
                            # the start of pass 2 (both oT halves are ready)
                            for i in range(4):
                                for cb in range(2):
                                    fill.append(
                                        (0, None, lambda i=i, cb=cb: emit_outproj_i(i, cb, "pAB"))
                                    )
                        if hp == 2 and nb == 3:
                            # last q-block: pre-accumulate head-groups 0-2 so
                            # the tail needs only one matmul + add per chain
                            for i in range(12, 16):
                                for cb in range(2):
                                    fill.append(
                                        (0, None, lambda i=i, cb=cb: emit_outproj_i(i, cb, "part"))
                                    )
                        if hp == 3:
                            # out-proj for q-block nb, one block behind
                            for i in range(4 * nb, 4 * nb + 4):
                                for cb in range(2):
                                    if nb == 3:
                                        fill.append(
                                            (0, None, lambda i=i, cb=cb: emit_outproj_i(i, cb, "fin"))
                                        )
                                    elif nb == 0:
                                        fill.append(
                                            (0, None, lambda i=i, cb=cb: emit_outproj_i(i, cb, "fCD"))
                                        )
                                    else:
                                        fill.append(
                                            (0, None, lambda i=i, cb=cb: emit_outproj_i(i, cb, "fullA"))
                                        )
                                        fill.append(
                                            (0, None, lambda i=i, cb=cb: emit_outproj_i(i, cb, "fullB"))
                                        )

                    pend = None  # carries the score->exp->PV pipeline
                    # across block boundaries
                    for hp, nb in schedule:
                        if hp > 0:
                            drain_until(("pair", hp))
                        if True:
                            if 4 * nb + 3 >= 4:
                                drain_until(("v", 4 * nb + 3))
                            # natural-layout PV accumulators: one PSUM bank
                            # per 2 q-tiles; col(i%2, h, d) = (i%2)*130+h*65+d
                            # (col 64 of each 65-group is the Z denominator)
                            onat = [
                                op.tile([128, 512], F32, tag="on", name=f"on_{nb}_{hp}_{g}")
                                for g in range(2)
                            ]
                            for j in range(4 * nb + 4):
                                if j // 4 == nb:
                                    qoff = j * 128
                                    w = 512 * (nb + 1) - qoff
                                else:
                                    qoff, w = nb * 512, 512
                                # diag tiles: score/exp only the causal width
                                # w of each head's half; qbase clamp keeps the
                                # window in-bounds at the tail (nb=3), where
                                # the causal range sits at [off2, off2+w)
                                qbase = min(qoff, N - 512)
                                off2 = qoff - qbase
                                ooff = qoff - 512 * nb
                                st = sp.tile([128, 1024], F32, tag="st", name=f"st_{nb}_{hp}_{j}")
                                _lbl(f"score{hp}_{nb}_{j}")
                                dg = j // 4 == nb
                                for hh in range(2):
                                    nc.tensor.matmul(
                                        st[:, hh * 512 + off2 : hh * 512 + off2 + w],
                                        qkT[4 + hp][hh * 64 : hh * 64 + 64, j * 128 : (j + 1) * 128],
                                        qkT[hp][hh * 64 : hh * 64 + 64, qbase + off2 : qbase + off2 + w],
                                        start=True,
                                        stop=not dg,
                                    )
                                if dg:
                                    # causal mask on PE: accumulate the 0/-1e9
                                    # triangle table through an identity lhsT
                                    # (keeps DVE out of the exp chain)
                                    for hh in range(2):
                                        nc.tensor.matmul(
                                            st[:, hh * 512 + off2 : hh * 512 + off2 + 128],
                                            ident_sb[:, :],
                                            mask_sb[:, :],
                                            start=False,
                                            stop=True,
                                            skip_group_check=True,
                                        )
                                pt = ptp.tile([128, 1024], BF16, tag="pt", name=f"pt_{nb}_{hp}_{j}")
                                if w < 512:
                                    nc.scalar.activation(
                                        pt.rearrange("p (b q) -> p b q", b=2)[:, :, off2 : off2 + w],
                                        st[:, 0:1024].rearrange("p (b q) -> p b q", b=2)[:, :, off2 : off2 + w],
                                        mybir.ActivationFunctionType.Exp,
                                    )
                                else:
                                    nc.scalar.activation(
                                        pt[:, :],
                                        st[:, :],
                                        mybir.ActivationFunctionType.Exp,
                                    )
                                # fill BEFORE the PV flush: the fill chain
                                # absorbs the exp wait instead of the PE
                                # head-of-line stalling on it
                                drain_one(0 if hp == 0 else 99)
                                if pend is not None:
                                    flush_pend(pend)
                                pend = (onat, nb, hp, j, pt, off2, w, ooff, j == 4 * nb + 3)
                    flush_pend(pend)
                    while state["pos"] < len(fill):
                        drain_one(99)
    nc.compile()
    return nc


def make_in_maps(x, Wqkv_w, Wqkv_b, out_w):
    """Host-side sharding/layout prep. Returns per-core input dicts."""
    in_maps = []
    # deinterleave perm within one head: even rope components then odd
    perm = np.concatenate([np.arange(0, D, 2), np.arange(1, D, 2)])
    # rope tables
    inv = 1.0 / (ROPE_THETA ** (np.arange(0, D, 2, dtype=np.float64) / D))
    ang = np.arange(N, dtype=np.float64)[:, None] * inv[None, :]  # [N, 32]
    cosT = np.cos(ang).T.astype(np.float32)  # [32, N]
    sinT = np.sin(ang).T.astype(np.float32)
    cosb = np.tile(cosT, (4, 1))  # [128, N]
    sinb = np.concatenate([sinT, -sinT, sinT, -sinT], axis=0)  # [128, N], block a holds out-block a^1's signed sin
    qc, kc = np.arange(128), np.arange(128)
    maskp = np.where(qc[None, :] >= kc[:, None], 0.0, NEG).astype(np.float32)
    identp = np.eye(128, dtype=np.float32)

    for c in range(8):
        b, g = c // 2, c % 2
        heads = np.arange(g * HPC, (g + 1) * HPC)
        qk_rows = (heads[:, None] * D + perm[None, :]).reshape(-1)  # [512]
        v_rows = (heads[:, None] * D + np.arange(D)[None, :]).reshape(-1)
        Wq = Wqkv_w[qk_rows] * SCALE
        bq = Wqkv_b[qk_rows] * SCALE
        Wk = Wqkv_w[C + qk_rows]
        bk = Wqkv_b[C + qk_rows]
        Wv = Wqkv_w[2 * C + v_rows]
        bv = Wqkv_b[2 * C + v_rows]
        Wcat = np.concatenate([Wq, Wk, Wv], axis=0)  # [1536, C]
        wt = np.ascontiguousarray(Wcat.T).reshape(CC, 128, 1536)
        xt = np.ascontiguousarray(x[b].T).reshape(CC, 128, N)
        bqk = np.ascontiguousarray(
            np.concatenate([bq, bk]).reshape(8, 128).T
        )  # [128, 8]
        owt = np.ascontiguousarray(out_w[:, g * JQK : (g + 1) * JQK].T).reshape(
            4, 128, C
        )
        import ml_dtypes
        in_maps.append(
            dict(
                onesp=np.ones((1, 128), dtype=np.float32),
                ones16=np.ones((128, 8), dtype=ml_dtypes.bfloat16),
                xt=xt.astype(ml_dtypes.bfloat16),
                wt=wt.astype(ml_dtypes.bfloat16),
                bqk=bqk.astype(np.float32),
                bv=np.ascontiguousarray(bv[None, :]).astype(np.float32),
                cosb=cosb.astype(ml_dtypes.bfloat16),
                sinb=sinb.astype(ml_dtypes.bfloat16),
                maskb=maskp.astype(ml_dtypes.bfloat16),
                identb=identp.astype(ml_dtypes.bfloat16),
                owt=owt.astype(ml_dtypes.bfloat16),
            )
        )
    return in_maps


_CACHED_NC = None


def kernel(x, Wqkv_w, Wqkv_b, out_w, out_b):
    from concourse.bass_utils import run_bass_kernel_spmd

    global _CACHED_NC
    x = np.asarray(x, dtype=np.float32)
    Wqkv_w = np.asarray(Wqkv_w, dtype=np.float32)
    Wqkv_b = np.asarray(Wqkv_b, dtype=np.float32)
    out_w = np.asarray(out_w, dtype=np.float32)
    out_b = np.asarray(out_b, dtype=np.float32)

    if _CACHED_NC is None:
        _CACHED_NC = build_nc()
    nc = _CACHED_NC
    in_maps = make_in_maps(x, Wqkv_w, Wqkv_b, out_w)
    res = run_bass_kernel_spmd(nc, in_maps, core_ids=list(range(8)))
    out = np.empty((B, N, C), dtype=np.float32)
    for b in range(B):
        out[b] = res.results[2 * b]["out"] + res.results[2 * b + 1]["out"] + out_b
    return out
